# revision 93
# baseline (speedup 1.0000x reference)
"""BiMPM Trainium2 Bass kernel — pure data parallel over batch (B=32 -> 4/core).

Per-core layouts (B_l=4, stack S=8 rows per step = [p:b0..3, h:b0..3]):
- token/row order: r = t*8 + s, s = seq*4 + b (seq0 = q1 = "p", seq1 = q2 = "h")
- xgT (input projections): (128 = g%128, 8 gc, 512 col=t*8+s) bf16 per dir,
  t-quartered in scan-consumption order so the ctx scan starts early
- scan: fused fw+bw per step; gates psum (128, 2dir, 8gc, 8s) in one 2KB
  bank with PER-DIR start/stop chains so each dir's sigmoid fires without
  waiting for the other's matmuls; g-gates host-prescaled x2 so ONE Sigmoid
  covers a dir's gates (tanh(g) = 2*sigmoid(2g)-1); h double-buffered and
  output copies on DVE (keeps the h-write's tanh RAW wait attached to the
  instruction instead of spilling to a SEQ-blocking EventSemaphore)
- conT f32r / conB bf16 (ctx outputs, hd-major): (128, 2c, 8s, 64t) per dir
- matching prep (csq/n1 norms, ctm transposes, rvn token norms, bld MAX
  builds) emitted in 16/32-token chunks INTERLEAVED into the ctx scan's
  engine idle time via a per-step hook; sqrt/recip finals batched post-scan
  (Sqrt shares no ACT table set with Sigmoid/Tanh — 1.3us reload each)
- matching: FULL/MAX/AM as before (MAX reduce = bf16 TT tree; AM scalar
  normalizations on ACT via per-partition scale APs; bld on idle Pool,
  dirs share one 40KB buffer)
- AX in 8-token chunk pairs (c, 7-c): products (Pool-biased 2:1) + joint
  2-channel bf16 tree-max; numerator/norm matmuls accumulate into one psum
  bank per pair; ONE batched Sqrt site per pair. Head pair (0,7) runs
  before the agg scan; mid pairs + t-chunked agg projections are emitted
  from the agg scan's per-step hook, paced so pair (c, 7-c) is fully
  emitted before scan step 8c reads its xgaT chunk (emission order IS the
  dependency order for the tile tracker — late emission = uninit reads)
- weights shipped bf16 from host (wih/whh/awhh/fc; agg proj stays f32r);
  fc head all-bf16 against bf16 hfin
- mvT (match features): 2 tiles (128, 512) f32r, feature rows at 32-aligned
  slots [full@0, max@32, am@64, ax@96, ones@116]

TimelineSim: 574376 ns (baseline 618195); HW rel err 7.4e-3 (gate 2e-2).
word_emb shipped bf16 (gather-then-round == round-then-gather: identical).
"""
import ml_dtypes
import numpy as np
from contextlib import ExitStack

BF16_NP = ml_dtypes.bfloat16

import concourse.bass as bass
import concourse.tile as tile
from concourse import bacc, mybir
from concourse.bass_utils import run_bass_kernel_spmd
from concourse.masks import make_identity

F32 = mybir.dt.float32
F32R = mybir.dt.float32r
BF16 = mybir.dt.bfloat16
I32 = mybir.dt.int32
AF = mybir.ActivationFunctionType
ALU = mybir.AluOpType
AX_X = mybir.AxisListType.X

B, T, V, D, H, L, NL = 32, 64, 50000, 300, 256, 20, 2
NCORES = 8
BL = B // NCORES
S = 2 * BL
EPS = 1e-8

_CACHE = {}
PHASES = 'full'  # 'ctx' | 'match' | 'full' (for TimelineSim bisection)


# ---------------------------------------------------------------- host prep

def _gate_reorder(w):
    # PyTorch gate order i,f,g,o -> chunk order [i, f, o, 2*g].
    # The x2 on g lets the scan use one Sigmoid for all gates:
    # tanh(g) == 2*sigmoid(2g) - 1.
    i, f, g, o = np.split(w, 4, axis=0)
    return np.concatenate([i, f, o, 2.0 * g], axis=0)


def _prep_weights(inp):
    w = {}
    f32 = np.float32

    def ctx_wT(dir_):
        # ws layout: [k%128, kc(3), gc(8), m(128)]; row 300 = bias, pad to 384
        wih = _gate_reorder(np.asarray(inp[f'ctx_wih_{dir_}'], f32))
        bias = _gate_reorder(
            np.asarray(inp[f'ctx_bih_{dir_}'] + inp[f'ctx_bhh_{dir_}'],
                       f32)[:, None]).T
        wt = np.concatenate([wih.T, bias, np.zeros((83, 1024), f32)], 0)
        return np.ascontiguousarray(
            wt.reshape(3, 128, 8, 128).transpose(1, 0, 2, 3)).astype(BF16_NP)

    def whhT(pfx, dir_):
        # ws layout: [k%128, kc, gc, m] = whh_reord[gc*128+m, kc*128+k]
        whh = _gate_reorder(np.asarray(inp[f'{pfx}_whh_{dir_}'], f32))
        return np.ascontiguousarray(
            whh.T.reshape(2, 128, 8, 128).transpose(1, 0, 2, 3)).astype(
                BF16_NP)

    w['wihT_f'], w['wihT_b'] = ctx_wT('f'), ctx_wT('b')
    w['whhT_f'], w['whhT_b'] = whhT('ctx', 'f'), whhT('ctx', 'b')
    w['awhhT_f'], w['awhhT_b'] = whhT('agg', 'f'), whhT('agg', 'b')

    def agg_wT(dir_):
        wih = _gate_reorder(np.asarray(inp[f'agg_wih_{dir_}'], f32))
        bias = _gate_reorder(
            np.asarray(inp[f'agg_bih_{dir_}'] + inp[f'agg_bhh_{dir_}'],
                       f32)[:, None]).T
        out = np.zeros((256, 1024), f32)
        for d in range(2):
            for ty in range(4):
                src = wih[:, d * 80 + ty * 20: d * 80 + ty * 20 + 20]
                out[d * 128 + 32 * ty: d * 128 + 32 * ty + 20] = src.T
        out[116] = bias[0]
        return np.ascontiguousarray(
            out.reshape(2, 128, 8, 128).transpose(1, 0, 2, 3), f32)

    w['aggwT_f'], w['aggwT_b'] = agg_wT('f'), agg_wT('b')

    # w2T80: (128 = h%128, 2 c, 2 dir, 80 = ty*20+l), ty in [full,max,am,ax]
    w2 = np.asarray(inp['mp_w'], f32) ** 2
    w2t = np.zeros((128, 2, 2, 80), f32)
    for d in range(2):
        for ty in range(4):
            src = w2[2 * ty + d]
            for c in range(2):
                w2t[:, c, d, ty * 20:(ty + 1) * 20] = \
                    src[:, c * 128:(c + 1) * 128].T
    w['w2T'] = np.ascontiguousarray(w2t)

    fc1 = np.asarray(inp['fc1_w'], f32)
    w['fc1T'] = np.ascontiguousarray(
        fc1.T.reshape(8, 128, 512).transpose(1, 0, 2)).astype(BF16_NP)
    w['fc1b'] = np.ascontiguousarray(
        np.broadcast_to(np.asarray(inp['fc1_b'], f32), (BL, 512))).astype(
            BF16_NP)
    fc2 = np.asarray(inp['fc2_w'], f32)
    w['fc2T'] = np.ascontiguousarray(
        fc2.T.reshape(4, 128, 2).transpose(1, 0, 2)).astype(BF16_NP)
    w['fc2b'] = np.ascontiguousarray(
        np.broadcast_to(np.asarray(inp['fc2_b'], f32), (BL, 2))).astype(
            BF16_NP)
    w['word_emb'] = np.ascontiguousarray(
        np.asarray(inp['word_emb'], f32)).astype(BF16_NP)
    return w


def _prep_tokens(q1, q2, core):
    q1c = np.asarray(q1[core * BL:(core + 1) * BL]).astype(np.int64)
    q2c = np.asarray(q2[core * BL:(core + 1) * BL]).astype(np.int64)
    tok = np.zeros((T * S,), np.int32)
    for seq, q in ((0, q1c), (1, q2c)):
        for b in range(BL):
            tok[np.arange(T) * S + seq * BL + b] = q[b]
    return np.ascontiguousarray(tok.reshape(4, 128))


# ---------------------------------------------------------------- build

def build_nc(debug=False):
    nc = bacc.Bacc("TRN2", target_bir_lowering=False, debug=False,
                   enable_asserts=True, num_devices=NCORES)
    dt = nc.dram_tensor
    dr = {}
    dr['tokp'] = dt("tokp", [4, 128], I32, kind="ExternalInput").ap()
    dr['word_emb'] = dt("word_emb", [V, D], BF16,
                        kind="ExternalInput").ap()
    for n, shp in [('wihT_f', [128, 3, 8, 128]), ('wihT_b', [128, 3, 8, 128]),
                   ('whhT_f', [128, 2, 8, 128]), ('whhT_b', [128, 2, 8, 128]),
                   ('awhhT_f', [128, 2, 8, 128]),
                   ('awhhT_b', [128, 2, 8, 128]),
                   ('fc1T', [128, 8, 512]), ('fc1b', [BL, 512]),
                   ('fc2T', [128, 4, 2]), ('fc2b', [BL, 2])]:
        dr[n] = dt(n, shp, BF16, kind="ExternalInput").ap()
    for n, shp in [('aggwT_f', [128, 2, 8, 128]), ('aggwT_b', [128, 2, 8, 128]),
                   ('w2T', [128, 2, 2, 80])]:
        dr[n] = dt(n, shp, F32, kind="ExternalInput").ap()
    y = dt("y", [BL, NL], F32, kind="ExternalOutput").ap()
    dbg = {}
    if debug:
        dbg['conT_f'] = dt("dbg_conT_f", [128, 2, 8, 64], F32,
                           kind="ExternalOutput").ap()
        dbg['conT_b'] = dt("dbg_conT_b", [128, 2, 8, 64], F32,
                           kind="ExternalOutput").ap()
        dbg['mvT0'] = dt("dbg_mvT0", [128, 512], F32,
                         kind="ExternalOutput").ap()
        dbg['mvT1'] = dt("dbg_mvT1", [128, 512], F32,
                         kind="ExternalOutput").ap()
        dbg['xT'] = dt("dbg_xT", [128, 2, 40], F32,
                       kind="ExternalOutput").ap()

    with tile.TileContext(nc) as tc, ExitStack() as ctx:
        _body(nc, tc, ctx, dr, y, dbg)
    nc.compile()
    return nc


def _body(nc, tc, ctx, dr, y, dbg):
    perm = ctx.enter_context(tc.tile_pool(name="perm", bufs=1))

    idf = perm.tile([128, 128], F32, name="idf")
    make_identity(nc, idf[:])
    idb = perm.tile([128, 128], BF16, name="idb")
    nc.vector.tensor_copy(idb[:], idf[:])
    selb = idb.rearrange("k (tl s) -> k tl s", s=8)

    def conv(src, dtype, name, engine=None, pool=None):
        t = (pool or perm).tile(list(src.shape), dtype, name=f"C_{name}")
        eng = engine or nc.vector
        if eng is nc.scalar:
            eng.activation(t[:], src[:], AF.Copy)
        else:
            eng.tensor_copy(t[:], src[:])
        return t

    wihT, whhTb, awhhTb, aggwT = {}, {}, {}, {}
    w2Tf = perm.tile([128, 2, 2, 80], F32, name="w2Tf")
    nc.sync.dma_start(w2Tf[:], dr['w2T'][:])
    w2Tr = conv(w2Tf, F32R, "w2Tr")
    w2Tb = conv(w2Tf, BF16, "w2Tb", nc.gpsimd)

    idx_sb = perm.tile([128, 4], I32, name="idx_sb")
    nc.sync.dma_start(idx_sb[:], dr['tokp'].rearrange("m p -> p m"))

    # ---------------- weight load + embedding gather + ctx projection (bf16)
    # xgT[d]: (128 = g%128, 8 gc, 512 cols) bf16 ; col r = t*8 + s
    # Gather tiles share scope with weight staging (no SBUF reuse between
    # the indirect-DMA writes and freed staging tiles).
    xgT = {'f': perm.tile([128, 8, 512], BF16, name="xgT_f"),
           'b': perm.tile([128, 8, 512], BF16, name="xgT_b")}
    with tc.tile_pool(name="embp", bufs=2) as embp, \
         tc.tile_pool(name="loadp", bufs=1) as loadp, \
         tc.tile_pool(name="epsum", bufs=2, space="PSUM") as epsum:
        # embT (128 = d%128, 3 kc, 512 tok) bf16
        embT = embp.tile([128, 3, 512], BF16, name="embT", tag="embT")
        embs = []
        for m in range(4):
            emb = embp.tile([128, 304], BF16, name=f"emb_{m}", tag=f"emb{m}")
            nc.gpsimd.indirect_dma_start(
                out=emb[:, 0:300], out_offset=None, in_=dr['word_emb'][:],
                in_offset=bass.IndirectOffsetOnAxis(ap=idx_sb[:, m:m + 1],
                                                    axis=0))
            nc.vector.memset(emb[:, 300:301], 1.0)
            embs.append(emb)

        def load_f32(name, shp, tag):
            t = loadp.tile(shp, F32, name=f"L_{name}", tag=tag)
            nc.sync.dma_start(t[:], dr[name][:])
            return t

        def load_bf16(name, shp, pool):
            t = pool.tile(shp, BF16, name=f"B_{name}")
            nc.sync.dma_start(t[:], dr[name][:])
            return t

        for d in 'fb':
            wihT[d] = load_bf16(f'wihT_{d}', [128, 3, 8, 128], embp)
            whhTb[d] = load_bf16(f'whhT_{d}', [128, 2, 8, 128], perm)
            awhhTb[d] = load_bf16(f'awhhT_{d}', [128, 2, 8, 128], perm)
            aggwT[d] = conv(load_f32(f'aggwT_{d}', [128, 2, 8, 128], "raw8k"),
                            F32R, f"aggw_{d}", nc.scalar)

        for m in range(4):
            embb = embs[m]
            for c in range(3):
                kc = min(128, 301 - 128 * c)
                tp = epsum.tile([128, 128], BF16, name=f"etp_{m}_{c}",
                                tag="etp")
                nc.tensor.transpose(tp[0:kc, :],
                                    embb[:, 128 * c:128 * c + kc], idb[:])
                if c % 2 == 0:
                    nc.scalar.activation(embT[0:kc, c, 128 * m:128 * (m + 1)],
                                         tp[0:kc, :], AF.Copy)
                else:
                    nc.vector.tensor_copy(
                        embT[0:kc, c, 128 * m:128 * (m + 1)], tp[0:kc, :])
        # t-quartered, scan-consumption-ordered (f ascending, b descending)
        # so the ctx scan's first steps start before the full projection
        qorder = [(0, 0), (1, 3), (0, 1), (1, 2), (0, 2), (1, 1), (0, 3),
                  (1, 0)]
        for di, q in qorder:
            d = 'fb'[di]
            for gc in range(8):
                ps = epsum.tile([128, 128], F32, name=f"xps_{d}_{gc}_{q}",
                                tag="xps")
                for c in range(3):
                    kc = min(128, 301 - 128 * c)
                    nc.tensor.matmul(ps[:], wihT[d][0:kc, c, gc, :],
                                     embT[0:kc, c, 128 * q:128 * (q + 1)],
                                     start=(c == 0), stop=(c == 2))
                if gc % 2 == 0:
                    nc.vector.tensor_copy(
                        xgT[d][:, gc, 128 * q:128 * (q + 1)], ps[:])
                else:
                    nc.scalar.activation(
                        xgT[d][:, gc, 128 * q:128 * (q + 1)], ps[:], AF.Copy)

    # ---------------- scan layer (shared ctx/agg), fused fw+bw per step
    # state h/c: (128 = hd%128, 2 dir, 2 kc, 8 s)
    # gates psum: (128 = g%128, 2 dir, 8 gc, 8 s), order [i0 i1 f0 f1 o0 o1 g0 g1]
    # g-gates pre-scaled x2 at host: tanh(g) = 2*sigmoid(2g) - 1, so one
    # Sigmoid covers all 8 chunks; xg injected via identity matmul (start=True).
    def scan_layer(xgd, whh_d, conT_out, conB_out, hfin, lname, ve=None,
                   hook=None):
        ve = ve or nc.vector
        sp = ctx2.enter_context(tc.tile_pool(name=f"sp_{lname}", bufs=12))
        pp = ctx2.enter_context(tc.tile_pool(name=f"pp_{lname}", bufs=2,
                                             space="PSUM"))
        cp = ctx2.enter_context(tc.tile_pool(name=f"cp_{lname}", bufs=1))
        c_sb = cp.tile([128, 2, 2, 8], F32, name=f"c_{lname}")
        # h double-buffered: the step-t write must not WAR against step-t's
        # own whh matmul reads (a 2-sem wait the tile framework lowers to a
        # SEQ-blocking EventSemaphore on DVE, ~200ns/step on the chain)
        h_bufs = [cp.tile([128, 2, 2, 8], BF16, name=f"h_{lname}_{i}")
                  for i in range(2)]
        nc.vector.memset(c_sb[:], 0.0)
        nc.vector.memset(h_bufs[0][:], 0.0)
        nc.vector.memset(h_bufs[1][:], 0.0)
        for tau in range(T):
            ts_ = {'f': tau, 'b': T - 1 - tau}
            h_prev = h_bufs[(tau + 1) % 2]
            h_sb = h_bufs[tau % 2]
            # one full psum bank (2KB); each dir's 1KB region runs its own
            # start/stop chain so dir f's sigmoid fires without waiting for
            # dir b's matmuls — the two cell-update chains then overlap.
            psb = pp.tile([128, 512], F32, name=f"g_{lname}_{tau}",
                          tag="gps")
            ps = psb[:, 0:128].rearrange("k (d g s) -> k d g s", d=2, g=8)
            sig = sp.tile([128, 2, 8, 8], F32, name=f"si_{lname}_{tau}",
                          tag="sig")
            t1 = sp.tile([128, 2, 2, 8], F32, name=f"t1_{lname}_{tau}",
                         tag="t1")
            t2h = sp.tile([128, 2, 2, 8], F32, name=f"t2_{lname}_{tau}",
                          tag="t2h")
            th = sp.tile([128, 2, 2, 8], F32, name=f"th_{lname}_{tau}",
                         tag="th")
            for di, d in enumerate('fb'):
                t = ts_[d]
                nc.tensor.matmul(ps[:, di, :, :], idb[:],
                                 xgd[d][:, :, 8 * t:8 * t + 8],
                                 start=True, stop=False)
                for gc in range(8):
                    for kc in range(2):
                        nc.tensor.matmul(
                            ps[:, di, gc, :], whh_d[d][:, kc, gc, :],
                            h_prev[:, di, kc, :], start=False,
                            stop=(gc == 7 and kc == 1))
                nc.scalar.activation(sig[:, di, :, :], ps[:, di, :, :],
                                     AF.Sigmoid)
            for di in range(2):
                ve.tensor_tensor(out=t1[:, di, :, :],
                                 in0=sig[:, di, 2:4, :],
                                 in1=c_sb[:, di, :, :], op=ALU.mult)
                ve.scalar_tensor_tensor(
                    out=t2h[:, di, :, :], in0=sig[:, di, 6:8, :], scalar=0.5,
                    in1=sig[:, di, 0:2, :], op0=ALU.subtract, op1=ALU.mult)
                ve.scalar_tensor_tensor(
                    out=c_sb[:, di, :, :], in0=t2h[:, di, :, :], scalar=2.0,
                    in1=t1[:, di, :, :], op0=ALU.mult, op1=ALU.add)
                nc.scalar.activation(th[:, di, :, :], c_sb[:, di, :, :],
                                     AF.Tanh)
            for di in range(2):
                ve.tensor_tensor(out=h_sb[:, di, :, :],
                                 in0=sig[:, di, 4:6, :],
                                 in1=th[:, di, :, :], op=ALU.mult)
            for di, d in enumerate('fb'):
                t = ts_[d]
                # copies on DVE: a Pool reader of h_sb would put a WAR wait
                # on the next h write, displacing its tanh RAW wait onto a
                # SEQ-blocking EventSemaphore (1-wait-per-instruction HW rule)
                if conT_out is not None:
                    nc.vector.tensor_copy(
                        conT_out[d][:, :, :, t].rearrange("k a b -> k (a b)"),
                        h_sb[:, di, :, :].rearrange("k a b -> k (a b)"))
                if conB_out is not None:
                    nc.vector.tensor_copy(
                        conB_out[d][:, :, :, t].rearrange("k a b -> k (a b)"),
                        h_sb[:, di, :, :].rearrange("k a b -> k (a b)"))
                if hfin is not None and tau == T - 1:
                    nc.vector.tensor_copy(
                        hfin[d].rearrange("k a b -> k (a b)"),
                        h_sb[:, di, :, :].rearrange("k a b -> k (a b)"))
            if hook is not None:
                hook(tau)

    conT = {'f': perm.tile([128, 2, 8, 64], F32R, name="conT_f"),
            'b': perm.tile([128, 2, 8, 64], F32R, name="conT_b")}
    conB = {'f': perm.tile([128, 2, 8, 64], BF16, name="conB_f"),
            'b': perm.tile([128, 2, 8, 64], BF16, name="conB_b")}

    # fc weights (bf16 host-prepped): plain DMAs, no staging/convert
    fcp = ctx.enter_context(tc.tile_pool(name="fcp", bufs=1))
    fc1T = fcp.tile([128, 8, 512], BF16, name="fc1T")
    fc2T = fcp.tile([128, 4, 2], BF16, name="fc2T")
    fc1b = fcp.tile([BL, 512], BF16, name="fc1b")
    fc2b = fcp.tile([BL, 2], BF16, name="fc2b")
    for nm, tgt in (('fc1T', fc1T), ('fc2T', fc2T), ('fc1b', fc1b),
                    ('fc2b', fc2b)):
        nc.sync.dma_start(tgt[:], dr[nm][:])

    # matching prep interleaved into the ctx scan's engine idle time;
    # (dir, quarter) becomes ready as the scan's two fronts advance
    prep_ps_stack = ExitStack()
    bld_stack = ExitStack()
    prep = _make_prep(nc, tc, ctx, prep_ps_stack, bld_stack, conT, conB,
                      w2Tr, w2Tf, idb)
    pq = []
    for qi, (fq, bq) in enumerate(((0, 3), (1, 2), (2, 1))):
        rt = 16 * (qi + 1) - 1
        for cl in prep['units']('f', fq):
            pq.append((rt, cl))
        for cl in prep['units']('b', bq):
            pq.append((rt, cl))
        if qi == 1:
            for cl in prep['halves']('f', 0):
                pq.append((31, cl))
            for cl in prep['halves']('b', 1):
                pq.append((31, cl))
            for cl in prep['bld_units']('f', 0, 32):
                pq.append((31, cl))
    ppos = [0]

    def ctx_hook(tau):
        n = 0
        while ppos[0] < len(pq) and n < 2:
            rt, cl = pq[ppos[0]]
            if rt > tau:
                break
            cl()
            ppos[0] += 1
            n += 1

    with ExitStack() as ctx2:
        scan_layer(xgT, whhTb, conT, conB, None, "ctx", hook=ctx_hook)
    while ppos[0] < len(pq):
        pq[ppos[0]][1]()
        ppos[0] += 1
    for cl in prep['units']('f', 3):
        cl()
    for cl in prep['units']('b', 0):
        cl()
    for cl in prep['halves']('f', 1):
        cl()
    for cl in prep['halves']('b', 0):
        cl()
    prep['finals']()
    prep_ps_stack.close()
    for cl in prep['bld_units']('f', 32, 64, mix=True):
        cl()

    if PHASES == 'ctx':
        y_sb0 = perm.tile([BL, NL], F32, name="y_sb0")
        nc.vector.tensor_copy(y_sb0[:], conT['f'][0:BL, 0, 0, 0:NL])
        nc.sync.dma_start(y[:], y_sb0[:])
        return

    # ---------------- matching
    mvT = [perm.tile([128, 512], F32R, name="mvT0"),
           perm.tile([128, 512], F32R, name="mvT1")]
    # f32r memset unsupported; fill via ACT copy with scale=0 (+bias)
    fill_src = bass.AP(tensor=idf.tensor, offset=idf.offset,
                       ap=[idf.ap[0], [0, 512]])
    nc.scalar.activation(mvT[0][:], fill_src, AF.Copy, bias=0.0, scale=0.0)
    nc.scalar.activation(mvT[1][:], fill_src, AF.Copy, bias=0.0, scale=0.0)
    nc.scalar.activation(mvT[0][96:128, :],
                         bass.AP(tensor=idf.tensor, offset=idf.offset,
                                 ap=[[idf.ap[0][0], 32], [0, 512]]),
                         AF.Copy, bias=1.0, scale=0.0)
    mctx = _matching(nc, tc, ctx, conT, conB, w2Tr, w2Tf, w2Tb, mvT,
                     idf, idb, prep)
    bld_stack.close()

    pipe_stack = ExitStack()
    ctx.enter_context(pipe_stack)
    ax_unit, ax_tail = _make_ax_emit(nc, tc, pipe_stack, conB, mctx['n1s'],
                                     w2Tb, mvT, mctx)

    def ax_pair(pair, during=False):
        for ch in pair:
            for d in 'fb':
                for role in range(2):
                    for b in range(BL):
                        ax_unit(d, b, role, ch, pair, during)
        ax_tail(pair)

    if PHASES == 'match':
        for pair in ((0, 7), (1, 6), (2, 5), (3, 4)):
            ax_pair(pair)
        y_sb0 = perm.tile([BL, NL], F32, name="y_sb0")
        nc.vector.tensor_copy(y_sb0[:], mvT[0][0:BL, 0:NL])
        nc.sync.dma_start(y[:], y_sb0[:])
        return

    # ---------------- AX + agg projection pipelined under the agg scan.
    # The agg scan consumes xgaT cols from both ends inward (fw t=tau,
    # bw t=63-tau), in 8-token chunks: chunk pair (c, 7-c) is needed at
    # scan step 8c. Chunks 0/7 (plus their AX features) are computed
    # before the scan; the middle chunks' AX units + projections are
    # emitted from the scan's per-step hook so they execute in engine
    # idle time.
    xgaT = {'f': perm.tile([128, 8, 512], BF16, name="xgaT_f"),
            'b': perm.tile([128, 8, 512], BF16, name="xgaT_b")}
    ap_ps = pipe_stack.enter_context(tc.tile_pool(name="aggps", bufs=2,
                                                  space="PSUM"))

    def proj_chunk(c):
        c0 = 64 * c
        for di, d in enumerate('fb'):
            for gc in range(8):
                ps = ap_ps.tile([128, 64], F32, name=f"ap_{d}_{gc}_{c}",
                                tag="aps")
                for kc in range(2):
                    nc.tensor.matmul(ps[:], aggwT[d][:, kc, gc, :],
                                     mvT[kc][:, c0:c0 + 64],
                                     start=(kc == 0), stop=(kc == 1))
                nc.scalar.activation(xgaT[d][:, gc, c0:c0 + 64], ps[:],
                                     AF.Copy)

    ax_pair((0, 7))
    proj_chunk(0)
    proj_chunk(7)

    def tail_proj(pair):
        ax_tail(pair)
        proj_chunk(pair[0])
        proj_chunk(pair[1])

    units = []
    for cpair in ((1, 6), (2, 5), (3, 4)):
        for c in cpair:
            for d in 'fb':
                for role in range(2):
                    for b in range(BL):
                        units.append((ax_unit, d, b, role, c, cpair, True))
        units.append((tail_proj, cpair))
    qpos = [0]

    def agg_hook(tau):
        # EMISSION-ORDER CORRECTNESS: the tile tracker only sees deps from
        # writes emitted BEFORE a read. Chunk pair k (chunks k, 7-k) is read
        # by scan step 8k, so its units+projection must be fully emitted
        # strictly before that step's instructions. Pace linearly to each
        # deadline (~4.2 units/step through step 21).
        target = min(len(units), (tau + 3) * len(units) // 25 + 1)
        while qpos[0] < target:
            u = units[qpos[0]]
            qpos[0] += 1
            u[0](*u[1:])

    # ---------------- agg scans + fc
    hfin = {d: perm.tile([128, 2, 8], BF16, name=f"hfin_{d}") for d in 'fb'}
    with ExitStack() as ctx2:
        scan_layer(xgaT, awhhTb, None, None, hfin, "agg", hook=agg_hook)
    assert qpos[0] >= len(units)
    pipe_stack.close()
    fps = ctx.enter_context(tc.tile_pool(name="fcps", bufs=1, space="PSUM"))

    # x k-chunks: [hpf c0, hpf c1, hpb c0, hpb c1, hhf c0, hhf c1, hhb c0, hhb c1]
    ksl = []
    for role0 in (0, 4):
        for d in 'fb':
            for c in range(2):
                ksl.append(hfin[d][:, c, role0:role0 + BL])
    x1 = fps.tile([BL, 512], F32, name="x1")
    for kc in range(8):
        nc.tensor.matmul(x1[:], ksl[kc], fc1T[:, kc, :],
                         start=(kc == 0), stop=False)
    nc.tensor.matmul(x1[:], idb[0:BL, 0:BL], fc1b[:], start=False, stop=True)
    xt1 = fcp.tile([BL, 512], F32, name="xt1")
    nc.scalar.activation(xt1[:], x1[:], AF.Tanh)
    xt1ps = fps.tile([128, 4, BL], F32, name="xt1ps")
    for c in range(4):
        nc.tensor.transpose(xt1ps[:, c, :], xt1[:, 128 * c:128 * (c + 1)],
                            idf[0:BL, 0:BL])
    xt1T = fcp.tile([128, 4, BL], BF16, name="xt1T")
    nc.vector.tensor_copy(xt1T[:], xt1ps[:])
    yps = fps.tile([BL, NL], F32, name="yps")
    for c in range(4):
        nc.tensor.matmul(yps[:], xt1T[:, c, :], fc2T[:, c, :],
                         start=(c == 0), stop=False)
    nc.tensor.matmul(yps[:], idb[0:BL, 0:BL], fc2b[:], start=False,
                     stop=True)
    y_sb = fcp.tile([BL, NL], F32, name="y_sb")
    nc.vector.tensor_copy(y_sb[:], yps[:])
    nc.sync.dma_start(y[:], y_sb[:])

    if dbg:
      with tc.tile_pool(name="dbgp", bufs=1) as dbp:
        for d in 'fb':
            cf = dbp.tile([128, 2, 8, 64], F32, name=f"dbgc_{d}")
            nc.scalar.activation(cf[:], conT[d][:], AF.Copy)
            nc.sync.dma_start(dbg[f'conT_{d}'][:], cf[:])
        for i in range(2):
            mf = dbp.tile([128, 512], F32, name=f"dbgm_{i}")
            nc.scalar.activation(mf[:], mvT[i][:], AF.Copy)
            nc.sync.dma_start(dbg[f'mvT{i}'][:], mf[:])
        xtd = dbp.tile([128, 2, 40], F32, name="xtd")
        nc.vector.memset(xtd[:], 0.0)
        nc.vector.tensor_copy(xtd[:, :, 0:8], hfin['f'][:])
        nc.vector.tensor_copy(xtd[:, :, 32:40], hfin['b'][:])
        nc.sync.dma_start(dbg['xT'][:], xtd[:])


# ---------------------------------------------------------------- matching
# ---------------------------------------------------------------- matching

def _make_prep(nc, tc, ctx, psum_stack, bld_stack, conT, conB, w2r, w2f,
               idb):
    """Matching prep (norms / t-major transposes / per-token norms / MAX
    builds), emitted in 16-token quarters so most of it runs in engine
    idle time during the ctx scan. Sqrt/recip finals are batched post-scan
    (Sqrt shares no ACT table set with the scan's Sigmoid/Tanh; scattering
    them through the scan would pay 1.3us table reloads each). bld goes to
    the otherwise-idle Pool engine; the two dirs share one 40KB buffer
    (tag rotation serializes b's builds behind f's MAX reads).
    """
    prep = ctx.enter_context(tc.tile_pool(name="prep", bufs=1))
    bldp = bld_stack.enter_context(tc.tile_pool(name="bldp", bufs=1))
    n1sqp = psum_stack.enter_context(tc.tile_pool(name="n1sqp", bufs=1))
    prepps = psum_stack.enter_context(tc.tile_pool(name="prepps", bufs=2,
                                                   space="PSUM"))
    t = {}
    for d in 'fb':
        t[f'n1sq_{d}'] = n1sqp.tile([20, 4, 8, 64], F32, name=f"n1sq_{d}")
        t[f'n1_{d}'] = prep.tile([20, 4, 8, 64], F32, name=f"n1_{d}")
        t[f'rn1_{d}'] = prep.tile([20, 4, 8, 64], F32, name=f"rn1_{d}")
        t[f'ctm_{d}'] = prep.tile([64, 8, 256], BF16, name=f"ctm_{d}")
        t[f'rvn_{d}'] = prep.tile([64, 8], F32, name=f"rvn_{d}")
    bldt = {}

    def bld_tile(d):
        if d not in bldt:
            bldt[d] = bldp.tile([128, 2, 20, 8, 64], BF16, name=f"bld_{d}",
                                tag="bld", bufs=1)
        return bldt[d]

    def units(d, q):
        di = 0 if d == 'f' else 1
        cT, cB = conT[d], conB[d]
        q0 = 16 * q
        n1sq, ctm, rvn = t[f'n1sq_{d}'], t[f'ctm_{d}'], t[f'rvn_{d}']

        def u_norm():
            csq = prep.tile([128, 2, 8, 16], F32R, name=f"csq_{d}_{q}",
                            tag="csq", bufs=3)
            nc.scalar.activation(csq[:], cT[:, :, :, q0:q0 + 16], AF.Square)
            n1q = prepps.tile([20, 4, 8, 16], F32, name=f"n1q_{d}_{q}",
                              tag="n1q", bufs=2)
            for ty in range(4):
                for c in range(2):
                    nc.tensor.matmul(n1q[:, ty, :, :],
                                     w2r[:, c, di, 20 * ty:20 * ty + 20],
                                     csq[:, c, :, :],
                                     start=(c == 0), stop=(c == 1))
            nc.vector.tensor_copy(n1sq[:, :, :, q0:q0 + 16], n1q[:])

        return [u_norm]

    def halves(d, h):
        # engine partition accesses must be 32-aligned, so the t-major
        # transposes and per-token norms go by 32-token halves
        cB = conB[d]
        h0 = 32 * h
        ctm, rvn = t[f'ctm_{d}'], t[f'rvn_{d}']

        def u_ctm(s0):
            for s in range(s0, s0 + 2):
                tp = prepps.tile([32, 2, 128], BF16, name=f"ct_{d}_{h}_{s}",
                                 tag="ctp", bufs=2)
                for c in range(2):
                    nc.tensor.transpose(tp[:, c, :], cB[:, c, s, h0:h0 + 32],
                                        idb[:])
                if s % 2 == 0:
                    nc.scalar.activation(ctm[h0:h0 + 32, s, :],
                                         tp.rearrange("t c k -> t (c k)"),
                                         AF.Copy)
                else:
                    nc.vector.tensor_copy(ctm[h0:h0 + 32, s, :],
                                          tp.rearrange("t c k -> t (c k)"))

        def u_rvn(s0):
            # bf16 out scratch keeps the STT in 4x DVE perf mode; the f32
            # accum_out (exempt scalar operand) carries the precision
            for s in range(s0, s0 + 4):
                scr = prep.tile([32, 256], BF16, name=f"rs_{d}_{h}_{s}",
                                tag="rvs", bufs=2)
                nc.vector.scalar_tensor_tensor(
                    out=scr[:], in0=ctm[h0:h0 + 32, s, :], scalar=1.0,
                    in1=ctm[h0:h0 + 32, s, :], op0=ALU.mult, op1=ALU.mult,
                    accum_out=rvn[h0:h0 + 32, s:s + 1])

        return [lambda s0=s0: u_ctm(s0) for s0 in range(0, 8, 2)] + \
               [lambda: u_rvn(0), lambda: u_rvn(4)]

    def bld_units(d, tlo, thi, mix=False):
        # mix=True (post-scan): mostly DVE — the bf16 tensor_scalar hits
        # the 4x perf mode (~190ns vs ~800ns Pool); Pool-only when
        # interleaved under the ctx scan where DVE is contended
        di = 0 if d == 'f' else 1
        cB = conB[d]
        bld = bld_tile(d)
        out = []
        for c in range(2):
            for l0 in range(0, L, 2):
                def cl(c=c, l0=l0):
                    for l in range(l0, l0 + 2):
                        eng = (nc.vector if mix and l % 4 != 3
                               else nc.gpsimd)
                        eng.tensor_scalar_mul(
                            bld[:, c, l, :, tlo:thi], cB[:, c, :, tlo:thi],
                            w2f[:, c, di, 20 + l:21 + l])
                out.append(cl)
        return out

    def finals():
        # one sqrt-table residency for all four batched Sqrts
        for d in 'fb':
            nc.scalar.activation(
                t[f'n1_{d}'].rearrange("l y s t -> l (y s t)"),
                t[f'n1sq_{d}'].rearrange("l y s t -> l (y s t)"), AF.Sqrt)
            nc.scalar.activation(t[f'rvn_{d}'][:], t[f'rvn_{d}'][:], AF.Sqrt)
        for d in 'fb':
            n1, rn1 = t[f'n1_{d}'], t[f'rn1_{d}']
            nc.vector.tensor_scalar_max(
                rn1.rearrange("l y s t -> l (y s t)"),
                n1.rearrange("l y s t -> l (y s t)"), EPS)
            nc.vector.reciprocal(rn1.rearrange("l y s t -> l (y s t)"),
                                 rn1.rearrange("l y s t -> l (y s t)"))
            rvn = t[f'rvn_{d}']
            nc.vector.tensor_scalar_max(rvn[:], rvn[:], EPS)
            nc.vector.reciprocal(rvn[:], rvn[:])

    return {'t': t, 'units': units, 'halves': halves,
            'bld_units': bld_units, 'bld_tile': bld_tile, 'finals': finals}


def _matching(nc, tc, ctx, conT, conB, w2r, w2f, w2b, mvT, idf, idb,
              prep):
    stage, n1s = {}, {}
    # feature-type offsets into w2 cols (ty*20) and mv row slots (ty*32)
    # greedy DVE/Pool balancer: Pool runs TT ~3.9x slower than DVE-2x
    # pool pre-charged: Pool's 8us products block their dependent DVE
    # tree stages, so bias assignment away from Pool (swept optimum)
    rot = {'dve': 0.0, 'pool': 30.0}

    def veng(cost=1.0):
        if rot['dve'] + cost <= rot['pool'] + 3.3 * cost:
            rot['dve'] += cost
            return nc.vector
        rot['pool'] += 3.3 * cost
        return nc.gpsimd

    dramp = ctx.enter_context(tc.tile_pool(name="mdram", bufs=1,
                                           space="DRAM"))

    def mcol(mt, slot, ri, b):
        # (20, 64) view of mvT rows [slot:slot+20], cols 8t + ri*4 + b
        return mt[slot:slot + 20, :].rearrange("l (t s) -> l t s",
                                               s=8)[:, :, ri * BL + b]

    for di, d in enumerate('fb'):
        cT, cB = conT[d], conB[d]
        anchor_t = (T - 1) if d == 'f' else 0
        mt = mvT[di]
        n1 = prep['t'][f'n1_{d}']
        rn1 = prep['t'][f'rn1_{d}']
        ctm = prep['t'][f'ctm_{d}']
        rvn = prep['t'][f'rvn_{d}']

        with tc.tile_pool(name=f"mn_{d}", bufs=1) as mn:
          with tc.tile_pool(name=f"mnp_{d}", bufs=2, space="PSUM") as mnp:
            n1s[d] = n1
            # ---- FULL
            ancv = mn.tile([128, 2, 8], F32, name=f"ancv_{d}", tag="ancv")
            nc.vector.tensor_copy(ancv[:], cT[:, :, :, anchor_t])
            for b in range(BL):
                for ri, (s_me, s_an) in enumerate(((b, BL + b), (BL + b, b))):
                    anc = mn.tile([128, 2, 20], BF16, name=f"an_{d}_{b}_{ri}",
                                  tag="anc", bufs=2)
                    for c in range(2):
                        nc.vector.tensor_scalar_mul(
                            anc[:, c, :], w2b[:, c, di, 0:20],
                            ancv[:, c, s_an:s_an + 1])
                    nps = mnp.tile([20, 64], F32, name=f"nf_{d}_{b}_{ri}",
                                   tag="nf")
                    for c in range(2):
                        nc.tensor.matmul(nps[:], anc[:, c, :],
                                         cB[:, c, s_me, :],
                                         start=(c == 0), stop=(c == 1))
                    den = mn.tile([20, 64], F32, name=f"de_{d}_{b}_{ri}",
                                  tag="den", bufs=2)
                    nc.vector.tensor_scalar(
                        out=den[:], in0=n1[:, 0, s_me, :],
                        scalar1=n1[:, 0, s_an, anchor_t:anchor_t + 1],
                        scalar2=EPS, op0=ALU.mult, op1=ALU.max)
                    nc.vector.reciprocal(den[:], den[:])
                    nc.vector.tensor_tensor(out=mcol(mt, 0, ri, b),
                                            in0=nps[:], in1=den[:],
                                            op=ALU.mult)

          # ---- MAX (pair max over the other sequence)
          with tc.tile_pool(name=f"mx_{d}", bufs=1) as mxp, \
               tc.tile_pool(name=f"mxps_{d}", bufs=1, space="PSUM") as mxps:
              bld = prep['bld_tile'](d)
              # stage MAX-type recip norms to DRAM (bf16) for broadcasts
              rnb = mxp.tile([20, 8, 64], BF16, name=f"rnb_{d}", tag="rnb")
              nc.vector.tensor_copy(rnb.rearrange("l s t -> l (s t)"),
                                    rn1[:, 1, :, :].rearrange(
                                        "l s t -> l (s t)"))
              rnd = dramp.tile([20, 8, 64], BF16, name=f"rnd_{d}")
              nc.sync.dma_start(rnd[:], rnb[:])
              mxs_all = {}
              for bp in range(2):
                  for side in range(2):
                      rs_me = 2 * bp if side == 0 else 4 + 2 * bp
                      rs_ot = 4 + 2 * bp if side == 0 else 2 * bp
                      for hf in range(2):
                          pps = mxps.tile([128, 10, 128], F32,
                                          name=f"pp_{d}_{bp}_{side}_{hf}",
                                          tag="pps", bufs=2)
                          for u in range(10):
                              l = 10 * hf + u
                              for c in range(2):
                                  nc.tensor.matmul(
                                      pps[:, u, :],
                                      bld[:, c, l, rs_me:rs_me + 2,
                                          :].rearrange("k e t -> k (e t)"),
                                      cB[:, c, rs_ot:rs_ot + 2,
                                         :].rearrange("k e t -> k (e t)"),
                                      start=(c == 0), stop=(c == 1))
                          for b2 in range(2):
                              b = 2 * bp + b2
                              s_ot = rs_ot + b2
                              key = (side, b)
                              if key not in mxs_all:
                                  mxs_all[key] = mxp.tile(
                                      [64, 2, 10], F32,
                                      name=f"mxs_{d}_{side}_{b}",
                                      tag=f"mxs_{side}_{b2}")
                              nbcb = mxp.tile([64, 10, 64], BF16,
                                              name=f"nb_{d}_{bp}_{side}"
                                                   f"_{hf}_{b2}",
                                              tag="nbcb", bufs=3)
                              nc.sync.dma_start(
                                  nbcb[:],
                                  bass.AP(tensor=rnd.tensor,
                                          offset=rnd.offset
                                          + (10 * hf) * 512 + s_ot * 64,
                                          ap=[[0, 64], [512, 10], [1, 64]]))
                              # stage pps to SBUF bf16 on the idle ACT so
                              # the multiply runs 2x from SBUF instead of
                              # 1x from f32 psum (791ns -> ~390ns on DVE)
                              ppsc = mxp.tile([64, 10, 64], BF16,
                                              name=f"pc_{d}_{bp}_{side}"
                                                   f"_{hf}_{b2}",
                                              tag="ppsc", bufs=3)
                              nc.scalar.activation(
                                  ppsc[:],
                                  pps[64 * b2:64 * b2 + 64, :,
                                      64 * b2:64 * b2 + 64], AF.Copy)
                              pn = mxp.tile([64, 10, 64], BF16,
                                            name=f"pn_{d}_{bp}_{side}"
                                                 f"_{hf}_{b2}",
                                            tag="pn", bufs=3)
                              rot['dve'] += 0.4
                              nc.vector.tensor_tensor(
                                  out=pn[:], in0=ppsc[:],
                                  in1=nbcb[:], op=ALU.mult)
                              # bf16 TT tree-max (2x DVE) beats the 1x
                              # tensor_reduce on 640-elem tiles
                              cur = pn
                              for w in (32, 16, 8, 4, 2):
                                  nxt = mxp.tile(
                                      [64, 10, w], BF16,
                                      name=f"mt_{d}_{bp}_{side}"
                                           f"_{hf}_{b2}_{w}",
                                      tag=f"mt{w}", bufs=2)
                                  nc.vector.tensor_tensor(
                                      out=nxt[:], in0=cur[:, :, 0:w],
                                      in1=cur[:, :, w:2 * w], op=ALU.max)
                                  cur = nxt
                              nc.vector.tensor_tensor(
                                  out=mxs_all[key][:, hf, :],
                                  in0=cur[:, :, 0:1].rearrange(
                                      "t u o -> t (u o)"),
                                  in1=cur[:, :, 1:2].rearrange(
                                      "t u o -> t (u o)"),
                                  op=ALU.max)
              for side in range(2):
                  for b in range(BL):
                      yt = mxps.tile([20, 64], F32,
                                     name=f"yt_{d}_{b}_{side}", tag="yt",
                                     bufs=2)
                      nc.tensor.transpose(
                          yt[:],
                          mxs_all[(side, b)].rearrange(
                              "t hf u -> t (hf u)"),
                          idf[0:64, 0:64])
                      ri_me = 0 if side == 0 else 1
                      s_me = b if side == 0 else BL + b
                      nc.vector.tensor_tensor(
                          out=mcol(mt, 32, ri_me, b), in0=yt[:],
                          in1=rn1[:, 1, s_me, :], op=ALU.mult)
          if d == 'f':
              # dir b's MAX builds now: Pool is idle while DVE chews on
              # dir f's AM blocks; the shared bld buffer (tag bufs=1)
              # WARs behind f's pps reads automatically
              for cl in prep['bld_units']('b', 0, 64, mix=True):
                  cl()
          # ---- AM + AX per batch item
          with tc.tile_pool(name=f"am_{d}", bufs=3) as amp, \
               tc.tile_pool(name=f"amps_{d}", bufs=2, space="PSUM") as amps:
              for b in range(BL):
                  _am_ax_block(nc, tc, d, di, b, cT, cB, w2b, n1, rn1,
                               rvn, ctm, mt, idf, idb, amp, amps, dramp,
                               mcol, anchor_t, veng, stage)



    return {'stage': stage, 'n1s': n1s, 'rot': rot, 'veng': veng,
            'mcol': mcol, 'dramp': dramp}


def _am_ax_block(nc, tc, d, di, b, cT, cB, w2b, n1, rn1, rvn, ctm, mt, idf,
                 idb, amp, amps, dramp, mcol, anchor_t, veng, stage):
    AM_SLOT, AX_SLOT = 64, 96
    sp, sh = b, BL + b

    # raw attention + normalization (attn = rvn_p[i] * raw * rvn_h[j])
    att_ps = amps.tile([64, 64], F32, name=f"at_{d}_{b}", tag="t64", bufs=3)
    for c in range(2):
        nc.tensor.matmul(att_ps[:], cB[:, c, sp, :], cB[:, c, sh, :],
                         start=(c == 0), stop=(c == 1))
    a1 = amp.tile([64, 64], F32, name=f"a1_{d}_{b}", tag="a1")
    nc.scalar.activation(a1[:], att_ps[:], AF.Copy, scale=rvn[:, sp:sp + 1])
    a1t_ps = amps.tile([64, 64], F32, name=f"a1t_{d}_{b}", tag="t64", bufs=3)
    nc.tensor.transpose(a1t_ps[:], a1[:], idf[0:64, 0:64])
    attTn = amp.tile([64, 64], F32, name=f"aTn_{d}_{b}", tag="attTn")
    nc.scalar.activation(attTn[:], a1t_ps[:], AF.Copy,
                         scale=rvn[:, sh:sh + 1])
    attn_ps = amps.tile([64, 64], F32, name=f"an2_{d}_{b}", tag="t64", bufs=3)
    nc.tensor.transpose(attn_ps[:], attTn[:], idf[0:64, 0:64])
    attn = amp.tile([64, 64], F32, name=f"an_{d}_{b}", tag="attn")
    nc.scalar.activation(attn[:], attn_ps[:], AF.Copy)

    # row sums + clamped recips
    rs_h = amp.tile([64, 1], F32, name=f"rh_{d}_{b}", tag="rsh")
    nc.vector.tensor_reduce(out=rs_h[:], in_=attn[:], axis=AX_X, op=ALU.add)
    nc.vector.tensor_scalar_max(rs_h[:], rs_h[:], EPS)
    nc.vector.reciprocal(rs_h[:], rs_h[:])
    rs_p = amp.tile([64, 1], F32, name=f"rp_{d}_{b}", tag="rsp")
    nc.vector.tensor_reduce(out=rs_p[:], in_=attTn[:], axis=AX_X, op=ALU.add)
    nc.vector.tensor_scalar_max(rs_p[:], rs_p[:], EPS)
    nc.vector.reciprocal(rs_p[:], rs_p[:])

    # weighted mean rhs: ahT = T(attn * rs_h) bf16, bpT = T(attTn * rs_p)
    ah = amp.tile([64, 64], F32, name=f"ah_{d}_{b}", tag="ah")
    nc.scalar.activation(ah[:], attn[:], AF.Copy, scale=rs_h[:, 0:1])
    ahT_ps = amps.tile([64, 64], F32, name=f"ahT_{d}_{b}", tag="t64", bufs=3)
    nc.tensor.transpose(ahT_ps[:], ah[:], idf[0:64, 0:64])
    ahT = amp.tile([64, 64], BF16, name=f"ahTs_{d}_{b}", tag="ahTs")
    nc.scalar.activation(ahT[:], ahT_ps[:], AF.Copy)
    bp_ = amp.tile([64, 64], F32, name=f"bp_{d}_{b}", tag="bp")
    nc.scalar.activation(bp_[:], attTn[:], AF.Copy, scale=rs_p[:, 0:1])
    bpT_ps = amps.tile([64, 64], F32, name=f"bpT_{d}_{b}", tag="t64", bufs=3)
    nc.tensor.transpose(bpT_ps[:], bp_[:], idf[0:64, 0:64])
    bpT = amp.tile([64, 64], BF16, name=f"bpTs_{d}_{b}", tag="bpTs")
    nc.scalar.activation(bpT[:], bpT_ps[:], AF.Copy)

    # am vectors + cosine under w_am
    for role, (rhs, s_ctm, s_me) in enumerate(
            ((ahT, sh, sp), (bpT, sp, sh))):
        amv_ps = amps.tile([128, 2, 64], F32, name=f"av_{d}_{b}_{role}",
                           tag="amv", bufs=2)
        for c in range(2):
            nc.tensor.matmul(amv_ps[:, c, :],
                             ctm[:, s_ctm, 128 * c:128 * (c + 1)], rhs[:],
                             start=True, stop=True)
        amv = amp.tile([128, 2, 64], F32R, name=f"am_{d}_{b}_{role}",
                       tag="amv_sb")
        nc.scalar.activation(amv.rearrange("k c t -> k (c t)"),
                             amv_ps.rearrange("k c t -> k (c t)"), AF.Copy)
        prod = amp.tile([128, 2, 64], BF16, name=f"pr_{d}_{b}_{role}",
                        tag="prod")
        for c in range(2):
            nc.vector.tensor_tensor(out=prod[:, c, :], in0=cB[:, c, s_me, :],
                                    in1=amv[:, c, :], op=ALU.mult)
        nump = amps.tile([20, 64], F32, name=f"nu_{d}_{b}_{role}", tag="s20",
                         bufs=2)
        for c in range(2):
            nc.tensor.matmul(nump[:], w2b[:, c, di, 40:60],
                             prod[:, c, :], start=(c == 0), stop=(c == 1))
        amsq = amp.tile([128, 2, 64], BF16, name=f"as_{d}_{b}_{role}",
                        tag="amsq")
        nc.scalar.activation(amsq.rearrange("k c t -> k (c t)"),
                             amv.rearrange("k c t -> k (c t)"), AF.Square)
        n2p = amps.tile([20, 64], F32, name=f"n2_{d}_{b}_{role}", tag="s20",
                        bufs=2)
        for c in range(2):
            nc.tensor.matmul(n2p[:], w2b[:, c, di, 40:60],
                             amsq[:, c, :], start=(c == 0), stop=(c == 1))
        n2s = amp.tile([20, 64], F32, name=f"ns_{d}_{b}_{role}", tag="n2s")
        nc.scalar.activation(n2s[:], n2p[:], AF.Sqrt)
        den = amp.tile([20, 64], F32, name=f"dn_{d}_{b}_{role}", tag="amden")
        nc.vector.tensor_tensor(out=den[:], in0=n1[:, 2, s_me, :],
                                in1=n2s[:], op=ALU.mult)
        nc.vector.tensor_scalar_max(den[:], den[:], EPS)
        nc.vector.reciprocal(den[:], den[:])
        nc.vector.tensor_tensor(out=mcol(mt, AM_SLOT, role, b), in0=nump[:],
                                in1=den[:], op=ALU.mult)

    # ---- stage normalized attention (bf16) to DRAM for the AX phases
    atb = amp.tile([64, 64], BF16, name=f"ab_{d}_{b}", tag="atb")
    nc.vector.tensor_copy(atb[:], attn[:])
    atbT = amp.tile([64, 64], BF16, name=f"abT_{d}_{b}", tag="atbT")
    nc.vector.tensor_copy(atbT[:], attTn[:])
    dsc = dramp.tile([64, 64], BF16, name=f"dx_{d}_{b}")
    nc.sync.dma_start(dsc[:], atb[:])
    dscT = dramp.tile([64, 64], BF16, name=f"dxT_{d}_{b}")
    nc.sync.dma_start(dscT[:], atbT[:])
    stage[(d, b)] = (dsc, dscT)



def _make_ax_emit(nc, tc, ctx, conB, n1s, w2b, mvT, mctx):
    """AX feature (max-attentive cosine), chunked by groups of 8 output
    tokens so the middle chunks interleave with the agg scan's emission.

    ax_unit(d, b, role, ch, during): products + joint bf16 tree-max +
    numerator/norm matmuls accumulated into per-(d, chunk) psum tiles.
    ax_tail(ch): ONE batched Sqrt per dir (both dirs adjacent in ACT
    program order — Sqrt lives in a different ACT table set than the
    scan's Sigmoid/Tanh, so scattering per-unit Sqrts through the scan
    would thrash 1.3us table reloads), then den/recip/feature write for
    all 8 (role, b) units of the chunk at once.

    `during=True` alternates products Pool/DVE for scan-concurrent
    execution; `during=False` uses the greedy DVE/Pool balancer.
    """
    veng = mctx['veng']
    stage, rot = mctx['stage'], mctx['rot']
    axp = ctx.enter_context(tc.tile_pool(name="axp", bufs=2))
    axps = ctx.enter_context(tc.tile_pool(name="axps", bufs=2, space="PSUM"))
    pcnt = [0]
    acc = {}

    def ax_unit(d, b, role, ch, pair, during=False):
        di = 0 if d == 'f' else 1
        i0 = 8 * ch
        cB = conB[d]
        u = role * BL + b
        if pair not in acc:
            acc[pair] = axps.tile([20, 4, 2, 8, 8], F32,
                                  name=f"acc_{pair[0]}_{pair[1]}",
                                  tag="axacc", bufs=2)
        slot = 2 * di + (0 if ch == pair[0] else 1)
        nuxc = acc[pair][:, slot, 0, :, :]
        n2c = acc[pair][:, slot, 1, :, :]
        sp, sh = b, BL + b
        dsc, dscT = stage[(d, b)]
        src = dsc if role == 0 else dscT
        s_v = sh if role == 0 else sp
        s_me = sp if role == 0 else sh
        # broadcast the staged attn rows [i0:i0+8) to all 128 partitions
        bc = axp.tile([128, 8, 64], BF16,
                      name=f"bc_{d}_{b}_{role}_{ch}", tag="bc", bufs=5)
        nc.sync.dma_start(
            bc[:], bass.AP(tensor=src.tensor, offset=src.offset + i0 * 64,
                           ap=[[0, 128], [64, 8], [1, 64]]))
        prod = axp.tile([128, 2, 8, 64], BF16,
                        name=f"xp_{d}_{b}_{role}_{ch}", tag="xprod", bufs=3)
        pcnt[0] += 1
        eng = nc.gpsimd if pcnt[0] % 3 != 0 else nc.vector
        vb = cB[:, :, s_v, :]
        eng.tensor_tensor(
            out=prod[:],
            in0=bass.AP(tensor=vb.tensor, offset=vb.offset,
                        ap=[vb.ap[0], vb.ap[1], [0, 8], vb.ap[2]]),
            in1=bass.AP(tensor=bc.tensor, offset=bc.offset,
                        ap=[bc.ap[0], [0, 2], bc.ap[1], bc.ap[2]]),
            op=ALU.mult)
        rot['dve'] += 0.62  # tree max: DVE only
        cur = prod
        for w in (32, 16, 8, 4, 2):
            nxt = axp.tile([128, 2, 8, w], BF16,
                           name=f"tm_{d}_{b}_{role}_{ch}_{w}",
                           tag=f"tm{w}", bufs=2)
            nc.vector.tensor_tensor(out=nxt[:], in0=cur[:, :, :, 0:w],
                                    in1=cur[:, :, :, w:2 * w], op=ALU.max)
            cur = nxt
        axm = axp.tile([128, 2, 8], F32R,
                       name=f"axm_{d}_{b}_{role}_{ch}", tag="axm", bufs=3)
        nc.vector.tensor_tensor(
            out=axm[:],
            in0=cur[:, :, :, 0:1].rearrange("k c t o -> k c (t o)"),
            in1=cur[:, :, :, 1:2].rearrange("k c t o -> k c (t o)"),
            op=ALU.max)
        prodx = axp.tile([128, 2, 8], BF16,
                         name=f"px_{d}_{b}_{role}_{ch}", tag="prodx", bufs=3)
        nc.vector.tensor_tensor(out=prodx[:], in0=cB[:, :, s_me, i0:i0 + 8],
                                in1=axm[:], op=ALU.mult)
        for c in range(2):
            nc.tensor.matmul(nuxc[:, u, :], w2b[:, c, di, 60:80],
                             prodx[:, c, :], start=(c == 0), stop=(c == 1))
        axsq = axp.tile([128, 2, 8], BF16,
                        name=f"xs_{d}_{b}_{role}_{ch}", tag="axsq", bufs=3)
        nc.scalar.activation(axsq.rearrange("k c t -> k (c t)"),
                             axm.rearrange("k c t -> k (c t)"), AF.Square)
        for c in range(2):
            nc.tensor.matmul(n2c[:, u, :], w2b[:, c, di, 60:80],
                             axsq[:, c, :], start=(c == 0), stop=(c == 1))

    def ax_tail(pair):
        at = acc.pop(pair)
        sq = {}
        for di, d in enumerate('fb'):
            for cpos, ch in enumerate(pair):
                n2s = axp.tile([20, 8, 8], F32, name=f"n2s_{d}_{ch}",
                               tag="n2s", bufs=4)
                nc.scalar.activation(n2s[:], at[:, 2 * di + cpos, 1, :, :],
                                     AF.Sqrt)
                sq[(d, ch)] = n2s
        for di, d in enumerate('fb'):
            n1 = n1s[d]
            for cpos, ch in enumerate(pair):
                i0 = 8 * ch
                nuxc = at[:, 2 * di + cpos, 0, :, :]
                n2s = sq[(d, ch)]
                den = axp.tile([20, 8, 8], F32, name=f"dnc_{d}_{ch}",
                               tag="denc", bufs=2)
                nc.vector.tensor_tensor(
                    out=den[:], in0=n1[:, 3, :, i0:i0 + 8],
                    in1=n2s[:], op=ALU.mult)
                nc.vector.tensor_scalar_max(
                    den.rearrange("l s t -> l (s t)"),
                    den.rearrange("l s t -> l (s t)"), EPS)
                nc.vector.reciprocal(den.rearrange("l s t -> l (s t)"),
                                     den.rearrange("l s t -> l (s t)"))
                out = mvT[di][96:116, 8 * i0:8 * i0 + 64].rearrange(
                    "l (t s) -> l t s", s=8)
                nc.vector.tensor_tensor(out=out,
                                        in0=nuxc.rearrange("l s t -> l t s"),
                                        in1=den.rearrange("l s t -> l t s"),
                                        op=ALU.mult)

    return ax_unit, ax_tail


# ---------------------------------------------------------------- entry

def _get_nc(debug=False):
    key = ('dbg' if debug else 'rel')
    if key not in _CACHE:
        _CACHE[key] = build_nc(debug)
    return _CACHE[key]


def kernel(**inputs):
    nc = _get_nc(False)
    w = _prep_weights(inputs)
    in_maps = []
    for core in range(NCORES):
        m = dict(w)
        m['tokp'] = _prep_tokens(inputs['q1_inputs'], inputs['q2_inputs'],
                                 core)
        in_maps.append(m)
    res = run_bass_kernel_spmd(nc, in_maps, core_ids=list(range(NCORES)))
    out = np.concatenate([res.results[c]['y'] for c in range(NCORES)], axis=0)
    return out.astype(np.float32)


def run_debug(inputs):
    nc = _get_nc(True)
    w = _prep_weights(inputs)
    in_maps = []
    for core in range(NCORES):
        m = dict(w)
        m['tokp'] = _prep_tokens(inputs['q1_inputs'], inputs['q2_inputs'],
                                 core)
        in_maps.append(m)
    res = run_bass_kernel_spmd(nc, in_maps, core_ids=list(range(NCORES)))
    return res



# revision 107
# speedup vs baseline: 1.0039x; 1.0039x over previous
"""BiMPM Trainium2 Bass kernel — pure data parallel over batch (B=32 -> 4/core).

Per-core layouts (B_l=4, stack S=8 rows per step = [p:b0..3, h:b0..3]):
- token/row order: r = t*8 + s, s = seq*4 + b (seq0 = q1 = "p", seq1 = q2 = "h")
- xgT (input projections): (128 = g%128, 8 gc, 512 col=t*8+s) bf16 per dir,
  t-quartered in scan-consumption order so the ctx scan starts early
- scan: fused fw+bw per step; gates psum (128, 2dir, 8gc, 8s) in one 2KB
  bank with PER-DIR start/stop chains so each dir's sigmoid fires without
  waiting for the other's matmuls; g-gates host-prescaled x2 so ONE Sigmoid
  covers a dir's gates (tanh(g) = 2*sigmoid(2g)-1); h double-buffered and
  output copies on DVE (keeps the h-write's tanh RAW wait attached to the
  instruction instead of spilling to a SEQ-blocking EventSemaphore)
- conT f32r / conB bf16 (ctx outputs, hd-major): (128, 2c, 8s, 64t) per dir
- matching prep (csq/n1 norms, ctm transposes, rvn token norms, bld MAX
  builds) emitted in 16/32-token chunks INTERLEAVED into the ctx scan's
  engine idle time via a per-step hook; sqrt/recip finals batched post-scan
  (Sqrt shares no ACT table set with Sigmoid/Tanh — 1.3us reload each)
- matching: FULL/MAX/AM as before (MAX reduce = bf16 TT tree; AM scalar
  normalizations on ACT via per-partition scale APs; bld on idle Pool,
  dirs share one 40KB buffer)
- AX in 8-token chunk pairs (c, 7-c): products (Pool-biased 2:1) + joint
  2-channel bf16 tree-max; numerator/norm matmuls accumulate into one psum
  bank per pair; ONE batched Sqrt site per pair. Head pair (0,7) runs
  before the agg scan; mid pairs + t-chunked agg projections are emitted
  from the agg scan's per-step hook, paced so pair (c, 7-c) is fully
  emitted before scan step 8c reads its xgaT chunk (emission order IS the
  dependency order for the tile tracker — late emission = uninit reads)
- weights shipped bf16 from host (wih/whh/awhh/fc; agg proj stays f32r);
  fc head all-bf16 against bf16 hfin
- mvT (match features): 2 tiles (128, 512) f32r, feature rows at 32-aligned
  slots [full@0, max@32, am@64, ax@96, ones@116]

TimelineSim: 574376 ns (baseline 618195); HW rel err 7.4e-3 (gate 2e-2).
word_emb shipped bf16 (gather-then-round == round-then-gather: identical).
"""
import ml_dtypes
import numpy as np
from contextlib import ExitStack

BF16_NP = ml_dtypes.bfloat16

import concourse.bass as bass
import concourse.tile as tile
from concourse import bacc, mybir
from concourse.bass_utils import run_bass_kernel_spmd
from concourse.masks import make_identity

F32 = mybir.dt.float32
F32R = mybir.dt.float32r
BF16 = mybir.dt.bfloat16
I32 = mybir.dt.int32
AF = mybir.ActivationFunctionType
ALU = mybir.AluOpType
AX_X = mybir.AxisListType.X

B, T, V, D, H, L, NL = 32, 64, 50000, 300, 256, 20, 2
NCORES = 8
BL = B // NCORES
S = 2 * BL
EPS = 1e-8

_CACHE = {}
PHASES = 'full'  # 'ctx' | 'match' | 'full' (for TimelineSim bisection)


# ---------------------------------------------------------------- host prep

def _gate_reorder(w):
    # PyTorch gate order i,f,g,o -> chunk order [i, f, o, 2*g].
    # The x2 on g lets the scan use one Sigmoid for all gates:
    # tanh(g) == 2*sigmoid(2g) - 1.
    i, f, g, o = np.split(w, 4, axis=0)
    return np.concatenate([i, f, o, 2.0 * g], axis=0)


def _prep_weights(inp):
    w = {}
    f32 = np.float32

    def ctx_wT(dir_):
        # ws layout: [k%128, kc(3), gc(8), m(128)]; row 300 = bias, pad to 384
        wih = _gate_reorder(np.asarray(inp[f'ctx_wih_{dir_}'], f32))
        bias = _gate_reorder(
            np.asarray(inp[f'ctx_bih_{dir_}'] + inp[f'ctx_bhh_{dir_}'],
                       f32)[:, None]).T
        wt = np.concatenate([wih.T, bias, np.zeros((83, 1024), f32)], 0)
        return np.ascontiguousarray(
            wt.reshape(3, 128, 8, 128).transpose(1, 0, 2, 3)).astype(BF16_NP)

    def whhT(pfx, dir_):
        # ws layout: [k%128, kc, gc, m] = whh_reord[gc*128+m, kc*128+k]
        whh = _gate_reorder(np.asarray(inp[f'{pfx}_whh_{dir_}'], f32))
        return np.ascontiguousarray(
            whh.T.reshape(2, 128, 8, 128).transpose(1, 0, 2, 3)).astype(
                BF16_NP)

    w['wihT_f'], w['wihT_b'] = ctx_wT('f'), ctx_wT('b')
    w['whhT_f'], w['whhT_b'] = whhT('ctx', 'f'), whhT('ctx', 'b')
    w['awhhT_f'], w['awhhT_b'] = whhT('agg', 'f'), whhT('agg', 'b')

    def agg_wT(dir_):
        wih = _gate_reorder(np.asarray(inp[f'agg_wih_{dir_}'], f32))
        bias = _gate_reorder(
            np.asarray(inp[f'agg_bih_{dir_}'] + inp[f'agg_bhh_{dir_}'],
                       f32)[:, None]).T
        out = np.zeros((256, 1024), f32)
        for d in range(2):
            for ty in range(4):
                src = wih[:, d * 80 + ty * 20: d * 80 + ty * 20 + 20]
                out[d * 128 + 32 * ty: d * 128 + 32 * ty + 20] = src.T
        out[116] = bias[0]
        return np.ascontiguousarray(
            out.reshape(2, 128, 8, 128).transpose(1, 0, 2, 3), f32)

    w['aggwT_f'], w['aggwT_b'] = agg_wT('f'), agg_wT('b')

    # w2T80: (128 = h%128, 2 c, 2 dir, 80 = ty*20+l), ty in [full,max,am,ax]
    w2 = np.asarray(inp['mp_w'], f32) ** 2
    w2t = np.zeros((128, 2, 2, 80), f32)
    for d in range(2):
        for ty in range(4):
            src = w2[2 * ty + d]
            for c in range(2):
                w2t[:, c, d, ty * 20:(ty + 1) * 20] = \
                    src[:, c * 128:(c + 1) * 128].T
    w['w2T'] = np.ascontiguousarray(w2t)

    fc1 = np.asarray(inp['fc1_w'], f32)
    w['fc1T'] = np.ascontiguousarray(
        fc1.T.reshape(8, 128, 512).transpose(1, 0, 2)).astype(BF16_NP)
    w['fc1b'] = np.ascontiguousarray(
        np.broadcast_to(np.asarray(inp['fc1_b'], f32), (BL, 512))).astype(
            BF16_NP)
    fc2 = np.asarray(inp['fc2_w'], f32)
    w['fc2T'] = np.ascontiguousarray(
        fc2.T.reshape(4, 128, 2).transpose(1, 0, 2)).astype(BF16_NP)
    w['fc2b'] = np.ascontiguousarray(
        np.broadcast_to(np.asarray(inp['fc2_b'], f32), (BL, 2))).astype(
            BF16_NP)
    w['word_emb'] = np.ascontiguousarray(
        np.asarray(inp['word_emb'], f32)).astype(BF16_NP)
    return w


def _prep_tokens(q1, q2, core):
    q1c = np.asarray(q1[core * BL:(core + 1) * BL]).astype(np.int64)
    q2c = np.asarray(q2[core * BL:(core + 1) * BL]).astype(np.int64)
    tok = np.zeros((T * S,), np.int32)
    for seq, q in ((0, q1c), (1, q2c)):
        for b in range(BL):
            tok[np.arange(T) * S + seq * BL + b] = q[b]
    return np.ascontiguousarray(tok.reshape(4, 128))


# ---------------------------------------------------------------- build

def build_nc(debug=False):
    nc = bacc.Bacc("TRN2", target_bir_lowering=False, debug=False,
                   enable_asserts=True, num_devices=NCORES)
    dt = nc.dram_tensor
    dr = {}
    dr['tokp'] = dt("tokp", [4, 128], I32, kind="ExternalInput").ap()
    dr['word_emb'] = dt("word_emb", [V, D], BF16,
                        kind="ExternalInput").ap()
    for n, shp in [('wihT_f', [128, 3, 8, 128]), ('wihT_b', [128, 3, 8, 128]),
                   ('whhT_f', [128, 2, 8, 128]), ('whhT_b', [128, 2, 8, 128]),
                   ('awhhT_f', [128, 2, 8, 128]),
                   ('awhhT_b', [128, 2, 8, 128]),
                   ('fc1T', [128, 8, 512]), ('fc1b', [BL, 512]),
                   ('fc2T', [128, 4, 2]), ('fc2b', [BL, 2])]:
        dr[n] = dt(n, shp, BF16, kind="ExternalInput").ap()
    for n, shp in [('aggwT_f', [128, 2, 8, 128]), ('aggwT_b', [128, 2, 8, 128]),
                   ('w2T', [128, 2, 2, 80])]:
        dr[n] = dt(n, shp, F32, kind="ExternalInput").ap()
    y = dt("y", [BL, NL], F32, kind="ExternalOutput").ap()
    dbg = {}
    if debug:
        dbg['conT_f'] = dt("dbg_conT_f", [128, 2, 8, 64], F32,
                           kind="ExternalOutput").ap()
        dbg['conT_b'] = dt("dbg_conT_b", [128, 2, 8, 64], F32,
                           kind="ExternalOutput").ap()
        dbg['mvT0'] = dt("dbg_mvT0", [128, 512], F32,
                         kind="ExternalOutput").ap()
        dbg['mvT1'] = dt("dbg_mvT1", [128, 512], F32,
                         kind="ExternalOutput").ap()
        dbg['xT'] = dt("dbg_xT", [128, 2, 40], F32,
                       kind="ExternalOutput").ap()

    with tile.TileContext(nc) as tc, ExitStack() as ctx:
        _body(nc, tc, ctx, dr, y, dbg)
    nc.compile()
    return nc


def _body(nc, tc, ctx, dr, y, dbg):
    perm = ctx.enter_context(tc.tile_pool(name="perm", bufs=1))

    idf = perm.tile([128, 128], F32, name="idf")
    make_identity(nc, idf[:])
    idb = perm.tile([128, 128], BF16, name="idb")
    nc.vector.tensor_copy(idb[:], idf[:])
    selb = idb.rearrange("k (tl s) -> k tl s", s=8)

    def conv(src, dtype, name, engine=None, pool=None):
        t = (pool or perm).tile(list(src.shape), dtype, name=f"C_{name}")
        eng = engine or nc.vector
        if eng is nc.scalar:
            eng.activation(t[:], src[:], AF.Copy)
        else:
            eng.tensor_copy(t[:], src[:])
        return t

    wihT, whhTb, awhhTb, aggwT = {}, {}, {}, {}
    w2Tf = perm.tile([128, 2, 2, 80], F32, name="w2Tf")
    nc.sync.dma_start(w2Tf[:], dr['w2T'][:])
    w2Tr = conv(w2Tf, F32R, "w2Tr")
    w2Tb = conv(w2Tf, BF16, "w2Tb", nc.gpsimd)

    idx_sb = perm.tile([128, 4], I32, name="idx_sb")
    nc.sync.dma_start(idx_sb[:], dr['tokp'].rearrange("m p -> p m"))

    # ---------------- weight load + embedding gather + ctx projection (bf16)
    # xgT[d]: (128 = g%128, 8 gc, 512 cols) bf16 ; col r = t*8 + s
    # Gather tiles share scope with weight staging (no SBUF reuse between
    # the indirect-DMA writes and freed staging tiles).
    xgT = {'f': perm.tile([128, 8, 512], BF16, name="xgT_f"),
           'b': perm.tile([128, 8, 512], BF16, name="xgT_b")}
    with tc.tile_pool(name="embp", bufs=2) as embp, \
         tc.tile_pool(name="loadp", bufs=1) as loadp, \
         tc.tile_pool(name="epsum", bufs=2, space="PSUM") as epsum:
        # embT (128 = d%128, 3 kc, 512 tok) bf16
        embT = embp.tile([128, 3, 512], BF16, name="embT", tag="embT")
        embs = []
        for m in range(4):
            emb = embp.tile([128, 304], BF16, name=f"emb_{m}", tag=f"emb{m}")
            nc.gpsimd.indirect_dma_start(
                out=emb[:, 0:300], out_offset=None, in_=dr['word_emb'][:],
                in_offset=bass.IndirectOffsetOnAxis(ap=idx_sb[:, m:m + 1],
                                                    axis=0))
            nc.vector.memset(emb[:, 300:301], 1.0)
            embs.append(emb)

        def load_f32(name, shp, tag):
            t = loadp.tile(shp, F32, name=f"L_{name}", tag=tag)
            nc.sync.dma_start(t[:], dr[name][:])
            return t

        def load_bf16(name, shp, pool):
            t = pool.tile(shp, BF16, name=f"B_{name}")
            nc.sync.dma_start(t[:], dr[name][:])
            return t

        for d in 'fb':
            wihT[d] = load_bf16(f'wihT_{d}', [128, 3, 8, 128], embp)
            whhTb[d] = load_bf16(f'whhT_{d}', [128, 2, 8, 128], perm)
            awhhTb[d] = load_bf16(f'awhhT_{d}', [128, 2, 8, 128], perm)
            aggwT[d] = conv(load_f32(f'aggwT_{d}', [128, 2, 8, 128], "raw8k"),
                            F32R, f"aggw_{d}", nc.scalar)

        for m in range(4):
            embb = embs[m]
            for c in range(3):
                kc = min(128, 301 - 128 * c)
                tp = epsum.tile([128, 128], BF16, name=f"etp_{m}_{c}",
                                tag="etp")
                nc.tensor.transpose(tp[0:kc, :],
                                    embb[:, 128 * c:128 * c + kc], idb[:])
                if c % 2 == 0:
                    nc.scalar.activation(embT[0:kc, c, 128 * m:128 * (m + 1)],
                                         tp[0:kc, :], AF.Copy)
                else:
                    nc.vector.tensor_copy(
                        embT[0:kc, c, 128 * m:128 * (m + 1)], tp[0:kc, :])
        # t-quartered, scan-consumption-ordered (f ascending, b descending)
        # so the ctx scan's first steps start before the full projection
        qorder = [(0, 0), (1, 3), (0, 1), (1, 2), (0, 2), (1, 1), (0, 3),
                  (1, 0)]
        for di, q in qorder:
            d = 'fb'[di]
            for gc in range(8):
                ps = epsum.tile([128, 128], F32, name=f"xps_{d}_{gc}_{q}",
                                tag="xps")
                for c in range(3):
                    kc = min(128, 301 - 128 * c)
                    nc.tensor.matmul(ps[:], wihT[d][0:kc, c, gc, :],
                                     embT[0:kc, c, 128 * q:128 * (q + 1)],
                                     start=(c == 0), stop=(c == 2))
                if gc % 2 == 0:
                    nc.vector.tensor_copy(
                        xgT[d][:, gc, 128 * q:128 * (q + 1)], ps[:])
                else:
                    nc.scalar.activation(
                        xgT[d][:, gc, 128 * q:128 * (q + 1)], ps[:], AF.Copy)

    # ---------------- scan layer (shared ctx/agg), fused fw+bw per step
    # state h/c: (128 = hd%128, 2 dir, 2 kc, 8 s)
    # gates psum: (128 = g%128, 2 dir, 8 gc, 8 s), order [i0 i1 f0 f1 o0 o1 g0 g1]
    # g-gates pre-scaled x2 at host: tanh(g) = 2*sigmoid(2g) - 1, so one
    # Sigmoid covers all 8 chunks; xg injected via identity matmul (start=True).
    def scan_layer(xgd, whh_d, conT_out, conB_out, hfin, lname, ve=None,
                   hook=None):
        ve = ve or nc.vector
        sp = ctx2.enter_context(tc.tile_pool(name=f"sp_{lname}", bufs=12))
        pp = ctx2.enter_context(tc.tile_pool(name=f"pp_{lname}", bufs=3,
                                             space="PSUM"))
        cp = ctx2.enter_context(tc.tile_pool(name=f"cp_{lname}", bufs=1))
        c_sb = cp.tile([128, 2, 2, 8], F32, name=f"c_{lname}")
        # h double-buffered: the step-t write must not WAR against step-t's
        # own whh matmul reads (a 2-sem wait the tile framework lowers to a
        # SEQ-blocking EventSemaphore on DVE, ~200ns/step on the chain)
        h_bufs = [cp.tile([128, 2, 2, 8], BF16, name=f"h_{lname}_{i}")
                  for i in range(2)]
        nc.vector.memset(c_sb[:], 0.0)
        nc.vector.memset(h_bufs[0][:], 0.0)
        nc.vector.memset(h_bufs[1][:], 0.0)
        for tau in range(T):
            ts_ = {'f': tau, 'b': T - 1 - tau}
            h_prev = h_bufs[(tau + 1) % 2]
            h_sb = h_bufs[tau % 2]
            # one full psum bank (2KB); each dir's 1KB region runs its own
            # start/stop chain so dir f's sigmoid fires without waiting for
            # dir b's matmuls — the two cell-update chains then overlap.
            psb = pp.tile([128, 512], F32, name=f"g_{lname}_{tau}",
                          tag="gps")
            ps = psb[:, 0:128].rearrange("k (d g s) -> k d g s", d=2, g=8)
            sig = sp.tile([128, 2, 8, 8], F32, name=f"si_{lname}_{tau}",
                          tag="sig")
            t1 = sp.tile([128, 2, 2, 8], F32, name=f"t1_{lname}_{tau}",
                         tag="t1")
            t2h = sp.tile([128, 2, 2, 8], F32, name=f"t2_{lname}_{tau}",
                          tag="t2h")
            th = sp.tile([128, 2, 2, 8], F32, name=f"th_{lname}_{tau}",
                         tag="th")
            for di, d in enumerate('fb'):
                t = ts_[d]
                nc.tensor.matmul(ps[:, di, :, :], idb[:],
                                 xgd[d][:, :, 8 * t:8 * t + 8],
                                 start=True, stop=False)
                for gc in range(8):
                    for kc in range(2):
                        nc.tensor.matmul(
                            ps[:, di, gc, :], whh_d[d][:, kc, gc, :],
                            h_prev[:, di, kc, :], start=False,
                            stop=(gc == 7 and kc == 1))
                nc.scalar.activation(sig[:, di, :, :], ps[:, di, :, :],
                                     AF.Sigmoid)
            for di in range(2):
                ve.tensor_tensor(out=t1[:, di, :, :],
                                 in0=sig[:, di, 2:4, :],
                                 in1=c_sb[:, di, :, :], op=ALU.mult)
                ve.scalar_tensor_tensor(
                    out=t2h[:, di, :, :], in0=sig[:, di, 6:8, :], scalar=0.5,
                    in1=sig[:, di, 0:2, :], op0=ALU.subtract, op1=ALU.mult)
                ve.scalar_tensor_tensor(
                    out=c_sb[:, di, :, :], in0=t2h[:, di, :, :], scalar=2.0,
                    in1=t1[:, di, :, :], op0=ALU.mult, op1=ALU.add)
                nc.scalar.activation(th[:, di, :, :], c_sb[:, di, :, :],
                                     AF.Tanh)
            for di in range(2):
                ve.tensor_tensor(out=h_sb[:, di, :, :],
                                 in0=sig[:, di, 4:6, :],
                                 in1=th[:, di, :, :], op=ALU.mult)
            for di, d in enumerate('fb'):
                t = ts_[d]
                # copies on DVE: a Pool reader of h_sb would put a WAR wait
                # on the next h write, displacing its tanh RAW wait onto a
                # SEQ-blocking EventSemaphore (1-wait-per-instruction HW rule)
                if conT_out is not None:
                    nc.vector.tensor_copy(
                        conT_out[d][:, :, :, t].rearrange("k a b -> k (a b)"),
                        h_sb[:, di, :, :].rearrange("k a b -> k (a b)"))
                if conB_out is not None:
                    nc.vector.tensor_copy(
                        conB_out[d][:, :, :, t].rearrange("k a b -> k (a b)"),
                        h_sb[:, di, :, :].rearrange("k a b -> k (a b)"))
                if hfin is not None and tau == T - 1:
                    nc.vector.tensor_copy(
                        hfin[d].rearrange("k a b -> k (a b)"),
                        h_sb[:, di, :, :].rearrange("k a b -> k (a b)"))
            if hook is not None:
                hook(tau)

    conT = {'f': perm.tile([128, 2, 8, 64], F32R, name="conT_f"),
            'b': perm.tile([128, 2, 8, 64], F32R, name="conT_b")}
    conB = {'f': perm.tile([128, 2, 8, 64], BF16, name="conB_f"),
            'b': perm.tile([128, 2, 8, 64], BF16, name="conB_b")}

    # fc weights (bf16 host-prepped): plain DMAs, no staging/convert
    fcp = ctx.enter_context(tc.tile_pool(name="fcp", bufs=1))
    fc1T = fcp.tile([128, 8, 512], BF16, name="fc1T")
    fc2T = fcp.tile([128, 4, 2], BF16, name="fc2T")
    fc1b = fcp.tile([BL, 512], BF16, name="fc1b")
    fc2b = fcp.tile([BL, 2], BF16, name="fc2b")
    for nm, tgt in (('fc1T', fc1T), ('fc2T', fc2T), ('fc1b', fc1b),
                    ('fc2b', fc2b)):
        nc.sync.dma_start(tgt[:], dr[nm][:])

    # matching prep interleaved into the ctx scan's engine idle time;
    # (dir, quarter) becomes ready as the scan's two fronts advance
    prep_ps_stack = ExitStack()
    bld_stack = ExitStack()
    prep = _make_prep(nc, tc, ctx, prep_ps_stack, bld_stack, conT, conB,
                      w2Tr, w2Tf, idb)
    pq = []
    for qi, (fq, bq) in enumerate(((0, 3), (1, 2), (2, 1))):
        rt = 16 * (qi + 1) - 1
        for cl in prep['units']('f', fq):
            pq.append((rt, cl))
        for cl in prep['units']('b', bq):
            pq.append((rt, cl))
        if qi == 1:
            for cl in prep['halves']('f', 0):
                pq.append((31, cl))
            for cl in prep['halves']('b', 1):
                pq.append((31, cl))
            for cl in prep['bld_units']('f', 0, 32):
                pq.append((31, cl))
    ppos = [0]

    def ctx_hook(tau):
        n = 0
        while ppos[0] < len(pq) and n < 2:
            rt, cl = pq[ppos[0]]
            if rt > tau:
                break
            cl()
            ppos[0] += 1
            n += 1

    with ExitStack() as ctx2:
        scan_layer(xgT, whhTb, conT, conB, None, "ctx", hook=ctx_hook)
    while ppos[0] < len(pq):
        pq[ppos[0]][1]()
        ppos[0] += 1
    for cl in prep['units']('f', 3):
        cl()
    for cl in prep['units']('b', 0):
        cl()
    for cl in prep['halves']('f', 1):
        cl()
    for cl in prep['halves']('b', 0):
        cl()
    prep['finals']()
    prep_ps_stack.close()
    for cl in prep['bld_units']('f', 32, 64, mix=True):
        cl()

    if PHASES == 'ctx':
        y_sb0 = perm.tile([BL, NL], F32, name="y_sb0")
        nc.vector.tensor_copy(y_sb0[:], conT['f'][0:BL, 0, 0, 0:NL])
        nc.sync.dma_start(y[:], y_sb0[:])
        return

    # ---------------- matching
    mvT = [perm.tile([128, 512], F32R, name="mvT0"),
           perm.tile([128, 512], F32R, name="mvT1")]
    # f32r memset unsupported; fill via ACT copy with scale=0 (+bias)
    fill_src = bass.AP(tensor=idf.tensor, offset=idf.offset,
                       ap=[idf.ap[0], [0, 512]])
    nc.scalar.activation(mvT[0][:], fill_src, AF.Copy, bias=0.0, scale=0.0)
    nc.scalar.activation(mvT[1][:], fill_src, AF.Copy, bias=0.0, scale=0.0)
    nc.scalar.activation(mvT[0][96:128, :],
                         bass.AP(tensor=idf.tensor, offset=idf.offset,
                                 ap=[[idf.ap[0][0], 32], [0, 512]]),
                         AF.Copy, bias=1.0, scale=0.0)
    mctx = _matching(nc, tc, ctx, conT, conB, w2Tr, w2Tf, w2Tb, mvT,
                     idf, idb, prep)
    bld_stack.close()

    pipe_stack = ExitStack()
    ctx.enter_context(pipe_stack)
    ax_unit, ax_tail = _make_ax_emit(nc, tc, pipe_stack, conB, mctx['n1s'],
                                     w2Tb, mvT, mctx)

    def ax_pair(pair, during=False):
        for ch in pair:
            for d in 'fb':
                for role in range(2):
                    for b in range(BL):
                        ax_unit(d, b, role, ch, pair, during)
        ax_tail(pair)

    if PHASES == 'match':
        for pair in ((0, 7), (1, 6), (2, 5), (3, 4)):
            ax_pair(pair)
        y_sb0 = perm.tile([BL, NL], F32, name="y_sb0")
        nc.vector.tensor_copy(y_sb0[:], mvT[0][0:BL, 0:NL])
        nc.sync.dma_start(y[:], y_sb0[:])
        return

    # ---------------- AX + agg projection pipelined under the agg scan.
    # The agg scan consumes xgaT cols from both ends inward (fw t=tau,
    # bw t=63-tau), in 8-token chunks: chunk pair (c, 7-c) is needed at
    # scan step 8c. Chunks 0/7 (plus their AX features) are computed
    # before the scan; the middle chunks' AX units + projections are
    # emitted from the scan's per-step hook so they execute in engine
    # idle time.
    xgaT = {'f': perm.tile([128, 8, 512], BF16, name="xgaT_f"),
            'b': perm.tile([128, 8, 512], BF16, name="xgaT_b")}
    ap_ps = pipe_stack.enter_context(tc.tile_pool(name="aggps", bufs=3,
                                                  space="PSUM"))

    def proj_chunk(c):
        c0 = 64 * c
        for di, d in enumerate('fb'):
            for gc in range(8):
                ps = ap_ps.tile([128, 64], F32, name=f"ap_{d}_{gc}_{c}",
                                tag="aps")
                for kc in range(2):
                    nc.tensor.matmul(ps[:], aggwT[d][:, kc, gc, :],
                                     mvT[kc][:, c0:c0 + 64],
                                     start=(kc == 0), stop=(kc == 1))
                nc.scalar.activation(xgaT[d][:, gc, c0:c0 + 64], ps[:],
                                     AF.Copy)

    ax_pair((0, 7))
    proj_chunk(0)
    proj_chunk(7)

    def tail_proj(pair):
        ax_tail(pair)
        proj_chunk(pair[0])
        proj_chunk(pair[1])

    units = []
    for cpair in ((1, 6), (2, 5), (3, 4)):
        for c in cpair:
            for d in 'fb':
                for role in range(2):
                    for b in range(BL):
                        units.append((ax_unit, d, b, role, c, cpair, True))
        units.append((tail_proj, cpair))
    qpos = [0]

    def agg_hook(tau):
        # EMISSION-ORDER CORRECTNESS: the tile tracker only sees deps from
        # writes emitted BEFORE a read. Chunk pair k (chunks k, 7-k) is read
        # by scan step 8k, so its units+projection must be fully emitted
        # strictly before that step's instructions. Pace linearly to each
        # deadline (~4.2 units/step through step 21).
        target = min(len(units), (tau + 3) * len(units) // 25 + 1)
        while qpos[0] < target:
            u = units[qpos[0]]
            qpos[0] += 1
            u[0](*u[1:])

    # ---------------- agg scans + fc
    hfin = {d: perm.tile([128, 2, 8], BF16, name=f"hfin_{d}") for d in 'fb'}
    with ExitStack() as ctx2:
        scan_layer(xgaT, awhhTb, None, None, hfin, "agg", hook=agg_hook)
    assert qpos[0] >= len(units)
    pipe_stack.close()
    fps = ctx.enter_context(tc.tile_pool(name="fcps", bufs=1, space="PSUM"))

    # x k-chunks: [hpf c0, hpf c1, hpb c0, hpb c1, hhf c0, hhf c1, hhb c0, hhb c1]
    ksl = []
    for role0 in (0, 4):
        for d in 'fb':
            for c in range(2):
                ksl.append(hfin[d][:, c, role0:role0 + BL])
    x1 = fps.tile([BL, 512], F32, name="x1")
    for kc in range(8):
        nc.tensor.matmul(x1[:], ksl[kc], fc1T[:, kc, :],
                         start=(kc == 0), stop=False)
    nc.tensor.matmul(x1[:], idb[0:BL, 0:BL], fc1b[:], start=False, stop=True)
    xt1 = fcp.tile([BL, 512], F32, name="xt1")
    nc.scalar.activation(xt1[:], x1[:], AF.Tanh)
    xt1ps = fps.tile([128, 4, BL], F32, name="xt1ps")
    for c in range(4):
        nc.tensor.transpose(xt1ps[:, c, :], xt1[:, 128 * c:128 * (c + 1)],
                            idf[0:BL, 0:BL])
    xt1T = fcp.tile([128, 4, BL], BF16, name="xt1T")
    nc.vector.tensor_copy(xt1T[:], xt1ps[:])
    yps = fps.tile([BL, NL], F32, name="yps")
    for c in range(4):
        nc.tensor.matmul(yps[:], xt1T[:, c, :], fc2T[:, c, :],
                         start=(c == 0), stop=False)
    nc.tensor.matmul(yps[:], idb[0:BL, 0:BL], fc2b[:], start=False,
                     stop=True)
    y_sb = fcp.tile([BL, NL], F32, name="y_sb")
    nc.vector.tensor_copy(y_sb[:], yps[:])
    nc.sync.dma_start(y[:], y_sb[:])

    if dbg:
      with tc.tile_pool(name="dbgp", bufs=1) as dbp:
        for d in 'fb':
            cf = dbp.tile([128, 2, 8, 64], F32, name=f"dbgc_{d}")
            nc.scalar.activation(cf[:], conT[d][:], AF.Copy)
            nc.sync.dma_start(dbg[f'conT_{d}'][:], cf[:])
        for i in range(2):
            mf = dbp.tile([128, 512], F32, name=f"dbgm_{i}")
            nc.scalar.activation(mf[:], mvT[i][:], AF.Copy)
            nc.sync.dma_start(dbg[f'mvT{i}'][:], mf[:])
        xtd = dbp.tile([128, 2, 40], F32, name="xtd")
        nc.vector.memset(xtd[:], 0.0)
        nc.vector.tensor_copy(xtd[:, :, 0:8], hfin['f'][:])
        nc.vector.tensor_copy(xtd[:, :, 32:40], hfin['b'][:])
        nc.sync.dma_start(dbg['xT'][:], xtd[:])


# ---------------------------------------------------------------- matching
# ---------------------------------------------------------------- matching

def _make_prep(nc, tc, ctx, psum_stack, bld_stack, conT, conB, w2r, w2f,
               idb):
    """Matching prep (norms / t-major transposes / per-token norms / MAX
    builds), emitted in 16-token quarters so most of it runs in engine
    idle time during the ctx scan. Sqrt/recip finals are batched post-scan
    (Sqrt shares no ACT table set with the scan's Sigmoid/Tanh; scattering
    them through the scan would pay 1.3us table reloads each). bld goes to
    the otherwise-idle Pool engine; the two dirs share one 40KB buffer
    (tag rotation serializes b's builds behind f's MAX reads).
    """
    prep = ctx.enter_context(tc.tile_pool(name="prep", bufs=1))
    bldp = bld_stack.enter_context(tc.tile_pool(name="bldp", bufs=1))
    n1sqp = psum_stack.enter_context(tc.tile_pool(name="n1sqp", bufs=1))
    prepps = psum_stack.enter_context(tc.tile_pool(name="prepps", bufs=2,
                                                   space="PSUM"))
    t = {}
    for d in 'fb':
        t[f'n1sq_{d}'] = n1sqp.tile([20, 4, 8, 64], F32, name=f"n1sq_{d}")
        t[f'n1_{d}'] = prep.tile([20, 4, 8, 64], F32, name=f"n1_{d}")
        t[f'rn1_{d}'] = prep.tile([20, 4, 8, 64], F32, name=f"rn1_{d}")
        t[f'ctm_{d}'] = prep.tile([64, 8, 256], BF16, name=f"ctm_{d}")
        t[f'rvn_{d}'] = prep.tile([64, 8], F32, name=f"rvn_{d}")
    bldt = {}

    def bld_tile(d):
        if d not in bldt:
            bldt[d] = bldp.tile([128, 2, 20, 8, 64], BF16, name=f"bld_{d}",
                                tag="bld", bufs=1)
        return bldt[d]

    def units(d, q):
        di = 0 if d == 'f' else 1
        cT, cB = conT[d], conB[d]
        q0 = 16 * q
        n1sq, ctm, rvn = t[f'n1sq_{d}'], t[f'ctm_{d}'], t[f'rvn_{d}']

        def u_norm():
            csq = prep.tile([128, 2, 8, 16], F32R, name=f"csq_{d}_{q}",
                            tag="csq", bufs=3)
            nc.scalar.activation(csq[:], cT[:, :, :, q0:q0 + 16], AF.Square)
            n1q = prepps.tile([20, 4, 8, 16], F32, name=f"n1q_{d}_{q}",
                              tag="n1q", bufs=2)
            for ty in range(4):
                for c in range(2):
                    nc.tensor.matmul(n1q[:, ty, :, :],
                                     w2r[:, c, di, 20 * ty:20 * ty + 20],
                                     csq[:, c, :, :],
                                     start=(c == 0), stop=(c == 1))
            nc.vector.tensor_copy(n1sq[:, :, :, q0:q0 + 16], n1q[:])

        return [u_norm]

    def halves(d, h):
        # engine partition accesses must be 32-aligned, so the t-major
        # transposes and per-token norms go by 32-token halves
        cB = conB[d]
        h0 = 32 * h
        ctm, rvn = t[f'ctm_{d}'], t[f'rvn_{d}']

        def u_ctm(s0):
            for s in range(s0, s0 + 2):
                tp = prepps.tile([32, 2, 128], BF16, name=f"ct_{d}_{h}_{s}",
                                 tag="ctp", bufs=2)
                for c in range(2):
                    nc.tensor.transpose(tp[:, c, :], cB[:, c, s, h0:h0 + 32],
                                        idb[:])
                if s % 2 == 0:
                    nc.scalar.activation(ctm[h0:h0 + 32, s, :],
                                         tp.rearrange("t c k -> t (c k)"),
                                         AF.Copy)
                else:
                    nc.vector.tensor_copy(ctm[h0:h0 + 32, s, :],
                                          tp.rearrange("t c k -> t (c k)"))

        def u_rvn(s0):
            # bf16 out scratch keeps the STT in 4x DVE perf mode; the f32
            # accum_out (exempt scalar operand) carries the precision
            for s in range(s0, s0 + 4):
                scr = prep.tile([32, 256], BF16, name=f"rs_{d}_{h}_{s}",
                                tag="rvs", bufs=2)
                nc.vector.scalar_tensor_tensor(
                    out=scr[:], in0=ctm[h0:h0 + 32, s, :], scalar=1.0,
                    in1=ctm[h0:h0 + 32, s, :], op0=ALU.mult, op1=ALU.mult,
                    accum_out=rvn[h0:h0 + 32, s:s + 1])

        return [lambda s0=s0: u_ctm(s0) for s0 in range(0, 8, 2)] + \
               [lambda: u_rvn(0), lambda: u_rvn(4)]

    def bld_units(d, tlo, thi, mix=False):
        # mix=True (post-scan): mostly DVE — the bf16 tensor_scalar hits
        # the 4x perf mode (~190ns vs ~800ns Pool); Pool-only when
        # interleaved under the ctx scan where DVE is contended
        di = 0 if d == 'f' else 1
        cB = conB[d]
        bld = bld_tile(d)
        out = []
        for c in range(2):
            for l0 in range(0, L, 2):
                def cl(c=c, l0=l0):
                    for l in range(l0, l0 + 2):
                        eng = (nc.vector if mix and l % 4 != 3
                               else nc.gpsimd)
                        eng.tensor_scalar_mul(
                            bld[:, c, l, :, tlo:thi], cB[:, c, :, tlo:thi],
                            w2f[:, c, di, 20 + l:21 + l])
                out.append(cl)
        return out

    def finals():
        # one sqrt-table residency for all four batched Sqrts
        for d in 'fb':
            nc.scalar.activation(
                t[f'n1_{d}'].rearrange("l y s t -> l (y s t)"),
                t[f'n1sq_{d}'].rearrange("l y s t -> l (y s t)"), AF.Sqrt)
            nc.scalar.activation(t[f'rvn_{d}'][:], t[f'rvn_{d}'][:], AF.Sqrt)
        for d in 'fb':
            n1, rn1 = t[f'n1_{d}'], t[f'rn1_{d}']
            nc.vector.tensor_scalar_max(
                rn1.rearrange("l y s t -> l (y s t)"),
                n1.rearrange("l y s t -> l (y s t)"), EPS)
            nc.vector.reciprocal(rn1.rearrange("l y s t -> l (y s t)"),
                                 rn1.rearrange("l y s t -> l (y s t)"))
            rvn = t[f'rvn_{d}']
            nc.vector.tensor_scalar_max(rvn[:], rvn[:], EPS)
            nc.vector.reciprocal(rvn[:], rvn[:])

    return {'t': t, 'units': units, 'halves': halves,
            'bld_units': bld_units, 'bld_tile': bld_tile, 'finals': finals}


def _matching(nc, tc, ctx, conT, conB, w2r, w2f, w2b, mvT, idf, idb,
              prep):
    stage, n1s = {}, {}
    # feature-type offsets into w2 cols (ty*20) and mv row slots (ty*32)
    # greedy DVE/Pool balancer: Pool runs TT ~3.9x slower than DVE-2x
    # pool pre-charged: Pool's 8us products block their dependent DVE
    # tree stages, so bias assignment away from Pool (swept optimum)
    rot = {'dve': 0.0, 'pool': 30.0}

    def veng(cost=1.0):
        if rot['dve'] + cost <= rot['pool'] + 3.3 * cost:
            rot['dve'] += cost
            return nc.vector
        rot['pool'] += 3.3 * cost
        return nc.gpsimd

    dramp = ctx.enter_context(tc.tile_pool(name="mdram", bufs=1,
                                           space="DRAM"))

    def mcol(mt, slot, ri, b):
        # (20, 64) view of mvT rows [slot:slot+20], cols 8t + ri*4 + b
        return mt[slot:slot + 20, :].rearrange("l (t s) -> l t s",
                                               s=8)[:, :, ri * BL + b]

    for di, d in enumerate('fb'):
        cT, cB = conT[d], conB[d]
        anchor_t = (T - 1) if d == 'f' else 0
        mt = mvT[di]
        n1 = prep['t'][f'n1_{d}']
        rn1 = prep['t'][f'rn1_{d}']
        ctm = prep['t'][f'ctm_{d}']
        rvn = prep['t'][f'rvn_{d}']

        with tc.tile_pool(name=f"mn_{d}", bufs=1) as mn:
          with tc.tile_pool(name=f"mnp_{d}", bufs=2, space="PSUM") as mnp:
            n1s[d] = n1
            # ---- FULL
            ancv = mn.tile([128, 2, 8], F32, name=f"ancv_{d}", tag="ancv")
            nc.vector.tensor_copy(ancv[:], cT[:, :, :, anchor_t])
            for b in range(BL):
                for ri, (s_me, s_an) in enumerate(((b, BL + b), (BL + b, b))):
                    anc = mn.tile([128, 2, 20], BF16, name=f"an_{d}_{b}_{ri}",
                                  tag="anc", bufs=2)
                    for c in range(2):
                        nc.vector.tensor_scalar_mul(
                            anc[:, c, :], w2b[:, c, di, 0:20],
                            ancv[:, c, s_an:s_an + 1])
                    nps = mnp.tile([20, 64], F32, name=f"nf_{d}_{b}_{ri}",
                                   tag="nf")
                    for c in range(2):
                        nc.tensor.matmul(nps[:], anc[:, c, :],
                                         cB[:, c, s_me, :],
                                         start=(c == 0), stop=(c == 1))
                    den = mn.tile([20, 64], F32, name=f"de_{d}_{b}_{ri}",
                                  tag="den", bufs=2)
                    nc.vector.tensor_scalar(
                        out=den[:], in0=n1[:, 0, s_me, :],
                        scalar1=n1[:, 0, s_an, anchor_t:anchor_t + 1],
                        scalar2=EPS, op0=ALU.mult, op1=ALU.max)
                    nc.vector.reciprocal(den[:], den[:])
                    nc.vector.tensor_tensor(out=mcol(mt, 0, ri, b),
                                            in0=nps[:], in1=den[:],
                                            op=ALU.mult)

          # ---- MAX (pair max over the other sequence)
          with tc.tile_pool(name=f"mx_{d}", bufs=1) as mxp, \
               tc.tile_pool(name=f"mxps_{d}", bufs=1, space="PSUM") as mxps:
              bld = prep['bld_tile'](d)
              # stage MAX-type recip norms to DRAM (bf16) for broadcasts
              rnb = mxp.tile([20, 8, 64], BF16, name=f"rnb_{d}", tag="rnb")
              nc.vector.tensor_copy(rnb.rearrange("l s t -> l (s t)"),
                                    rn1[:, 1, :, :].rearrange(
                                        "l s t -> l (s t)"))
              rnd = dramp.tile([20, 8, 64], BF16, name=f"rnd_{d}")
              nc.sync.dma_start(rnd[:], rnb[:])
              mxs_all = {}
              for bp in range(2):
                  for side in range(2):
                      rs_me = 2 * bp if side == 0 else 4 + 2 * bp
                      rs_ot = 4 + 2 * bp if side == 0 else 2 * bp
                      for hf in range(2):
                          pps = mxps.tile([128, 10, 128], F32,
                                          name=f"pp_{d}_{bp}_{side}_{hf}",
                                          tag="pps", bufs=2)
                          for u in range(10):
                              l = 10 * hf + u
                              for c in range(2):
                                  nc.tensor.matmul(
                                      pps[:, u, :],
                                      bld[:, c, l, rs_me:rs_me + 2,
                                          :].rearrange("k e t -> k (e t)"),
                                      cB[:, c, rs_ot:rs_ot + 2,
                                         :].rearrange("k e t -> k (e t)"),
                                      start=(c == 0), stop=(c == 1))
                          for b2 in range(2):
                              b = 2 * bp + b2
                              s_ot = rs_ot + b2
                              key = (side, b)
                              if key not in mxs_all:
                                  mxs_all[key] = mxp.tile(
                                      [64, 2, 10], F32,
                                      name=f"mxs_{d}_{side}_{b}",
                                      tag=f"mxs_{side}_{b2}")
                              nbcb = mxp.tile([64, 10, 64], BF16,
                                              name=f"nb_{d}_{bp}_{side}"
                                                   f"_{hf}_{b2}",
                                              tag="nbcb", bufs=3)
                              nc.sync.dma_start(
                                  nbcb[:],
                                  bass.AP(tensor=rnd.tensor,
                                          offset=rnd.offset
                                          + (10 * hf) * 512 + s_ot * 64,
                                          ap=[[0, 64], [512, 10], [1, 64]]))
                              # stage pps to SBUF bf16 on the idle ACT so
                              # the multiply runs 2x from SBUF instead of
                              # 1x from f32 psum (791ns -> ~390ns on DVE)
                              ppsc = mxp.tile([64, 10, 64], BF16,
                                              name=f"pc_{d}_{bp}_{side}"
                                                   f"_{hf}_{b2}",
                                              tag="ppsc", bufs=3)
                              nc.scalar.activation(
                                  ppsc[:],
                                  pps[64 * b2:64 * b2 + 64, :,
                                      64 * b2:64 * b2 + 64], AF.Copy)
                              pn = mxp.tile([64, 10, 64], BF16,
                                            name=f"pn_{d}_{bp}_{side}"
                                                 f"_{hf}_{b2}",
                                            tag="pn", bufs=3)
                              rot['dve'] += 0.4
                              nc.vector.tensor_tensor(
                                  out=pn[:], in0=ppsc[:],
                                  in1=nbcb[:], op=ALU.mult)
                              # bf16 TT tree-max (2x DVE) beats the 1x
                              # tensor_reduce on 640-elem tiles
                              cur = pn
                              for w in (32, 16, 8, 4, 2):
                                  nxt = mxp.tile(
                                      [64, 10, w], BF16,
                                      name=f"mt_{d}_{bp}_{side}"
                                           f"_{hf}_{b2}_{w}",
                                      tag=f"mt{w}", bufs=2)
                                  nc.vector.tensor_tensor(
                                      out=nxt[:], in0=cur[:, :, 0:w],
                                      in1=cur[:, :, w:2 * w], op=ALU.max)
                                  cur = nxt
                              nc.vector.tensor_tensor(
                                  out=mxs_all[key][:, hf, :],
                                  in0=cur[:, :, 0:1].rearrange(
                                      "t u o -> t (u o)"),
                                  in1=cur[:, :, 1:2].rearrange(
                                      "t u o -> t (u o)"),
                                  op=ALU.max)
              for side in range(2):
                  for b in range(BL):
                      yt = mxps.tile([20, 64], F32,
                                     name=f"yt_{d}_{b}_{side}", tag="yt",
                                     bufs=2)
                      nc.tensor.transpose(
                          yt[:],
                          mxs_all[(side, b)].rearrange(
                              "t hf u -> t (hf u)"),
                          idf[0:64, 0:64])
                      ri_me = 0 if side == 0 else 1
                      s_me = b if side == 0 else BL + b
                      nc.vector.tensor_tensor(
                          out=mcol(mt, 32, ri_me, b), in0=yt[:],
                          in1=rn1[:, 1, s_me, :], op=ALU.mult)
          if d == 'f':
              # dir b's MAX builds now: Pool is idle while DVE chews on
              # dir f's AM blocks; the shared bld buffer (tag bufs=1)
              # WARs behind f's pps reads automatically
              for cl in prep['bld_units']('b', 0, 64, mix=True):
                  cl()
          # ---- AM + AX per batch item
          with tc.tile_pool(name=f"am_{d}", bufs=3) as amp, \
               tc.tile_pool(name=f"amps_{d}", bufs=2, space="PSUM") as amps:
              for b in range(BL):
                  _am_ax_block(nc, tc, d, di, b, cT, cB, w2b, n1, rn1,
                               rvn, ctm, mt, idf, idb, amp, amps, dramp,
                               mcol, anchor_t, veng, stage)



    return {'stage': stage, 'n1s': n1s, 'rot': rot, 'veng': veng,
            'mcol': mcol, 'dramp': dramp}


def _am_ax_block(nc, tc, d, di, b, cT, cB, w2b, n1, rn1, rvn, ctm, mt, idf,
                 idb, amp, amps, dramp, mcol, anchor_t, veng, stage):
    AM_SLOT, AX_SLOT = 64, 96
    sp, sh = b, BL + b

    # raw attention + normalization (attn = rvn_p[i] * raw * rvn_h[j])
    att_ps = amps.tile([64, 64], F32, name=f"at_{d}_{b}", tag="t64", bufs=3)
    for c in range(2):
        nc.tensor.matmul(att_ps[:], cB[:, c, sp, :], cB[:, c, sh, :],
                         start=(c == 0), stop=(c == 1))
    a1 = amp.tile([64, 64], F32, name=f"a1_{d}_{b}", tag="a1")
    nc.scalar.activation(a1[:], att_ps[:], AF.Copy, scale=rvn[:, sp:sp + 1])
    a1t_ps = amps.tile([64, 64], F32, name=f"a1t_{d}_{b}", tag="t64", bufs=3)
    nc.tensor.transpose(a1t_ps[:], a1[:], idf[0:64, 0:64])
    attTn = amp.tile([64, 64], F32, name=f"aTn_{d}_{b}", tag="attTn")
    nc.scalar.activation(attTn[:], a1t_ps[:], AF.Copy,
                         scale=rvn[:, sh:sh + 1])
    attn_ps = amps.tile([64, 64], F32, name=f"an2_{d}_{b}", tag="t64", bufs=3)
    nc.tensor.transpose(attn_ps[:], attTn[:], idf[0:64, 0:64])
    attn = amp.tile([64, 64], F32, name=f"an_{d}_{b}", tag="attn")
    nc.scalar.activation(attn[:], attn_ps[:], AF.Copy)

    # row sums + clamped recips
    rs_h = amp.tile([64, 1], F32, name=f"rh_{d}_{b}", tag="rsh")
    nc.vector.tensor_reduce(out=rs_h[:], in_=attn[:], axis=AX_X, op=ALU.add)
    nc.vector.tensor_scalar_max(rs_h[:], rs_h[:], EPS)
    nc.vector.reciprocal(rs_h[:], rs_h[:])
    rs_p = amp.tile([64, 1], F32, name=f"rp_{d}_{b}", tag="rsp")
    nc.vector.tensor_reduce(out=rs_p[:], in_=attTn[:], axis=AX_X, op=ALU.add)
    nc.vector.tensor_scalar_max(rs_p[:], rs_p[:], EPS)
    nc.vector.reciprocal(rs_p[:], rs_p[:])

    # weighted mean rhs: ahT = T(attn * rs_h) bf16, bpT = T(attTn * rs_p)
    ah = amp.tile([64, 64], F32, name=f"ah_{d}_{b}", tag="ah")
    nc.scalar.activation(ah[:], attn[:], AF.Copy, scale=rs_h[:, 0:1])
    ahT_ps = amps.tile([64, 64], F32, name=f"ahT_{d}_{b}", tag="t64", bufs=3)
    nc.tensor.transpose(ahT_ps[:], ah[:], idf[0:64, 0:64])
    ahT = amp.tile([64, 64], BF16, name=f"ahTs_{d}_{b}", tag="ahTs")
    nc.scalar.activation(ahT[:], ahT_ps[:], AF.Copy)
    bp_ = amp.tile([64, 64], F32, name=f"bp_{d}_{b}", tag="bp")
    nc.scalar.activation(bp_[:], attTn[:], AF.Copy, scale=rs_p[:, 0:1])
    bpT_ps = amps.tile([64, 64], F32, name=f"bpT_{d}_{b}", tag="t64", bufs=3)
    nc.tensor.transpose(bpT_ps[:], bp_[:], idf[0:64, 0:64])
    bpT = amp.tile([64, 64], BF16, name=f"bpTs_{d}_{b}", tag="bpTs")
    nc.scalar.activation(bpT[:], bpT_ps[:], AF.Copy)

    # am vectors + cosine under w_am
    for role, (rhs, s_ctm, s_me) in enumerate(
            ((ahT, sh, sp), (bpT, sp, sh))):
        amv_ps = amps.tile([128, 2, 64], F32, name=f"av_{d}_{b}_{role}",
                           tag="amv", bufs=2)
        for c in range(2):
            nc.tensor.matmul(amv_ps[:, c, :],
                             ctm[:, s_ctm, 128 * c:128 * (c + 1)], rhs[:],
                             start=True, stop=True)
        amv = amp.tile([128, 2, 64], F32R, name=f"am_{d}_{b}_{role}",
                       tag="amv_sb")
        nc.scalar.activation(amv.rearrange("k c t -> k (c t)"),
                             amv_ps.rearrange("k c t -> k (c t)"), AF.Copy)
        prod = amp.tile([128, 2, 64], BF16, name=f"pr_{d}_{b}_{role}",
                        tag="prod")
        for c in range(2):
            nc.vector.tensor_tensor(out=prod[:, c, :], in0=cB[:, c, s_me, :],
                                    in1=amv[:, c, :], op=ALU.mult)
        nump = amps.tile([20, 64], F32, name=f"nu_{d}_{b}_{role}", tag="s20",
                         bufs=2)
        for c in range(2):
            nc.tensor.matmul(nump[:], w2b[:, c, di, 40:60],
                             prod[:, c, :], start=(c == 0), stop=(c == 1))
        amsq = amp.tile([128, 2, 64], BF16, name=f"as_{d}_{b}_{role}",
                        tag="amsq")
        nc.scalar.activation(amsq.rearrange("k c t -> k (c t)"),
                             amv.rearrange("k c t -> k (c t)"), AF.Square)
        n2p = amps.tile([20, 64], F32, name=f"n2_{d}_{b}_{role}", tag="s20",
                        bufs=2)
        for c in range(2):
            nc.tensor.matmul(n2p[:], w2b[:, c, di, 40:60],
                             amsq[:, c, :], start=(c == 0), stop=(c == 1))
        n2s = amp.tile([20, 64], F32, name=f"ns_{d}_{b}_{role}", tag="n2s")
        nc.scalar.activation(n2s[:], n2p[:], AF.Sqrt)
        den = amp.tile([20, 64], F32, name=f"dn_{d}_{b}_{role}", tag="amden")
        nc.vector.tensor_tensor(out=den[:], in0=n1[:, 2, s_me, :],
                                in1=n2s[:], op=ALU.mult)
        nc.vector.tensor_scalar_max(den[:], den[:], EPS)
        nc.vector.reciprocal(den[:], den[:])
        nc.vector.tensor_tensor(out=mcol(mt, AM_SLOT, role, b), in0=nump[:],
                                in1=den[:], op=ALU.mult)

    # ---- stage normalized attention (bf16) to DRAM for the AX phases
    atb = amp.tile([64, 64], BF16, name=f"ab_{d}_{b}", tag="atb")
    nc.vector.tensor_copy(atb[:], attn[:])
    atbT = amp.tile([64, 64], BF16, name=f"abT_{d}_{b}", tag="atbT")
    nc.vector.tensor_copy(atbT[:], attTn[:])
    dsc = dramp.tile([64, 64], BF16, name=f"dx_{d}_{b}")
    nc.sync.dma_start(dsc[:], atb[:])
    dscT = dramp.tile([64, 64], BF16, name=f"dxT_{d}_{b}")
    nc.sync.dma_start(dscT[:], atbT[:])
    stage[(d, b)] = (dsc, dscT)



def _make_ax_emit(nc, tc, ctx, conB, n1s, w2b, mvT, mctx):
    """AX feature (max-attentive cosine), chunked by groups of 8 output
    tokens so the middle chunks interleave with the agg scan's emission.

    ax_unit(d, b, role, ch, during): products + joint bf16 tree-max +
    numerator/norm matmuls accumulated into per-(d, chunk) psum tiles.
    ax_tail(ch): ONE batched Sqrt per dir (both dirs adjacent in ACT
    program order — Sqrt lives in a different ACT table set than the
    scan's Sigmoid/Tanh, so scattering per-unit Sqrts through the scan
    would thrash 1.3us table reloads), then den/recip/feature write for
    all 8 (role, b) units of the chunk at once.

    `during=True` alternates products Pool/DVE for scan-concurrent
    execution; `during=False` uses the greedy DVE/Pool balancer.
    """
    veng = mctx['veng']
    stage, rot = mctx['stage'], mctx['rot']
    axp = ctx.enter_context(tc.tile_pool(name="axp", bufs=2))
    axps = ctx.enter_context(tc.tile_pool(name="axps", bufs=2, space="PSUM"))
    pcnt = [0]
    acc = {}

    def ax_unit(d, b, role, ch, pair, during=False):
        di = 0 if d == 'f' else 1
        i0 = 8 * ch
        cB = conB[d]
        u = role * BL + b
        if pair not in acc:
            acc[pair] = axps.tile([20, 4, 2, 8, 8], F32,
                                  name=f"acc_{pair[0]}_{pair[1]}",
                                  tag="axacc", bufs=2)
        slot = 2 * di + (0 if ch == pair[0] else 1)
        nuxc = acc[pair][:, slot, 0, :, :]
        n2c = acc[pair][:, slot, 1, :, :]
        sp, sh = b, BL + b
        dsc, dscT = stage[(d, b)]
        src = dsc if role == 0 else dscT
        s_v = sh if role == 0 else sp
        s_me = sp if role == 0 else sh
        # broadcast the staged attn rows [i0:i0+8) to all 128 partitions
        bc = axp.tile([128, 8, 64], BF16,
                      name=f"bc_{d}_{b}_{role}_{ch}", tag="bc", bufs=5)
        nc.sync.dma_start(
            bc[:], bass.AP(tensor=src.tensor, offset=src.offset + i0 * 64,
                           ap=[[0, 128], [64, 8], [1, 64]]))
        prod = axp.tile([128, 2, 8, 64], BF16,
                        name=f"xp_{d}_{b}_{role}_{ch}", tag="xprod", bufs=3)
        pcnt[0] += 1
        eng = nc.gpsimd if pcnt[0] % 3 != 0 else nc.vector
        vb = cB[:, :, s_v, :]
        eng.tensor_tensor(
            out=prod[:],
            in0=bass.AP(tensor=vb.tensor, offset=vb.offset,
                        ap=[vb.ap[0], vb.ap[1], [0, 8], vb.ap[2]]),
            in1=bass.AP(tensor=bc.tensor, offset=bc.offset,
                        ap=[bc.ap[0], [0, 2], bc.ap[1], bc.ap[2]]),
            op=ALU.mult)
        rot['dve'] += 0.62  # tree max: DVE only
        cur = prod
        for w in (32, 16, 8, 4, 2):
            nxt = axp.tile([128, 2, 8, w], BF16,
                           name=f"tm_{d}_{b}_{role}_{ch}_{w}",
                           tag=f"tm{w}", bufs=2)
            nc.vector.tensor_tensor(out=nxt[:], in0=cur[:, :, :, 0:w],
                                    in1=cur[:, :, :, w:2 * w], op=ALU.max)
            cur = nxt
        axm = axp.tile([128, 2, 8], F32R,
                       name=f"axm_{d}_{b}_{role}_{ch}", tag="axm", bufs=3)
        nc.vector.tensor_tensor(
            out=axm[:],
            in0=cur[:, :, :, 0:1].rearrange("k c t o -> k c (t o)"),
            in1=cur[:, :, :, 1:2].rearrange("k c t o -> k c (t o)"),
            op=ALU.max)
        prodx = axp.tile([128, 2, 8], BF16,
                         name=f"px_{d}_{b}_{role}_{ch}", tag="prodx", bufs=3)
        nc.vector.tensor_tensor(out=prodx[:], in0=cB[:, :, s_me, i0:i0 + 8],
                                in1=axm[:], op=ALU.mult)
        for c in range(2):
            nc.tensor.matmul(nuxc[:, u, :], w2b[:, c, di, 60:80],
                             prodx[:, c, :], start=(c == 0), stop=(c == 1))
        axsq = axp.tile([128, 2, 8], BF16,
                        name=f"xs_{d}_{b}_{role}_{ch}", tag="axsq", bufs=3)
        nc.scalar.activation(axsq.rearrange("k c t -> k (c t)"),
                             axm.rearrange("k c t -> k (c t)"), AF.Square)
        for c in range(2):
            nc.tensor.matmul(n2c[:, u, :], w2b[:, c, di, 60:80],
                             axsq[:, c, :], start=(c == 0), stop=(c == 1))

    def ax_tail(pair):
        at = acc.pop(pair)
        sq = {}
        for di, d in enumerate('fb'):
            for cpos, ch in enumerate(pair):
                n2s = axp.tile([20, 8, 8], F32, name=f"n2s_{d}_{ch}",
                               tag="n2s", bufs=4)
                nc.scalar.activation(n2s[:], at[:, 2 * di + cpos, 1, :, :],
                                     AF.Sqrt)
                sq[(d, ch)] = n2s
        for di, d in enumerate('fb'):
            n1 = n1s[d]
            for cpos, ch in enumerate(pair):
                i0 = 8 * ch
                nuxc = at[:, 2 * di + cpos, 0, :, :]
                n2s = sq[(d, ch)]
                den = axp.tile([20, 8, 8], F32, name=f"dnc_{d}_{ch}",
                               tag="denc", bufs=2)
                nc.vector.tensor_tensor(
                    out=den[:], in0=n1[:, 3, :, i0:i0 + 8],
                    in1=n2s[:], op=ALU.mult)
                nc.vector.tensor_scalar_max(
                    den.rearrange("l s t -> l (s t)"),
                    den.rearrange("l s t -> l (s t)"), EPS)
                nc.vector.reciprocal(den.rearrange("l s t -> l (s t)"),
                                     den.rearrange("l s t -> l (s t)"))
                out = mvT[di][96:116, 8 * i0:8 * i0 + 64].rearrange(
                    "l (t s) -> l t s", s=8)
                nc.vector.tensor_tensor(out=out,
                                        in0=nuxc.rearrange("l s t -> l t s"),
                                        in1=den.rearrange("l s t -> l t s"),
                                        op=ALU.mult)

    return ax_unit, ax_tail


# ---------------------------------------------------------------- entry

def _get_nc(debug=False):
    key = ('dbg' if debug else 'rel')
    if key not in _CACHE:
        _CACHE[key] = build_nc(debug)
    return _CACHE[key]


def kernel(**inputs):
    nc = _get_nc(False)
    w = _prep_weights(inputs)
    in_maps = []
    for core in range(NCORES):
        m = dict(w)
        m['tokp'] = _prep_tokens(inputs['q1_inputs'], inputs['q2_inputs'],
                                 core)
        in_maps.append(m)
    res = run_bass_kernel_spmd(nc, in_maps, core_ids=list(range(NCORES)))
    out = np.concatenate([res.results[c]['y'] for c in range(NCORES)], axis=0)
    return out.astype(np.float32)


def run_debug(inputs):
    nc = _get_nc(True)
    w = _prep_weights(inputs)
    in_maps = []
    for core in range(NCORES):
        m = dict(w)
        m['tokp'] = _prep_tokens(inputs['q1_inputs'], inputs['q2_inputs'],
                                 core)
        in_maps.append(m)
    res = run_bass_kernel_spmd(nc, in_maps, core_ids=list(range(NCORES)))
    return res



# revision 113
# speedup vs baseline: 1.0131x; 1.0091x over previous
"""BiMPM Trainium2 Bass kernel — pure data parallel over batch (B=32 -> 4/core).

Per-core layouts (B_l=4, stack S=8 rows per step = [p:b0..3, h:b0..3]):
- token/row order: r = t*8 + s, s = seq*4 + b (seq0 = q1 = "p", seq1 = q2 = "h")
- xgT (input projections): (128 = g%128, 8 gc, 512 col=t*8+s) bf16 per dir,
  t-quartered in scan-consumption order so the ctx scan starts early
- scan: fused fw+bw per step; gates psum (128, 2dir, 8gc, 8s) in one 2KB
  bank with PER-DIR start/stop chains so each dir's sigmoid fires without
  waiting for the other's matmuls; g-gates host-prescaled x2 so ONE Sigmoid
  covers a dir's gates (tanh(g) = 2*sigmoid(2g)-1); h double-buffered and
  output copies on DVE (keeps the h-write's tanh RAW wait attached to the
  instruction instead of spilling to a SEQ-blocking EventSemaphore)
- conT f32r / conB bf16 (ctx outputs, hd-major): (128, 2c, 8s, 64t) per dir
- matching prep (csq/n1 norms, ctm transposes, rvn token norms, bld MAX
  builds) emitted in 16/32-token chunks INTERLEAVED into the ctx scan's
  engine idle time via a per-step hook; sqrt/recip finals batched post-scan
  (Sqrt shares no ACT table set with Sigmoid/Tanh — 1.3us reload each)
- matching: FULL/MAX/AM as before (MAX reduce = bf16 TT tree; AM scalar
  normalizations on ACT via per-partition scale APs; bld on idle Pool,
  dirs share one 40KB buffer)
- AX in 8-token chunk pairs (c, 7-c): products (Pool-biased 2:1) + joint
  2-channel bf16 tree-max; numerator/norm matmuls accumulate into one psum
  bank per pair; ONE batched Sqrt site per pair. Head pair (0,7) runs
  before the agg scan; mid pairs + t-chunked agg projections are emitted
  from the agg scan's per-step hook, paced so pair (c, 7-c) is fully
  emitted before scan step 8c reads its xgaT chunk (emission order IS the
  dependency order for the tile tracker — late emission = uninit reads)
- weights shipped bf16 from host (wih/whh/awhh/fc; agg proj stays f32r);
  fc head all-bf16 against bf16 hfin
- mvT (match features): 2 tiles (128, 512) f32r, feature rows at 32-aligned
  slots [full@0, max@32, am@64, ax@96, ones@116]

TimelineSim: 574376 ns (baseline 618195); HW rel err 7.4e-3 (gate 2e-2).
word_emb shipped bf16 (gather-then-round == round-then-gather: identical).
"""
import ml_dtypes
import numpy as np
from contextlib import ExitStack

BF16_NP = ml_dtypes.bfloat16

import concourse.bass as bass
import concourse.tile as tile
from concourse import bacc, mybir
from concourse.bass_utils import run_bass_kernel_spmd
from concourse.masks import make_identity

F32 = mybir.dt.float32
F32R = mybir.dt.float32r
BF16 = mybir.dt.bfloat16
I32 = mybir.dt.int32
AF = mybir.ActivationFunctionType
ALU = mybir.AluOpType
AX_X = mybir.AxisListType.X

B, T, V, D, H, L, NL = 32, 64, 50000, 300, 256, 20, 2
NCORES = 8
BL = B // NCORES
S = 2 * BL
EPS = 1e-8

_CACHE = {}
PHASES = 'full'  # 'ctx' | 'match' | 'full' (for TimelineSim bisection)


# ---------------------------------------------------------------- host prep

def _gate_reorder(w):
    # PyTorch gate order i,f,g,o -> chunk order [i, f, o, 2*g].
    # The x2 on g lets the scan use one Sigmoid for all gates:
    # tanh(g) == 2*sigmoid(2g) - 1.
    i, f, g, o = np.split(w, 4, axis=0)
    return np.concatenate([i, f, o, 2.0 * g], axis=0)


def _prep_weights(inp):
    w = {}
    f32 = np.float32

    def ctx_wT(dir_):
        # ws layout: [k%128, kc(3), gc(8), m(128)]; row 300 = bias, pad to 384
        wih = _gate_reorder(np.asarray(inp[f'ctx_wih_{dir_}'], f32))
        bias = _gate_reorder(
            np.asarray(inp[f'ctx_bih_{dir_}'] + inp[f'ctx_bhh_{dir_}'],
                       f32)[:, None]).T
        wt = np.concatenate([wih.T, bias, np.zeros((83, 1024), f32)], 0)
        return np.ascontiguousarray(
            wt.reshape(3, 128, 8, 128).transpose(1, 0, 2, 3)).astype(BF16_NP)

    def whhT(pfx, dir_):
        # ws layout: [k%128, kc, gc, m] = whh_reord[gc*128+m, kc*128+k]
        whh = _gate_reorder(np.asarray(inp[f'{pfx}_whh_{dir_}'], f32))
        return np.ascontiguousarray(
            whh.T.reshape(2, 128, 8, 128).transpose(1, 0, 2, 3)).astype(
                BF16_NP)

    w['wihT_f'], w['wihT_b'] = ctx_wT('f'), ctx_wT('b')
    w['whhT_f'], w['whhT_b'] = whhT('ctx', 'f'), whhT('ctx', 'b')
    w['awhhT_f'], w['awhhT_b'] = whhT('agg', 'f'), whhT('agg', 'b')

    def agg_wT(dir_):
        wih = _gate_reorder(np.asarray(inp[f'agg_wih_{dir_}'], f32))
        bias = _gate_reorder(
            np.asarray(inp[f'agg_bih_{dir_}'] + inp[f'agg_bhh_{dir_}'],
                       f32)[:, None]).T
        out = np.zeros((256, 1024), f32)
        for d in range(2):
            for ty in range(4):
                src = wih[:, d * 80 + ty * 20: d * 80 + ty * 20 + 20]
                out[d * 128 + 32 * ty: d * 128 + 32 * ty + 20] = src.T
        out[116] = bias[0]
        return np.ascontiguousarray(
            out.reshape(2, 128, 8, 128).transpose(1, 0, 2, 3), f32)

    w['aggwT_f'], w['aggwT_b'] = agg_wT('f'), agg_wT('b')

    # w2T80: (128 = h%128, 2 c, 2 dir, 80 = ty*20+l), ty in [full,max,am,ax]
    w2 = np.asarray(inp['mp_w'], f32) ** 2
    w2t = np.zeros((128, 2, 2, 80), f32)
    for d in range(2):
        for ty in range(4):
            src = w2[2 * ty + d]
            for c in range(2):
                w2t[:, c, d, ty * 20:(ty + 1) * 20] = \
                    src[:, c * 128:(c + 1) * 128].T
    w['w2T'] = np.ascontiguousarray(w2t)

    fc1 = np.asarray(inp['fc1_w'], f32)
    w['fc1T'] = np.ascontiguousarray(
        fc1.T.reshape(8, 128, 512).transpose(1, 0, 2)).astype(BF16_NP)
    w['fc1b'] = np.ascontiguousarray(
        np.broadcast_to(np.asarray(inp['fc1_b'], f32), (BL, 512))).astype(
            BF16_NP)
    fc2 = np.asarray(inp['fc2_w'], f32)
    w['fc2T'] = np.ascontiguousarray(
        fc2.T.reshape(4, 128, 2).transpose(1, 0, 2)).astype(BF16_NP)
    w['fc2b'] = np.ascontiguousarray(
        np.broadcast_to(np.asarray(inp['fc2_b'], f32), (BL, 2))).astype(
            BF16_NP)
    w['word_emb'] = np.ascontiguousarray(
        np.asarray(inp['word_emb'], f32)).astype(BF16_NP)
    return w


def _prep_tokens(q1, q2, core):
    q1c = np.asarray(q1[core * BL:(core + 1) * BL]).astype(np.int64)
    q2c = np.asarray(q2[core * BL:(core + 1) * BL]).astype(np.int64)
    tok = np.zeros((T * S,), np.int32)
    for seq, q in ((0, q1c), (1, q2c)):
        for b in range(BL):
            tok[np.arange(T) * S + seq * BL + b] = q[b]
    return np.ascontiguousarray(tok.reshape(4, 128))


# ---------------------------------------------------------------- build

def build_nc(debug=False):
    nc = bacc.Bacc("TRN2", target_bir_lowering=False, debug=False,
                   enable_asserts=True, num_devices=NCORES)
    dt = nc.dram_tensor
    dr = {}
    dr['tokp'] = dt("tokp", [4, 128], I32, kind="ExternalInput").ap()
    dr['word_emb'] = dt("word_emb", [V, D], BF16,
                        kind="ExternalInput").ap()
    for n, shp in [('wihT_f', [128, 3, 8, 128]), ('wihT_b', [128, 3, 8, 128]),
                   ('whhT_f', [128, 2, 8, 128]), ('whhT_b', [128, 2, 8, 128]),
                   ('awhhT_f', [128, 2, 8, 128]),
                   ('awhhT_b', [128, 2, 8, 128]),
                   ('fc1T', [128, 8, 512]), ('fc1b', [BL, 512]),
                   ('fc2T', [128, 4, 2]), ('fc2b', [BL, 2])]:
        dr[n] = dt(n, shp, BF16, kind="ExternalInput").ap()
    for n, shp in [('aggwT_f', [128, 2, 8, 128]), ('aggwT_b', [128, 2, 8, 128]),
                   ('w2T', [128, 2, 2, 80])]:
        dr[n] = dt(n, shp, F32, kind="ExternalInput").ap()
    y = dt("y", [BL, NL], F32, kind="ExternalOutput").ap()
    dbg = {}
    if debug:
        dbg['conT_f'] = dt("dbg_conT_f", [128, 2, 8, 64], F32,
                           kind="ExternalOutput").ap()
        dbg['conT_b'] = dt("dbg_conT_b", [128, 2, 8, 64], F32,
                           kind="ExternalOutput").ap()
        dbg['mvT0'] = dt("dbg_mvT0", [128, 512], F32,
                         kind="ExternalOutput").ap()
        dbg['mvT1'] = dt("dbg_mvT1", [128, 512], F32,
                         kind="ExternalOutput").ap()
        dbg['xT'] = dt("dbg_xT", [128, 2, 40], F32,
                       kind="ExternalOutput").ap()

    with tile.TileContext(nc) as tc, ExitStack() as ctx:
        _body(nc, tc, ctx, dr, y, dbg)
    nc.compile()
    return nc


def _body(nc, tc, ctx, dr, y, dbg):
    perm = ctx.enter_context(tc.tile_pool(name="perm", bufs=1))

    idf = perm.tile([128, 128], F32, name="idf")
    make_identity(nc, idf[:])
    idb = perm.tile([128, 128], BF16, name="idb")
    nc.vector.tensor_copy(idb[:], idf[:])
    selb = idb.rearrange("k (tl s) -> k tl s", s=8)

    def conv(src, dtype, name, engine=None, pool=None):
        t = (pool or perm).tile(list(src.shape), dtype, name=f"C_{name}")
        eng = engine or nc.vector
        if eng is nc.scalar:
            eng.activation(t[:], src[:], AF.Copy)
        else:
            eng.tensor_copy(t[:], src[:])
        return t

    wihT, whhTb, awhhTb, aggwT = {}, {}, {}, {}
    w2Tf = perm.tile([128, 2, 2, 80], F32, name="w2Tf")
    nc.sync.dma_start(w2Tf[:], dr['w2T'][:])
    w2Tr = conv(w2Tf, F32R, "w2Tr")
    w2Tb = conv(w2Tf, BF16, "w2Tb", nc.gpsimd)

    idx_sb = perm.tile([128, 4], I32, name="idx_sb")
    nc.sync.dma_start(idx_sb[:], dr['tokp'].rearrange("m p -> p m"))

    # ---------------- weight load + embedding gather + ctx projection (bf16)
    # xgT[d]: (128 = g%128, 8 gc, 512 cols) bf16 ; col r = t*8 + s
    # Gather tiles share scope with weight staging (no SBUF reuse between
    # the indirect-DMA writes and freed staging tiles).
    xgT = {'f': perm.tile([128, 8, 512], BF16, name="xgT_f"),
           'b': perm.tile([128, 8, 512], BF16, name="xgT_b")}
    with tc.tile_pool(name="embp", bufs=2) as embp, \
         tc.tile_pool(name="loadp", bufs=1) as loadp, \
         tc.tile_pool(name="epsum", bufs=2, space="PSUM") as epsum:
        # embT (128 = d%128, 3 kc, 512 tok) bf16
        embT = embp.tile([128, 3, 512], BF16, name="embT", tag="embT")
        embs = []
        for m in range(4):
            emb = embp.tile([128, 304], BF16, name=f"emb_{m}", tag=f"emb{m}")
            nc.gpsimd.indirect_dma_start(
                out=emb[:, 0:300], out_offset=None, in_=dr['word_emb'][:],
                in_offset=bass.IndirectOffsetOnAxis(ap=idx_sb[:, m:m + 1],
                                                    axis=0))
            nc.vector.memset(emb[:, 300:301], 1.0)
            embs.append(emb)

        def load_f32(name, shp, tag):
            t = loadp.tile(shp, F32, name=f"L_{name}", tag=tag)
            nc.sync.dma_start(t[:], dr[name][:])
            return t

        def load_bf16(name, shp, pool):
            t = pool.tile(shp, BF16, name=f"B_{name}")
            nc.sync.dma_start(t[:], dr[name][:])
            return t

        for d in 'fb':
            wihT[d] = load_bf16(f'wihT_{d}', [128, 3, 8, 128], embp)
            whhTb[d] = load_bf16(f'whhT_{d}', [128, 2, 8, 128], perm)
            awhhTb[d] = load_bf16(f'awhhT_{d}', [128, 2, 8, 128], perm)
            aggwT[d] = conv(load_f32(f'aggwT_{d}', [128, 2, 8, 128], "raw8k"),
                            F32R, f"aggw_{d}", nc.scalar)

        for m in range(4):
            embb = embs[m]
            for c in range(3):
                kc = min(128, 301 - 128 * c)
                tp = epsum.tile([128, 128], BF16, name=f"etp_{m}_{c}",
                                tag="etp")
                nc.tensor.transpose(tp[0:kc, :],
                                    embb[:, 128 * c:128 * c + kc], idb[:])
                if c % 2 == 0:
                    nc.scalar.activation(embT[0:kc, c, 128 * m:128 * (m + 1)],
                                         tp[0:kc, :], AF.Copy)
                else:
                    nc.vector.tensor_copy(
                        embT[0:kc, c, 128 * m:128 * (m + 1)], tp[0:kc, :])
        # t-quartered, scan-consumption-ordered (f ascending, b descending)
        # so the ctx scan's first steps start before the full projection
        qorder = [(0, 0), (1, 3), (0, 1), (1, 2), (0, 2), (1, 1), (0, 3),
                  (1, 0)]
        for di, q in qorder:
            d = 'fb'[di]
            for gc in range(8):
                ps = epsum.tile([128, 128], F32, name=f"xps_{d}_{gc}_{q}",
                                tag="xps")
                for c in range(3):
                    kc = min(128, 301 - 128 * c)
                    nc.tensor.matmul(ps[:], wihT[d][0:kc, c, gc, :],
                                     embT[0:kc, c, 128 * q:128 * (q + 1)],
                                     start=(c == 0), stop=(c == 2))
                if gc % 2 == 0:
                    nc.vector.tensor_copy(
                        xgT[d][:, gc, 128 * q:128 * (q + 1)], ps[:])
                else:
                    nc.scalar.activation(
                        xgT[d][:, gc, 128 * q:128 * (q + 1)], ps[:], AF.Copy)

    # ---------------- scan layer (shared ctx/agg), fused fw+bw per step
    # state h/c: (128 = hd%128, 2 dir, 2 kc, 8 s)
    # gates psum: (128 = g%128, 2 dir, 8 gc, 8 s), order [i0 i1 f0 f1 o0 o1 g0 g1]
    # g-gates pre-scaled x2 at host: tanh(g) = 2*sigmoid(2g) - 1, so one
    # Sigmoid covers all 8 chunks; xg injected via identity matmul (start=True).
    def scan_layer(xgd, whh_d, conT_out, conB_out, hfin, lname, ve=None,
                   hook=None):
        ve = ve or nc.vector
        sp = ctx2.enter_context(tc.tile_pool(name=f"sp_{lname}", bufs=12))
        pp = ctx2.enter_context(tc.tile_pool(name=f"pp_{lname}", bufs=3,
                                             space="PSUM"))
        cp = ctx2.enter_context(tc.tile_pool(name=f"cp_{lname}", bufs=1))
        c_sb = cp.tile([128, 2, 2, 8], F32, name=f"c_{lname}")
        # h double-buffered: the step-t write must not WAR against step-t's
        # own whh matmul reads (a 2-sem wait the tile framework lowers to a
        # SEQ-blocking EventSemaphore on DVE, ~200ns/step on the chain)
        h_bufs = [cp.tile([128, 2, 2, 8], BF16, name=f"h_{lname}_{i}")
                  for i in range(2)]
        nc.vector.memset(c_sb[:], 0.0)
        nc.vector.memset(h_bufs[0][:], 0.0)
        nc.vector.memset(h_bufs[1][:], 0.0)
        for tau in range(T):
            ts_ = {'f': tau, 'b': T - 1 - tau}
            h_prev = h_bufs[(tau + 1) % 2]
            h_sb = h_bufs[tau % 2]
            # one full psum bank (2KB); each dir's 1KB region runs its own
            # start/stop chain so dir f's sigmoid fires without waiting for
            # dir b's matmuls — the two cell-update chains then overlap.
            psb = pp.tile([128, 512], F32, name=f"g_{lname}_{tau}",
                          tag="gps")
            ps = psb[:, 0:128].rearrange("k (d g s) -> k d g s", d=2, g=8)
            sig = sp.tile([128, 2, 8, 8], F32, name=f"si_{lname}_{tau}",
                          tag="sig")
            t1 = sp.tile([128, 2, 2, 8], F32, name=f"t1_{lname}_{tau}",
                         tag="t1")
            t2h = sp.tile([128, 2, 2, 8], F32, name=f"t2_{lname}_{tau}",
                          tag="t2h")
            th = sp.tile([128, 2, 2, 8], F32, name=f"th_{lname}_{tau}",
                         tag="th")
            for di, d in enumerate('fb'):
                t = ts_[d]
                nc.tensor.matmul(ps[:, di, :, :], idb[:],
                                 xgd[d][:, :, 8 * t:8 * t + 8],
                                 start=True, stop=False)
                for gc in range(8):
                    for kc in range(2):
                        nc.tensor.matmul(
                            ps[:, di, gc, :], whh_d[d][:, kc, gc, :],
                            h_prev[:, di, kc, :], start=False,
                            stop=(gc == 7 and kc == 1))
                nc.scalar.activation(sig[:, di, :, :], ps[:, di, :, :],
                                     AF.Sigmoid)
            for di in range(2):
                ve.tensor_tensor(out=t1[:, di, :, :],
                                 in0=sig[:, di, 2:4, :],
                                 in1=c_sb[:, di, :, :], op=ALU.mult)
                ve.scalar_tensor_tensor(
                    out=t2h[:, di, :, :], in0=sig[:, di, 6:8, :], scalar=0.5,
                    in1=sig[:, di, 0:2, :], op0=ALU.subtract, op1=ALU.mult)
                ve.scalar_tensor_tensor(
                    out=c_sb[:, di, :, :], in0=t2h[:, di, :, :], scalar=2.0,
                    in1=t1[:, di, :, :], op0=ALU.mult, op1=ALU.add)
                nc.scalar.activation(th[:, di, :, :], c_sb[:, di, :, :],
                                     AF.Tanh)
            for di in range(2):
                ve.tensor_tensor(out=h_sb[:, di, :, :],
                                 in0=sig[:, di, 4:6, :],
                                 in1=th[:, di, :, :], op=ALU.mult)
            for di, d in enumerate('fb'):
                t = ts_[d]
                # copies on DVE: a Pool reader of h_sb would put a WAR wait
                # on the next h write, displacing its tanh RAW wait onto a
                # SEQ-blocking EventSemaphore (1-wait-per-instruction HW rule)
                if conT_out is not None:
                    nc.vector.tensor_copy(
                        conT_out[d][:, :, :, t].rearrange("k a b -> k (a b)"),
                        h_sb[:, di, :, :].rearrange("k a b -> k (a b)"))
                if conB_out is not None:
                    nc.vector.tensor_copy(
                        conB_out[d][:, :, :, t].rearrange("k a b -> k (a b)"),
                        h_sb[:, di, :, :].rearrange("k a b -> k (a b)"))
                if hfin is not None and tau == T - 1:
                    nc.vector.tensor_copy(
                        hfin[d].rearrange("k a b -> k (a b)"),
                        h_sb[:, di, :, :].rearrange("k a b -> k (a b)"))
            if hook is not None:
                hook(tau)

    conT = {'f': perm.tile([128, 2, 8, 64], F32R, name="conT_f"),
            'b': perm.tile([128, 2, 8, 64], F32R, name="conT_b")}
    conB = {'f': perm.tile([128, 2, 8, 64], BF16, name="conB_f"),
            'b': perm.tile([128, 2, 8, 64], BF16, name="conB_b")}

    # fc weights (bf16 host-prepped): plain DMAs, no staging/convert
    fcp = ctx.enter_context(tc.tile_pool(name="fcp", bufs=1))
    fc1T = fcp.tile([128, 8, 512], BF16, name="fc1T")
    fc2T = fcp.tile([128, 4, 2], BF16, name="fc2T")
    fc1b = fcp.tile([BL, 512], BF16, name="fc1b")
    fc2b = fcp.tile([BL, 2], BF16, name="fc2b")
    for nm, tgt in (('fc1T', fc1T), ('fc2T', fc2T), ('fc1b', fc1b),
                    ('fc2b', fc2b)):
        nc.sync.dma_start(tgt[:], dr[nm][:])

    # matching prep interleaved into the ctx scan's engine idle time;
    # (dir, quarter) becomes ready as the scan's two fronts advance
    prep_ps_stack = ExitStack()
    bld_stack = ExitStack()
    prep = _make_prep(nc, tc, ctx, prep_ps_stack, bld_stack, conT, conB,
                      w2Tr, w2Tf, idb)
    pq = []
    for qi, (fq, bq) in enumerate(((0, 3), (1, 2), (2, 1))):
        rt = 16 * (qi + 1) - 1
        for cl in prep['units']('f', fq):
            pq.append((rt, cl))
        for cl in prep['units']('b', bq):
            pq.append((rt, cl))
        if qi == 1:
            for cl in prep['halves']('f', 0):
                pq.append((31, cl))
            for cl in prep['halves']('b', 1):
                pq.append((31, cl))
            for cl in prep['bld_units']('f', 0, 32):
                pq.append((31, cl))
    ppos = [0]

    def ctx_hook(tau):
        n = 0
        while ppos[0] < len(pq) and n < 2:
            rt, cl = pq[ppos[0]]
            if rt > tau:
                break
            cl()
            ppos[0] += 1
            n += 1

    with ExitStack() as ctx2:
        scan_layer(xgT, whhTb, conT, conB, None, "ctx", hook=ctx_hook)
    while ppos[0] < len(pq):
        pq[ppos[0]][1]()
        ppos[0] += 1
    for cl in prep['units']('f', 3):
        cl()
    for cl in prep['units']('b', 0):
        cl()
    for cl in prep['halves']('f', 1):
        cl()
    for cl in prep['halves']('b', 0):
        cl()
    prep['finals']()
    prep_ps_stack.close()
    for cl in prep['bld_units']('f', 32, 64, mix=True):
        cl()

    if PHASES == 'ctx':
        y_sb0 = perm.tile([BL, NL], F32, name="y_sb0")
        nc.vector.tensor_copy(y_sb0[:], conT['f'][0:BL, 0, 0, 0:NL])
        nc.sync.dma_start(y[:], y_sb0[:])
        return

    # ---------------- matching
    mvT = [perm.tile([128, 512], F32R, name="mvT0"),
           perm.tile([128, 512], F32R, name="mvT1")]
    # f32r memset unsupported; fill via ACT copy with scale=0 (+bias)
    fill_src = bass.AP(tensor=idf.tensor, offset=idf.offset,
                       ap=[idf.ap[0], [0, 512]])
    nc.scalar.activation(mvT[0][:], fill_src, AF.Copy, bias=0.0, scale=0.0)
    nc.scalar.activation(mvT[1][:], fill_src, AF.Copy, bias=0.0, scale=0.0)
    nc.scalar.activation(mvT[0][96:128, :],
                         bass.AP(tensor=idf.tensor, offset=idf.offset,
                                 ap=[[idf.ap[0][0], 32], [0, 512]]),
                         AF.Copy, bias=1.0, scale=0.0)
    mctx = _matching(nc, tc, ctx, conT, conB, w2Tr, w2Tf, w2Tb, mvT,
                     idf, idb, prep)
    bld_stack.close()

    pipe_stack = ExitStack()
    ctx.enter_context(pipe_stack)
    ax_unit, ax_tail = _make_ax_emit(nc, tc, pipe_stack, conB, mctx['n1s'],
                                     w2Tb, mvT, mctx)

    def ax_pair(pair, during=False):
        for ch in pair:
            for d in 'fb':
                for role in range(2):
                    for b in range(BL):
                        ax_unit(d, b, role, ch, pair, during)
        ax_tail(pair)

    if PHASES == 'match':
        for pair in ((0, 7), (1, 6), (2, 5), (3, 4)):
            ax_pair(pair)
        y_sb0 = perm.tile([BL, NL], F32, name="y_sb0")
        nc.vector.tensor_copy(y_sb0[:], mvT[0][0:BL, 0:NL])
        nc.sync.dma_start(y[:], y_sb0[:])
        return

    # ---------------- AX + agg projection pipelined under the agg scan.
    # The agg scan consumes xgaT cols from both ends inward (fw t=tau,
    # bw t=63-tau), in 8-token chunks: chunk pair (c, 7-c) is needed at
    # scan step 8c. Chunks 0/7 (plus their AX features) are computed
    # before the scan; the middle chunks' AX units + projections are
    # emitted from the scan's per-step hook so they execute in engine
    # idle time.
    xgaT = {'f': perm.tile([128, 8, 512], BF16, name="xgaT_f"),
            'b': perm.tile([128, 8, 512], BF16, name="xgaT_b")}
    ap_ps = pipe_stack.enter_context(tc.tile_pool(name="aggps", bufs=3,
                                                  space="PSUM"))

    def proj_chunk(c):
        c0 = 64 * c
        for di, d in enumerate('fb'):
            for gc in range(8):
                ps = ap_ps.tile([128, 64], F32, name=f"ap_{d}_{gc}_{c}",
                                tag="aps")
                for kc in range(2):
                    nc.tensor.matmul(ps[:], aggwT[d][:, kc, gc, :],
                                     mvT[kc][:, c0:c0 + 64],
                                     start=(kc == 0), stop=(kc == 1))
                nc.scalar.activation(xgaT[d][:, gc, c0:c0 + 64], ps[:],
                                     AF.Copy)

    ax_pair((0, 7))
    proj_chunk(0)
    proj_chunk(7)

    def tail_proj(pair):
        ax_tail(pair)
        proj_chunk(pair[0])
        proj_chunk(pair[1])

    units = []
    for cpair in ((1, 6), (2, 5), (3, 4)):
        for c in cpair:
            for d in 'fb':
                for role in range(2):
                    for b in range(BL):
                        units.append((ax_unit, d, b, role, c, cpair, True))
        units.append((tail_proj, cpair))
    qpos = [0]

    def agg_hook(tau):
        # EMISSION-ORDER CORRECTNESS: the tile tracker only sees deps from
        # writes emitted BEFORE a read. Chunk pair k (chunks k, 7-k) is read
        # by scan step 8k, so its units+projection must be fully emitted
        # strictly before that step's instructions. Pace linearly to each
        # deadline (~4.2 units/step through step 21).
        target = min(len(units), (tau + 3) * len(units) // 25 + 1)
        while qpos[0] < target:
            u = units[qpos[0]]
            qpos[0] += 1
            u[0](*u[1:])

    # ---------------- agg scans + fc
    hfin = {d: perm.tile([128, 2, 8], BF16, name=f"hfin_{d}") for d in 'fb'}
    with ExitStack() as ctx2:
        scan_layer(xgaT, awhhTb, None, None, hfin, "agg", hook=agg_hook)
    assert qpos[0] >= len(units)
    pipe_stack.close()
    fps = ctx.enter_context(tc.tile_pool(name="fcps", bufs=1, space="PSUM"))

    # x k-chunks: [hpf c0, hpf c1, hpb c0, hpb c1, hhf c0, hhf c1, hhb c0, hhb c1]
    ksl = []
    for role0 in (0, 4):
        for d in 'fb':
            for c in range(2):
                ksl.append(hfin[d][:, c, role0:role0 + BL])
    x1 = fps.tile([BL, 512], F32, name="x1")
    for kc in range(8):
        nc.tensor.matmul(x1[:], ksl[kc], fc1T[:, kc, :],
                         start=(kc == 0), stop=False)
    nc.tensor.matmul(x1[:], idb[0:BL, 0:BL], fc1b[:], start=False, stop=True)
    xt1 = fcp.tile([BL, 512], F32, name="xt1")
    nc.scalar.activation(xt1[:], x1[:], AF.Tanh)
    xt1ps = fps.tile([128, 4, BL], F32, name="xt1ps")
    for c in range(4):
        nc.tensor.transpose(xt1ps[:, c, :], xt1[:, 128 * c:128 * (c + 1)],
                            idf[0:BL, 0:BL])
    xt1T = fcp.tile([128, 4, BL], BF16, name="xt1T")
    nc.vector.tensor_copy(xt1T[:], xt1ps[:])
    yps = fps.tile([BL, NL], F32, name="yps")
    for c in range(4):
        nc.tensor.matmul(yps[:], xt1T[:, c, :], fc2T[:, c, :],
                         start=(c == 0), stop=False)
    nc.tensor.matmul(yps[:], idb[0:BL, 0:BL], fc2b[:], start=False,
                     stop=True)
    y_sb = fcp.tile([BL, NL], F32, name="y_sb")
    nc.vector.tensor_copy(y_sb[:], yps[:])
    nc.sync.dma_start(y[:], y_sb[:])

    if dbg:
      with tc.tile_pool(name="dbgp", bufs=1) as dbp:
        for d in 'fb':
            cf = dbp.tile([128, 2, 8, 64], F32, name=f"dbgc_{d}")
            nc.scalar.activation(cf[:], conT[d][:], AF.Copy)
            nc.sync.dma_start(dbg[f'conT_{d}'][:], cf[:])
        for i in range(2):
            mf = dbp.tile([128, 512], F32, name=f"dbgm_{i}")
            nc.scalar.activation(mf[:], mvT[i][:], AF.Copy)
            nc.sync.dma_start(dbg[f'mvT{i}'][:], mf[:])
        xtd = dbp.tile([128, 2, 40], F32, name="xtd")
        nc.vector.memset(xtd[:], 0.0)
        nc.vector.tensor_copy(xtd[:, :, 0:8], hfin['f'][:])
        nc.vector.tensor_copy(xtd[:, :, 32:40], hfin['b'][:])
        nc.sync.dma_start(dbg['xT'][:], xtd[:])


# ---------------------------------------------------------------- matching
# ---------------------------------------------------------------- matching

def _make_prep(nc, tc, ctx, psum_stack, bld_stack, conT, conB, w2r, w2f,
               idb):
    """Matching prep (norms / t-major transposes / per-token norms / MAX
    builds), emitted in 16-token quarters so most of it runs in engine
    idle time during the ctx scan. Sqrt/recip finals are batched post-scan
    (Sqrt shares no ACT table set with the scan's Sigmoid/Tanh; scattering
    them through the scan would pay 1.3us table reloads each). bld goes to
    the otherwise-idle Pool engine; the two dirs share one 40KB buffer
    (tag rotation serializes b's builds behind f's MAX reads).
    """
    prep = ctx.enter_context(tc.tile_pool(name="prep", bufs=1))
    bldp = bld_stack.enter_context(tc.tile_pool(name="bldp", bufs=1))
    n1sqp = psum_stack.enter_context(tc.tile_pool(name="n1sqp", bufs=1))
    prepps = psum_stack.enter_context(tc.tile_pool(name="prepps", bufs=2,
                                                   space="PSUM"))
    t = {}
    for d in 'fb':
        t[f'n1sq_{d}'] = n1sqp.tile([20, 4, 8, 64], F32, name=f"n1sq_{d}")
        t[f'n1_{d}'] = prep.tile([20, 4, 8, 64], F32, name=f"n1_{d}")
        t[f'rn1_{d}'] = prep.tile([20, 4, 8, 64], F32, name=f"rn1_{d}")
        t[f'ctm_{d}'] = prep.tile([64, 8, 256], BF16, name=f"ctm_{d}")
        t[f'rvn_{d}'] = prep.tile([64, 8], F32, name=f"rvn_{d}")
    bldt = {}

    def bld_tile(d):
        if d not in bldt:
            bldt[d] = bldp.tile([128, 2, 20, 8, 64], BF16, name=f"bld_{d}",
                                tag="bld", bufs=1)
        return bldt[d]

    def units(d, q):
        di = 0 if d == 'f' else 1
        cT, cB = conT[d], conB[d]
        q0 = 16 * q
        n1sq, ctm, rvn = t[f'n1sq_{d}'], t[f'ctm_{d}'], t[f'rvn_{d}']

        def u_norm():
            csq = prep.tile([128, 2, 8, 16], F32R, name=f"csq_{d}_{q}",
                            tag="csq", bufs=3)
            nc.scalar.activation(csq[:], cT[:, :, :, q0:q0 + 16], AF.Square)
            n1q = prepps.tile([20, 4, 8, 16], F32, name=f"n1q_{d}_{q}",
                              tag="n1q", bufs=2)
            for ty in range(4):
                for c in range(2):
                    nc.tensor.matmul(n1q[:, ty, :, :],
                                     w2r[:, c, di, 20 * ty:20 * ty + 20],
                                     csq[:, c, :, :],
                                     start=(c == 0), stop=(c == 1))
            nc.vector.tensor_copy(n1sq[:, :, :, q0:q0 + 16], n1q[:])

        return [u_norm]

    def halves(d, h):
        # engine partition accesses must be 32-aligned, so the t-major
        # transposes and per-token norms go by 32-token halves
        cB = conB[d]
        h0 = 32 * h
        ctm, rvn = t[f'ctm_{d}'], t[f'rvn_{d}']

        def u_ctm(s0):
            for s in range(s0, s0 + 2):
                tp = prepps.tile([32, 2, 128], BF16, name=f"ct_{d}_{h}_{s}",
                                 tag="ctp", bufs=2)
                for c in range(2):
                    nc.tensor.transpose(tp[:, c, :], cB[:, c, s, h0:h0 + 32],
                                        idb[:])
                if s % 2 == 0:
                    nc.scalar.activation(ctm[h0:h0 + 32, s, :],
                                         tp.rearrange("t c k -> t (c k)"),
                                         AF.Copy)
                else:
                    nc.vector.tensor_copy(ctm[h0:h0 + 32, s, :],
                                          tp.rearrange("t c k -> t (c k)"))

        def u_rvn(s0):
            # bf16 out scratch keeps the STT in 4x DVE perf mode; the f32
            # accum_out (exempt scalar operand) carries the precision
            for s in range(s0, s0 + 4):
                scr = prep.tile([32, 256], BF16, name=f"rs_{d}_{h}_{s}",
                                tag="rvs", bufs=2)
                nc.vector.scalar_tensor_tensor(
                    out=scr[:], in0=ctm[h0:h0 + 32, s, :], scalar=1.0,
                    in1=ctm[h0:h0 + 32, s, :], op0=ALU.mult, op1=ALU.mult,
                    accum_out=rvn[h0:h0 + 32, s:s + 1])

        return [lambda s0=s0: u_ctm(s0) for s0 in range(0, 8, 2)] + \
               [lambda: u_rvn(0), lambda: u_rvn(4)]

    def bld_units(d, tlo, thi, mix=False):
        # mix=True (post-scan): mostly DVE — the bf16 tensor_scalar hits
        # the 4x perf mode (~190ns vs ~800ns Pool); Pool-only when
        # interleaved under the ctx scan where DVE is contended
        di = 0 if d == 'f' else 1
        cB = conB[d]
        bld = bld_tile(d)
        out = []
        for c in range(2):
            for l0 in range(0, L, 2):
                def cl(c=c, l0=l0):
                    for l in range(l0, l0 + 2):
                        eng = (nc.vector if mix and l % 4 != 3
                               else nc.gpsimd)
                        eng.tensor_scalar_mul(
                            bld[:, c, l, :, tlo:thi], cB[:, c, :, tlo:thi],
                            w2f[:, c, di, 20 + l:21 + l])
                out.append(cl)
        return out

    def finals():
        # one sqrt-table residency for all four batched Sqrts
        for d in 'fb':
            nc.scalar.activation(
                t[f'n1_{d}'].rearrange("l y s t -> l (y s t)"),
                t[f'n1sq_{d}'].rearrange("l y s t -> l (y s t)"), AF.Sqrt)
            nc.scalar.activation(t[f'rvn_{d}'][:], t[f'rvn_{d}'][:], AF.Sqrt)
        for d in 'fb':
            n1, rn1 = t[f'n1_{d}'], t[f'rn1_{d}']
            nc.vector.tensor_scalar_max(
                rn1.rearrange("l y s t -> l (y s t)"),
                n1.rearrange("l y s t -> l (y s t)"), EPS)
            nc.vector.reciprocal(rn1.rearrange("l y s t -> l (y s t)"),
                                 rn1.rearrange("l y s t -> l (y s t)"))
            rvn = t[f'rvn_{d}']
            nc.vector.tensor_scalar_max(rvn[:], rvn[:], EPS)
            nc.vector.reciprocal(rvn[:], rvn[:])

    return {'t': t, 'units': units, 'halves': halves,
            'bld_units': bld_units, 'bld_tile': bld_tile, 'finals': finals}


def _matching(nc, tc, ctx, conT, conB, w2r, w2f, w2b, mvT, idf, idb,
              prep):
    stage, n1s = {}, {}
    # feature-type offsets into w2 cols (ty*20) and mv row slots (ty*32)
    # greedy DVE/Pool balancer: Pool runs TT ~3.9x slower than DVE-2x
    # pool pre-charged: Pool's 8us products block their dependent DVE
    # tree stages, so bias assignment away from Pool (swept optimum)
    rot = {'dve': 0.0, 'pool': 30.0}

    def veng(cost=1.0):
        if rot['dve'] + cost <= rot['pool'] + 3.3 * cost:
            rot['dve'] += cost
            return nc.vector
        rot['pool'] += 3.3 * cost
        return nc.gpsimd

    dramp = ctx.enter_context(tc.tile_pool(name="mdram", bufs=1,
                                           space="DRAM"))

    def mcol(mt, slot, ri, b):
        # (20, 64) view of mvT rows [slot:slot+20], cols 8t + ri*4 + b
        return mt[slot:slot + 20, :].rearrange("l (t s) -> l t s",
                                               s=8)[:, :, ri * BL + b]

    for di, d in enumerate('fb'):
        cT, cB = conT[d], conB[d]
        anchor_t = (T - 1) if d == 'f' else 0
        mt = mvT[di]
        n1 = prep['t'][f'n1_{d}']
        rn1 = prep['t'][f'rn1_{d}']
        ctm = prep['t'][f'ctm_{d}']
        rvn = prep['t'][f'rvn_{d}']

        with tc.tile_pool(name=f"mn_{d}", bufs=1) as mn:
          with tc.tile_pool(name=f"mnp_{d}", bufs=2, space="PSUM") as mnp:
            n1s[d] = n1
            # ---- FULL
            ancv = mn.tile([128, 2, 8], F32, name=f"ancv_{d}", tag="ancv")
            nc.vector.tensor_copy(ancv[:], cT[:, :, :, anchor_t])
            for b in range(BL):
                for ri, (s_me, s_an) in enumerate(((b, BL + b), (BL + b, b))):
                    anc = mn.tile([128, 2, 20], BF16, name=f"an_{d}_{b}_{ri}",
                                  tag="anc", bufs=2)
                    for c in range(2):
                        nc.vector.tensor_scalar_mul(
                            anc[:, c, :], w2b[:, c, di, 0:20],
                            ancv[:, c, s_an:s_an + 1])
                    nps = mnp.tile([20, 64], F32, name=f"nf_{d}_{b}_{ri}",
                                   tag="nf")
                    for c in range(2):
                        nc.tensor.matmul(nps[:], anc[:, c, :],
                                         cB[:, c, s_me, :],
                                         start=(c == 0), stop=(c == 1))
                    den = mn.tile([20, 64], F32, name=f"de_{d}_{b}_{ri}",
                                  tag="den", bufs=2)
                    nc.vector.tensor_scalar(
                        out=den[:], in0=n1[:, 0, s_me, :],
                        scalar1=n1[:, 0, s_an, anchor_t:anchor_t + 1],
                        scalar2=EPS, op0=ALU.mult, op1=ALU.max)
                    nc.vector.reciprocal(den[:], den[:])
                    nc.vector.tensor_tensor(out=mcol(mt, 0, ri, b),
                                            in0=nps[:], in1=den[:],
                                            op=ALU.mult)

          # ---- MAX (pair max over the other sequence)
          with tc.tile_pool(name=f"mx_{d}", bufs=1) as mxp, \
               tc.tile_pool(name=f"mxps_{d}", bufs=1, space="PSUM") as mxps:
              bld = prep['bld_tile'](d)
              # stage MAX-type recip norms to DRAM (bf16) for broadcasts
              rnb = mxp.tile([20, 8, 64], BF16, name=f"rnb_{d}", tag="rnb")
              nc.vector.tensor_copy(rnb.rearrange("l s t -> l (s t)"),
                                    rn1[:, 1, :, :].rearrange(
                                        "l s t -> l (s t)"))
              rnd = dramp.tile([20, 8, 64], BF16, name=f"rnd_{d}")
              nc.sync.dma_start(rnd[:], rnb[:])
              mxs_all = {}
              for bp in range(2):
                  for side in range(2):
                      rs_me = 2 * bp if side == 0 else 4 + 2 * bp
                      rs_ot = 4 + 2 * bp if side == 0 else 2 * bp
                      for hf in range(2):
                          pps = mxps.tile([128, 10, 128], F32,
                                          name=f"pp_{d}_{bp}_{side}_{hf}",
                                          tag="pps", bufs=2)
                          for u in range(10):
                              l = 10 * hf + u
                              for c in range(2):
                                  nc.tensor.matmul(
                                      pps[:, u, :],
                                      bld[:, c, l, rs_me:rs_me + 2,
                                          :].rearrange("k e t -> k (e t)"),
                                      cB[:, c, rs_ot:rs_ot + 2,
                                         :].rearrange("k e t -> k (e t)"),
                                      start=(c == 0), stop=(c == 1))
                          for b2 in range(2):
                              b = 2 * bp + b2
                              s_ot = rs_ot + b2
                              key = (side, b)
                              if key not in mxs_all:
                                  mxs_all[key] = mxp.tile(
                                      [64, 2, 10], F32,
                                      name=f"mxs_{d}_{side}_{b}",
                                      tag=f"mxs_{side}_{b2}")
                              nbcb = mxp.tile([64, 10, 64], BF16,
                                              name=f"nb_{d}_{bp}_{side}"
                                                   f"_{hf}_{b2}",
                                              tag="nbcb", bufs=3)
                              nc.sync.dma_start(
                                  nbcb[:],
                                  bass.AP(tensor=rnd.tensor,
                                          offset=rnd.offset
                                          + (10 * hf) * 512 + s_ot * 64,
                                          ap=[[0, 64], [512, 10], [1, 64]]))
                              # stage pps to SBUF bf16 on the idle ACT so
                              # the multiply runs 2x from SBUF instead of
                              # 1x from f32 psum (791ns -> ~390ns on DVE)
                              ppsc = mxp.tile([64, 10, 64], BF16,
                                              name=f"pc_{d}_{bp}_{side}"
                                                   f"_{hf}_{b2}",
                                              tag="ppsc", bufs=3)
                              nc.scalar.activation(
                                  ppsc[:],
                                  pps[64 * b2:64 * b2 + 64, :,
                                      64 * b2:64 * b2 + 64], AF.Copy)
                              pn = mxp.tile([64, 10, 64], BF16,
                                            name=f"pn_{d}_{bp}_{side}"
                                                 f"_{hf}_{b2}",
                                            tag="pn", bufs=3)
                              rot['dve'] += 0.4
                              nc.vector.tensor_tensor(
                                  out=pn[:], in0=ppsc[:],
                                  in1=nbcb[:], op=ALU.mult)
                              # bf16 TT tree-max (2x DVE) beats the 1x
                              # tensor_reduce on 640-elem tiles
                              cur = pn
                              for w in (32, 16, 8, 4, 2):
                                  nxt = mxp.tile(
                                      [64, 10, w], BF16,
                                      name=f"mt_{d}_{bp}_{side}"
                                           f"_{hf}_{b2}_{w}",
                                      tag=f"mt{w}", bufs=2)
                                  nc.vector.tensor_tensor(
                                      out=nxt[:], in0=cur[:, :, 0:w],
                                      in1=cur[:, :, w:2 * w], op=ALU.max)
                                  cur = nxt
                              nc.vector.tensor_tensor(
                                  out=mxs_all[key][:, hf, :],
                                  in0=cur[:, :, 0:1].rearrange(
                                      "t u o -> t (u o)"),
                                  in1=cur[:, :, 1:2].rearrange(
                                      "t u o -> t (u o)"),
                                  op=ALU.max)
              for side in range(2):
                  for b in range(BL):
                      yt = mxps.tile([20, 64], F32,
                                     name=f"yt_{d}_{b}_{side}", tag="yt",
                                     bufs=2)
                      nc.tensor.transpose(
                          yt[:],
                          mxs_all[(side, b)].rearrange(
                              "t hf u -> t (hf u)"),
                          idf[0:64, 0:64])
                      ri_me = 0 if side == 0 else 1
                      s_me = b if side == 0 else BL + b
                      nc.vector.tensor_tensor(
                          out=mcol(mt, 32, ri_me, b), in0=yt[:],
                          in1=rn1[:, 1, s_me, :], op=ALU.mult)
          if d == 'f':
              # dir b's MAX builds now: Pool is idle while DVE chews on
              # dir f's AM blocks; the shared bld buffer (tag bufs=1)
              # WARs behind f's pps reads automatically
              for cl in prep['bld_units']('b', 0, 64, mix=True):
                  cl()
          # ---- AM + AX per batch item
          with tc.tile_pool(name=f"am_{d}", bufs=3) as amp, \
               tc.tile_pool(name=f"amps_{d}", bufs=2, space="PSUM") as amps:
              for b in range(BL):
                  _am_ax_block(nc, tc, d, di, b, cT, cB, w2b, n1, rn1,
                               rvn, ctm, mt, idf, idb, amp, amps, dramp,
                               mcol, anchor_t, veng, stage)



    return {'stage': stage, 'n1s': n1s, 'rot': rot, 'veng': veng,
            'mcol': mcol, 'dramp': dramp}


def _am_ax_block(nc, tc, d, di, b, cT, cB, w2b, n1, rn1, rvn, ctm, mt, idf,
                 idb, amp, amps, dramp, mcol, anchor_t, veng, stage):
    AM_SLOT, AX_SLOT = 64, 96
    sp, sh = b, BL + b

    # raw attention + normalization (attn = rvn_p[i] * raw * rvn_h[j])
    att_ps = amps.tile([64, 64], F32, name=f"at_{d}_{b}", tag="t64", bufs=3)
    for c in range(2):
        nc.tensor.matmul(att_ps[:], cB[:, c, sp, :], cB[:, c, sh, :],
                         start=(c == 0), stop=(c == 1))
    a1 = amp.tile([64, 64], F32, name=f"a1_{d}_{b}", tag="a1")
    nc.scalar.activation(a1[:], att_ps[:], AF.Copy, scale=rvn[:, sp:sp + 1])
    a1t_ps = amps.tile([64, 64], F32, name=f"a1t_{d}_{b}", tag="t64", bufs=3)
    nc.tensor.transpose(a1t_ps[:], a1[:], idf[0:64, 0:64])
    attTn = amp.tile([64, 64], F32, name=f"aTn_{d}_{b}", tag="attTn")
    nc.scalar.activation(attTn[:], a1t_ps[:], AF.Copy,
                         scale=rvn[:, sh:sh + 1])
    attn_ps = amps.tile([64, 64], F32, name=f"an2_{d}_{b}", tag="t64", bufs=3)
    nc.tensor.transpose(attn_ps[:], attTn[:], idf[0:64, 0:64])
    attn = amp.tile([64, 64], F32, name=f"an_{d}_{b}", tag="attn")
    nc.scalar.activation(attn[:], attn_ps[:], AF.Copy)

    # row sums + clamped recips
    rs_h = amp.tile([64, 1], F32, name=f"rh_{d}_{b}", tag="rsh")
    nc.vector.tensor_reduce(out=rs_h[:], in_=attn[:], axis=AX_X, op=ALU.add)
    nc.vector.tensor_scalar_max(rs_h[:], rs_h[:], EPS)
    nc.vector.reciprocal(rs_h[:], rs_h[:])
    rs_p = amp.tile([64, 1], F32, name=f"rp_{d}_{b}", tag="rsp")
    nc.vector.tensor_reduce(out=rs_p[:], in_=attTn[:], axis=AX_X, op=ALU.add)
    nc.vector.tensor_scalar_max(rs_p[:], rs_p[:], EPS)
    nc.vector.reciprocal(rs_p[:], rs_p[:])

    # weighted mean rhs: ahT = T(attn * rs_h) bf16, bpT = T(attTn * rs_p)
    ah = amp.tile([64, 64], F32, name=f"ah_{d}_{b}", tag="ah")
    nc.scalar.activation(ah[:], attn[:], AF.Copy, scale=rs_h[:, 0:1])
    ahT_ps = amps.tile([64, 64], F32, name=f"ahT_{d}_{b}", tag="t64", bufs=3)
    nc.tensor.transpose(ahT_ps[:], ah[:], idf[0:64, 0:64])
    ahT = amp.tile([64, 64], BF16, name=f"ahTs_{d}_{b}", tag="ahTs")
    nc.scalar.activation(ahT[:], ahT_ps[:], AF.Copy)
    bp_ = amp.tile([64, 64], F32, name=f"bp_{d}_{b}", tag="bp")
    nc.scalar.activation(bp_[:], attTn[:], AF.Copy, scale=rs_p[:, 0:1])
    bpT_ps = amps.tile([64, 64], F32, name=f"bpT_{d}_{b}", tag="t64", bufs=3)
    nc.tensor.transpose(bpT_ps[:], bp_[:], idf[0:64, 0:64])
    bpT = amp.tile([64, 64], BF16, name=f"bpTs_{d}_{b}", tag="bpTs")
    nc.scalar.activation(bpT[:], bpT_ps[:], AF.Copy)

    # am vectors + cosine under w_am
    for role, (rhs, s_ctm, s_me) in enumerate(
            ((ahT, sh, sp), (bpT, sp, sh))):
        amv_ps = amps.tile([128, 2, 64], F32, name=f"av_{d}_{b}_{role}",
                           tag="amv", bufs=2)
        for c in range(2):
            nc.tensor.matmul(amv_ps[:, c, :],
                             ctm[:, s_ctm, 128 * c:128 * (c + 1)], rhs[:],
                             start=True, stop=True)
        amv = amp.tile([128, 2, 64], F32R, name=f"am_{d}_{b}_{role}",
                       tag="amv_sb")
        nc.scalar.activation(amv.rearrange("k c t -> k (c t)"),
                             amv_ps.rearrange("k c t -> k (c t)"), AF.Copy)
        prod = amp.tile([128, 2, 64], BF16, name=f"pr_{d}_{b}_{role}",
                        tag="prod")
        for c in range(2):
            nc.vector.tensor_tensor(out=prod[:, c, :], in0=cB[:, c, s_me, :],
                                    in1=amv[:, c, :], op=ALU.mult)
        nump = amps.tile([20, 64], F32, name=f"nu_{d}_{b}_{role}", tag="s20",
                         bufs=2)
        for c in range(2):
            nc.tensor.matmul(nump[:], w2b[:, c, di, 40:60],
                             prod[:, c, :], start=(c == 0), stop=(c == 1))
        amsq = amp.tile([128, 2, 64], BF16, name=f"as_{d}_{b}_{role}",
                        tag="amsq")
        nc.scalar.activation(amsq.rearrange("k c t -> k (c t)"),
                             amv.rearrange("k c t -> k (c t)"), AF.Square)
        n2p = amps.tile([20, 64], F32, name=f"n2_{d}_{b}_{role}", tag="s20",
                        bufs=2)
        for c in range(2):
            nc.tensor.matmul(n2p[:], w2b[:, c, di, 40:60],
                             amsq[:, c, :], start=(c == 0), stop=(c == 1))
        n2s = amp.tile([20, 64], F32, name=f"ns_{d}_{b}_{role}", tag="n2s")
        nc.scalar.activation(n2s[:], n2p[:], AF.Sqrt)
        den = amp.tile([20, 64], F32, name=f"dn_{d}_{b}_{role}", tag="amden")
        nc.vector.tensor_tensor(out=den[:], in0=n1[:, 2, s_me, :],
                                in1=n2s[:], op=ALU.mult)
        nc.vector.tensor_scalar_max(den[:], den[:], EPS)
        nc.vector.reciprocal(den[:], den[:])
        nc.vector.tensor_tensor(out=mcol(mt, AM_SLOT, role, b), in0=nump[:],
                                in1=den[:], op=ALU.mult)

    # ---- stage normalized attention (bf16) to DRAM for the AX phases
    atb = amp.tile([64, 64], BF16, name=f"ab_{d}_{b}", tag="atb")
    nc.vector.tensor_copy(atb[:], attn[:])
    atbT = amp.tile([64, 64], BF16, name=f"abT_{d}_{b}", tag="atbT")
    nc.vector.tensor_copy(atbT[:], attTn[:])
    dsc = dramp.tile([64, 64], BF16, name=f"dx_{d}_{b}")
    nc.sync.dma_start(dsc[:], atb[:])
    dscT = dramp.tile([64, 64], BF16, name=f"dxT_{d}_{b}")
    nc.sync.dma_start(dscT[:], atbT[:])
    stage[(d, b)] = (dsc, dscT)



def _make_ax_emit(nc, tc, ctx, conB, n1s, w2b, mvT, mctx):
    """AX feature (max-attentive cosine), chunked by groups of 8 output
    tokens so the middle chunks interleave with the agg scan's emission.

    ax_unit(d, b, role, ch, during): products + joint bf16 tree-max +
    numerator/norm matmuls accumulated into per-(d, chunk) psum tiles.
    ax_tail(ch): ONE batched Sqrt per dir (both dirs adjacent in ACT
    program order — Sqrt lives in a different ACT table set than the
    scan's Sigmoid/Tanh, so scattering per-unit Sqrts through the scan
    would thrash 1.3us table reloads), then den/recip/feature write for
    all 8 (role, b) units of the chunk at once.

    `during=True` alternates products Pool/DVE for scan-concurrent
    execution; `during=False` uses the greedy DVE/Pool balancer.
    """
    veng = mctx['veng']
    stage, rot = mctx['stage'], mctx['rot']
    axp = ctx.enter_context(tc.tile_pool(name="axp", bufs=2))
    axps = ctx.enter_context(tc.tile_pool(name="axps", bufs=2, space="PSUM"))
    pcnt = [0]
    acc = {}

    def ax_unit(d, b, role, ch, pair, during=False):
        di = 0 if d == 'f' else 1
        i0 = 8 * ch
        cB = conB[d]
        u = role * BL + b
        if pair not in acc:
            acc[pair] = axps.tile([20, 4, 2, 8, 8], F32,
                                  name=f"acc_{pair[0]}_{pair[1]}",
                                  tag="axacc", bufs=2)
        slot = 2 * di + (0 if ch == pair[0] else 1)
        nuxc = acc[pair][:, slot, 0, :, :]
        n2c = acc[pair][:, slot, 1, :, :]
        sp, sh = b, BL + b
        dsc, dscT = stage[(d, b)]
        src = dsc if role == 0 else dscT
        s_v = sh if role == 0 else sp
        s_me = sp if role == 0 else sh
        # broadcast the staged attn rows [i0:i0+8) to all 128 partitions
        bc = axp.tile([128, 8, 64], BF16,
                      name=f"bc_{d}_{b}_{role}_{ch}", tag="bc", bufs=5)
        nc.sync.dma_start(
            bc[:], bass.AP(tensor=src.tensor, offset=src.offset + i0 * 64,
                           ap=[[0, 128], [64, 8], [1, 64]]))
        prod = axp.tile([128, 2, 8, 64], BF16,
                        name=f"xp_{d}_{b}_{role}_{ch}", tag="xprod", bufs=3)
        pcnt[0] += 1
        eng = nc.gpsimd if pcnt[0] % 3 != 0 else nc.vector
        vb = cB[:, :, s_v, :]
        eng.tensor_tensor(
            out=prod[:],
            in0=bass.AP(tensor=vb.tensor, offset=vb.offset,
                        ap=[vb.ap[0], vb.ap[1], [0, 8], vb.ap[2]]),
            in1=bass.AP(tensor=bc.tensor, offset=bc.offset,
                        ap=[bc.ap[0], [0, 2], bc.ap[1], bc.ap[2]]),
            op=ALU.mult)
        rot['dve'] += 0.62  # tree max: DVE only
        cur = prod
        for w in (32, 16, 8, 4, 2):
            nxt = axp.tile([128, 2, 8, w], BF16,
                           name=f"tm_{d}_{b}_{role}_{ch}_{w}",
                           tag=f"tm{w}", bufs=2)
            nc.vector.tensor_tensor(out=nxt[:], in0=cur[:, :, :, 0:w],
                                    in1=cur[:, :, :, w:2 * w], op=ALU.max)
            cur = nxt
        axm = axp.tile([128, 2, 8], F32R,
                       name=f"axm_{d}_{b}_{role}_{ch}", tag="axm", bufs=3)
        nc.vector.tensor_tensor(
            out=axm[:],
            in0=cur[:, :, :, 0:1].rearrange("k c t o -> k c (t o)"),
            in1=cur[:, :, :, 1:2].rearrange("k c t o -> k c (t o)"),
            op=ALU.max)
        prodx = axp.tile([128, 2, 8], BF16,
                         name=f"px_{d}_{b}_{role}_{ch}", tag="prodx", bufs=3)
        nc.vector.tensor_tensor(out=prodx[:], in0=cB[:, :, s_me, i0:i0 + 8],
                                in1=axm[:], op=ALU.mult)
        for c in range(2):
            nc.tensor.matmul(nuxc[:, u, :], w2b[:, c, di, 60:80],
                             prodx[:, c, :], start=(c == 0), stop=(c == 1))
        axsq = axp.tile([128, 2, 8], BF16,
                        name=f"xs_{d}_{b}_{role}_{ch}", tag="axsq", bufs=3)
        nc.scalar.activation(axsq.rearrange("k c t -> k (c t)"),
                             axm.rearrange("k c t -> k (c t)"), AF.Square)
        for c in range(2):
            nc.tensor.matmul(n2c[:, u, :], w2b[:, c, di, 60:80],
                             axsq[:, c, :], start=(c == 0), stop=(c == 1))

    def ax_tail(pair):
        at = acc.pop(pair)
        sq = {}
        for di, d in enumerate('fb'):
            for cpos, ch in enumerate(pair):
                n2s = axp.tile([20, 8, 8], F32, name=f"n2s_{d}_{ch}",
                               tag="n2s", bufs=4)
                nc.scalar.activation(n2s[:], at[:, 2 * di + cpos, 1, :, :],
                                     AF.Sqrt)
                sq[(d, ch)] = n2s
        for di, d in enumerate('fb'):
            n1 = n1s[d]
            for cpos, ch in enumerate(pair):
                i0 = 8 * ch
                nuxc = at[:, 2 * di + cpos, 0, :, :]
                n2s = sq[(d, ch)]
                den = axp.tile([20, 8, 8], F32, name=f"dnc_{d}_{ch}",
                               tag="denc", bufs=2)
                nc.vector.tensor_tensor(
                    out=den[:], in0=n1[:, 3, :, i0:i0 + 8],
                    in1=n2s[:], op=ALU.mult)
                nc.vector.tensor_scalar_max(
                    den.rearrange("l s t -> l (s t)"),
                    den.rearrange("l s t -> l (s t)"), EPS)
                nc.vector.reciprocal(den.rearrange("l s t -> l (s t)"),
                                     den.rearrange("l s t -> l (s t)"))
                out = mvT[di][96:116, 8 * i0:8 * i0 + 64].rearrange(
                    "l (t s) -> l t s", s=8)
                nc.vector.tensor_tensor(out=out,
                                        in0=nuxc.rearrange("l s t -> l t s"),
                                        in1=den.rearrange("l s t -> l t s"),
                                        op=ALU.mult)

    return ax_unit, ax_tail


# ---------------------------------------------------------------- entry

def _get_nc(debug=False):
    key = ('dbg' if debug else 'rel')
    if key not in _CACHE:
        _CACHE[key] = build_nc(debug)
    return _CACHE[key]


def kernel(**inputs):
    nc = _get_nc(False)
    w = _prep_weights(inputs)
    in_maps = []
    for core in range(NCORES):
        m = dict(w)
        m['tokp'] = _prep_tokens(inputs['q1_inputs'], inputs['q2_inputs'],
                                 core)
        in_maps.append(m)
    res = run_bass_kernel_spmd(nc, in_maps, core_ids=list(range(NCORES)))
    out = np.concatenate([res.results[c]['y'] for c in range(NCORES)], axis=0)
    return out.astype(np.float32)


def run_debug(inputs):
    nc = _get_nc(True)
    w = _prep_weights(inputs)
    in_maps = []
    for core in range(NCORES):
        m = dict(w)
        m['tokp'] = _prep_tokens(inputs['q1_inputs'], inputs['q2_inputs'],
                                 core)
        in_maps.append(m)
    res = run_bass_kernel_spmd(nc, in_maps, core_ids=list(range(NCORES)))
    return res



# revision 115
# speedup vs baseline: 1.0177x; 1.0046x over previous
"""BiMPM Trainium2 Bass kernel — pure data parallel over batch (B=32 -> 4/core).

Per-core layouts (B_l=4, stack S=8 rows per step = [p:b0..3, h:b0..3]):
- token/row order: r = t*8 + s, s = seq*4 + b (seq0 = q1 = "p", seq1 = q2 = "h")
- xgT (input projections): (128 = g%128, 8 gc, 512 col=t*8+s) bf16 per dir,
  t-quartered in scan-consumption order so the ctx scan starts early
- scan: fused fw+bw per step; gates psum (128, 2dir, 8gc, 8s) in one 2KB
  bank with PER-DIR start/stop chains so each dir's sigmoid fires without
  waiting for the other's matmuls; g-gates host-prescaled x2 so ONE Sigmoid
  covers a dir's gates (tanh(g) = 2*sigmoid(2g)-1); h double-buffered and
  output copies on DVE (keeps the h-write's tanh RAW wait attached to the
  instruction instead of spilling to a SEQ-blocking EventSemaphore)
- conT f32r / conB bf16 (ctx outputs, hd-major): (128, 2c, 8s, 64t) per dir
- matching prep (csq/n1 norms, ctm transposes, rvn token norms, bld MAX
  builds) emitted in 16/32-token chunks INTERLEAVED into the ctx scan's
  engine idle time via a per-step hook; sqrt/recip finals batched post-scan
  (Sqrt shares no ACT table set with Sigmoid/Tanh — 1.3us reload each)
- matching: FULL/MAX/AM as before (MAX reduce = bf16 TT tree; AM scalar
  normalizations on ACT via per-partition scale APs; bld on idle Pool,
  dirs share one 40KB buffer)
- AX in 8-token chunk pairs (c, 7-c): products (Pool-biased 2:1) + joint
  2-channel bf16 tree-max; numerator/norm matmuls accumulate into one psum
  bank per pair; ONE batched Sqrt site per pair. Head pair (0,7) runs
  before the agg scan; mid pairs + t-chunked agg projections are emitted
  from the agg scan's per-step hook, paced so pair (c, 7-c) is fully
  emitted before scan step 8c reads its xgaT chunk (emission order IS the
  dependency order for the tile tracker — late emission = uninit reads)
- weights shipped bf16 from host (wih/whh/awhh/fc; agg proj stays f32r);
  fc head all-bf16 against bf16 hfin
- mvT (match features): 2 tiles (128, 512) f32r, feature rows at 32-aligned
  slots [full@0, max@32, am@64, ax@96, ones@116]

TimelineSim: 574376 ns (baseline 618195); HW rel err 7.4e-3 (gate 2e-2).
word_emb shipped bf16 (gather-then-round == round-then-gather: identical).
"""
import ml_dtypes
import numpy as np
from contextlib import ExitStack

BF16_NP = ml_dtypes.bfloat16

import concourse.bass as bass
import concourse.tile as tile
from concourse import bacc, mybir
from concourse.bass_utils import run_bass_kernel_spmd
from concourse.masks import make_identity

F32 = mybir.dt.float32
F32R = mybir.dt.float32r
BF16 = mybir.dt.bfloat16
I32 = mybir.dt.int32
AF = mybir.ActivationFunctionType
ALU = mybir.AluOpType
AX_X = mybir.AxisListType.X

B, T, V, D, H, L, NL = 32, 64, 50000, 300, 256, 20, 2
NCORES = 8
BL = B // NCORES
S = 2 * BL
EPS = 1e-8

_CACHE = {}
PHASES = 'full'  # 'ctx' | 'match' | 'full' (for TimelineSim bisection)


# ---------------------------------------------------------------- host prep

def _gate_reorder(w):
    # PyTorch gate order i,f,g,o -> chunk order [i, f, o, 2*g].
    # The x2 on g lets the scan use one Sigmoid for all gates:
    # tanh(g) == 2*sigmoid(2g) - 1.
    i, f, g, o = np.split(w, 4, axis=0)
    return np.concatenate([i, f, o, 2.0 * g], axis=0)


def _prep_weights(inp):
    w = {}
    f32 = np.float32

    def ctx_wT(dir_):
        # ws layout: [k%128, kc(3), gc(8), m(128)]; row 300 = bias, pad to 384
        wih = _gate_reorder(np.asarray(inp[f'ctx_wih_{dir_}'], f32))
        bias = _gate_reorder(
            np.asarray(inp[f'ctx_bih_{dir_}'] + inp[f'ctx_bhh_{dir_}'],
                       f32)[:, None]).T
        wt = np.concatenate([wih.T, bias, np.zeros((83, 1024), f32)], 0)
        return np.ascontiguousarray(
            wt.reshape(3, 128, 8, 128).transpose(1, 0, 2, 3)).astype(BF16_NP)

    def whhT(pfx, dir_):
        # ws layout: [k%128, kc, gc, m] = whh_reord[gc*128+m, kc*128+k]
        whh = _gate_reorder(np.asarray(inp[f'{pfx}_whh_{dir_}'], f32))
        return np.ascontiguousarray(
            whh.T.reshape(2, 128, 8, 128).transpose(1, 0, 2, 3)).astype(
                BF16_NP)

    w['wihT_f'], w['wihT_b'] = ctx_wT('f'), ctx_wT('b')
    w['whhT_f'], w['whhT_b'] = whhT('ctx', 'f'), whhT('ctx', 'b')
    w['awhhT_f'], w['awhhT_b'] = whhT('agg', 'f'), whhT('agg', 'b')

    def agg_wT(dir_):
        wih = _gate_reorder(np.asarray(inp[f'agg_wih_{dir_}'], f32))
        bias = _gate_reorder(
            np.asarray(inp[f'agg_bih_{dir_}'] + inp[f'agg_bhh_{dir_}'],
                       f32)[:, None]).T
        out = np.zeros((256, 1024), f32)
        for d in range(2):
            for ty in range(4):
                src = wih[:, d * 80 + ty * 20: d * 80 + ty * 20 + 20]
                out[d * 128 + 32 * ty: d * 128 + 32 * ty + 20] = src.T
        out[116] = bias[0]
        return np.ascontiguousarray(
            out.reshape(2, 128, 8, 128).transpose(1, 0, 2, 3), f32)

    w['aggwT_f'], w['aggwT_b'] = agg_wT('f'), agg_wT('b')

    # w2T80: (128 = h%128, 2 c, 2 dir, 80 = ty*20+l), ty in [full,max,am,ax]
    w2 = np.asarray(inp['mp_w'], f32) ** 2
    w2t = np.zeros((128, 2, 2, 80), f32)
    for d in range(2):
        for ty in range(4):
            src = w2[2 * ty + d]
            for c in range(2):
                w2t[:, c, d, ty * 20:(ty + 1) * 20] = \
                    src[:, c * 128:(c + 1) * 128].T
    w['w2T'] = np.ascontiguousarray(w2t)

    fc1 = np.asarray(inp['fc1_w'], f32)
    w['fc1T'] = np.ascontiguousarray(
        fc1.T.reshape(8, 128, 512).transpose(1, 0, 2)).astype(BF16_NP)
    w['fc1b'] = np.ascontiguousarray(
        np.broadcast_to(np.asarray(inp['fc1_b'], f32), (BL, 512))).astype(
            BF16_NP)
    fc2 = np.asarray(inp['fc2_w'], f32)
    w['fc2T'] = np.ascontiguousarray(
        fc2.T.reshape(4, 128, 2).transpose(1, 0, 2)).astype(BF16_NP)
    w['fc2b'] = np.ascontiguousarray(
        np.broadcast_to(np.asarray(inp['fc2_b'], f32), (BL, 2))).astype(
            BF16_NP)
    w['word_emb'] = np.ascontiguousarray(
        np.asarray(inp['word_emb'], f32)).astype(BF16_NP)
    return w


def _prep_tokens(q1, q2, core):
    q1c = np.asarray(q1[core * BL:(core + 1) * BL]).astype(np.int64)
    q2c = np.asarray(q2[core * BL:(core + 1) * BL]).astype(np.int64)
    tok = np.zeros((T * S,), np.int32)
    for seq, q in ((0, q1c), (1, q2c)):
        for b in range(BL):
            tok[np.arange(T) * S + seq * BL + b] = q[b]
    return np.ascontiguousarray(tok.reshape(4, 128))


# ---------------------------------------------------------------- build

def build_nc(debug=False):
    nc = bacc.Bacc("TRN2", target_bir_lowering=False, debug=False,
                   enable_asserts=True, num_devices=NCORES)
    dt = nc.dram_tensor
    dr = {}
    dr['tokp'] = dt("tokp", [4, 128], I32, kind="ExternalInput").ap()
    dr['word_emb'] = dt("word_emb", [V, D], BF16,
                        kind="ExternalInput").ap()
    for n, shp in [('wihT_f', [128, 3, 8, 128]), ('wihT_b', [128, 3, 8, 128]),
                   ('whhT_f', [128, 2, 8, 128]), ('whhT_b', [128, 2, 8, 128]),
                   ('awhhT_f', [128, 2, 8, 128]),
                   ('awhhT_b', [128, 2, 8, 128]),
                   ('fc1T', [128, 8, 512]), ('fc1b', [BL, 512]),
                   ('fc2T', [128, 4, 2]), ('fc2b', [BL, 2])]:
        dr[n] = dt(n, shp, BF16, kind="ExternalInput").ap()
    for n, shp in [('aggwT_f', [128, 2, 8, 128]), ('aggwT_b', [128, 2, 8, 128]),
                   ('w2T', [128, 2, 2, 80])]:
        dr[n] = dt(n, shp, F32, kind="ExternalInput").ap()
    y = dt("y", [BL, NL], F32, kind="ExternalOutput").ap()
    dbg = {}
    if debug:
        dbg['conT_f'] = dt("dbg_conT_f", [128, 2, 8, 64], F32,
                           kind="ExternalOutput").ap()
        dbg['conT_b'] = dt("dbg_conT_b", [128, 2, 8, 64], F32,
                           kind="ExternalOutput").ap()
        dbg['mvT0'] = dt("dbg_mvT0", [128, 512], F32,
                         kind="ExternalOutput").ap()
        dbg['mvT1'] = dt("dbg_mvT1", [128, 512], F32,
                         kind="ExternalOutput").ap()
        dbg['xT'] = dt("dbg_xT", [128, 2, 40], F32,
                       kind="ExternalOutput").ap()

    with tile.TileContext(nc) as tc, ExitStack() as ctx:
        _body(nc, tc, ctx, dr, y, dbg)
    nc.compile()
    return nc


def _body(nc, tc, ctx, dr, y, dbg):
    perm = ctx.enter_context(tc.tile_pool(name="perm", bufs=1))

    idf = perm.tile([128, 128], F32, name="idf")
    make_identity(nc, idf[:])
    idb = perm.tile([128, 128], BF16, name="idb")
    nc.vector.tensor_copy(idb[:], idf[:])
    selb = idb.rearrange("k (tl s) -> k tl s", s=8)

    def conv(src, dtype, name, engine=None, pool=None):
        t = (pool or perm).tile(list(src.shape), dtype, name=f"C_{name}")
        eng = engine or nc.vector
        if eng is nc.scalar:
            eng.activation(t[:], src[:], AF.Copy)
        else:
            eng.tensor_copy(t[:], src[:])
        return t

    wihT, whhTb, awhhTb, aggwT = {}, {}, {}, {}
    w2Tf = perm.tile([128, 2, 2, 80], F32, name="w2Tf")
    nc.sync.dma_start(w2Tf[:], dr['w2T'][:])
    w2Tr = conv(w2Tf, F32R, "w2Tr")
    w2Tb = conv(w2Tf, BF16, "w2Tb", nc.gpsimd)

    idx_sb = perm.tile([128, 4], I32, name="idx_sb")
    nc.sync.dma_start(idx_sb[:], dr['tokp'].rearrange("m p -> p m"))

    # ---------------- weight load + embedding gather + ctx projection (bf16)
    # xgT[d]: (128 = g%128, 8 gc, 512 cols) bf16 ; col r = t*8 + s
    # Gather tiles share scope with weight staging (no SBUF reuse between
    # the indirect-DMA writes and freed staging tiles).
    xgT = {'f': perm.tile([128, 8, 512], BF16, name="xgT_f"),
           'b': perm.tile([128, 8, 512], BF16, name="xgT_b")}
    with tc.tile_pool(name="embp", bufs=2) as embp, \
         tc.tile_pool(name="loadp", bufs=1) as loadp, \
         tc.tile_pool(name="epsum", bufs=2, space="PSUM") as epsum:
        # embT (128 = d%128, 3 kc, 512 tok) bf16
        embT = embp.tile([128, 3, 512], BF16, name="embT", tag="embT")
        embs = []
        for m in range(4):
            emb = embp.tile([128, 304], BF16, name=f"emb_{m}", tag=f"emb{m}")
            nc.gpsimd.indirect_dma_start(
                out=emb[:, 0:300], out_offset=None, in_=dr['word_emb'][:],
                in_offset=bass.IndirectOffsetOnAxis(ap=idx_sb[:, m:m + 1],
                                                    axis=0))
            nc.vector.memset(emb[:, 300:301], 1.0)
            embs.append(emb)

        def load_f32(name, shp, tag):
            t = loadp.tile(shp, F32, name=f"L_{name}", tag=tag)
            nc.sync.dma_start(t[:], dr[name][:])
            return t

        def load_bf16(name, shp, pool):
            t = pool.tile(shp, BF16, name=f"B_{name}")
            nc.sync.dma_start(t[:], dr[name][:])
            return t

        for d in 'fb':
            wihT[d] = load_bf16(f'wihT_{d}', [128, 3, 8, 128], embp)
            whhTb[d] = load_bf16(f'whhT_{d}', [128, 2, 8, 128], perm)
            awhhTb[d] = load_bf16(f'awhhT_{d}', [128, 2, 8, 128], perm)
            aggwT[d] = conv(load_f32(f'aggwT_{d}', [128, 2, 8, 128], "raw8k"),
                            F32R, f"aggw_{d}", nc.scalar)

        for m in range(4):
            embb = embs[m]
            for c in range(3):
                kc = min(128, 301 - 128 * c)
                tp = epsum.tile([128, 128], BF16, name=f"etp_{m}_{c}",
                                tag="etp")
                nc.tensor.transpose(tp[0:kc, :],
                                    embb[:, 128 * c:128 * c + kc], idb[:])
                if c % 2 == 0:
                    nc.scalar.activation(embT[0:kc, c, 128 * m:128 * (m + 1)],
                                         tp[0:kc, :], AF.Copy)
                else:
                    nc.vector.tensor_copy(
                        embT[0:kc, c, 128 * m:128 * (m + 1)], tp[0:kc, :])
        # t-quartered, scan-consumption-ordered (f ascending, b descending)
        # so the ctx scan's first steps start before the full projection
        qorder = [(0, 0), (1, 3), (0, 1), (1, 2), (0, 2), (1, 1), (0, 3),
                  (1, 0)]
        for di, q in qorder:
            d = 'fb'[di]
            for gc in range(8):
                ps = epsum.tile([128, 128], F32, name=f"xps_{d}_{gc}_{q}",
                                tag="xps")
                for c in range(3):
                    kc = min(128, 301 - 128 * c)
                    nc.tensor.matmul(ps[:], wihT[d][0:kc, c, gc, :],
                                     embT[0:kc, c, 128 * q:128 * (q + 1)],
                                     start=(c == 0), stop=(c == 2))
                if gc % 2 == 0:
                    nc.vector.tensor_copy(
                        xgT[d][:, gc, 128 * q:128 * (q + 1)], ps[:])
                else:
                    nc.scalar.activation(
                        xgT[d][:, gc, 128 * q:128 * (q + 1)], ps[:], AF.Copy)

    # ---------------- scan layer (shared ctx/agg), fused fw+bw per step
    # state h/c: (128 = hd%128, 2 dir, 2 kc, 8 s)
    # gates psum: (128 = g%128, 2 dir, 8 gc, 8 s), order [i0 i1 f0 f1 o0 o1 g0 g1]
    # g-gates pre-scaled x2 at host: tanh(g) = 2*sigmoid(2g) - 1, so one
    # Sigmoid covers all 8 chunks; xg injected via identity matmul (start=True).
    def scan_layer(xgd, whh_d, conT_out, conB_out, hfin, lname, ve=None,
                   hook=None):
        ve = ve or nc.vector
        sp = ctx2.enter_context(tc.tile_pool(name=f"sp_{lname}", bufs=12))
        pp = ctx2.enter_context(tc.tile_pool(name=f"pp_{lname}", bufs=3,
                                             space="PSUM"))
        cp = ctx2.enter_context(tc.tile_pool(name=f"cp_{lname}", bufs=1))
        c_sb = cp.tile([128, 2, 2, 8], F32, name=f"c_{lname}")
        # h double-buffered: the step-t write must not WAR against step-t's
        # own whh matmul reads (a 2-sem wait the tile framework lowers to a
        # SEQ-blocking EventSemaphore on DVE, ~200ns/step on the chain)
        h_bufs = [cp.tile([128, 2, 2, 8], BF16, name=f"h_{lname}_{i}")
                  for i in range(2)]
        nc.vector.memset(c_sb[:], 0.0)
        nc.vector.memset(h_bufs[0][:], 0.0)
        nc.vector.memset(h_bufs[1][:], 0.0)
        for tau in range(T):
            ts_ = {'f': tau, 'b': T - 1 - tau}
            h_prev = h_bufs[(tau + 1) % 2]
            h_sb = h_bufs[tau % 2]
            # one full psum bank (2KB); each dir's 1KB region runs its own
            # start/stop chain so dir f's sigmoid fires without waiting for
            # dir b's matmuls — the two cell-update chains then overlap.
            psb = pp.tile([128, 512], F32, name=f"g_{lname}_{tau}",
                          tag="gps")
            ps = psb[:, 0:128].rearrange("k (d g s) -> k d g s", d=2, g=8)
            sig = sp.tile([128, 2, 8, 8], F32, name=f"si_{lname}_{tau}",
                          tag="sig")
            t1 = sp.tile([128, 2, 2, 8], F32, name=f"t1_{lname}_{tau}",
                         tag="t1")
            t2h = sp.tile([128, 2, 2, 8], F32, name=f"t2_{lname}_{tau}",
                          tag="t2h")
            th = sp.tile([128, 2, 2, 8], F32, name=f"th_{lname}_{tau}",
                         tag="th")
            for di, d in enumerate('fb'):
                t = ts_[d]
                nc.tensor.matmul(ps[:, di, :, :], idb[:],
                                 xgd[d][:, :, 8 * t:8 * t + 8],
                                 start=True, stop=False)
                for gc in range(8):
                    for kc in range(2):
                        nc.tensor.matmul(
                            ps[:, di, gc, :], whh_d[d][:, kc, gc, :],
                            h_prev[:, di, kc, :], start=False,
                            stop=(gc == 7 and kc == 1))
                nc.scalar.activation(sig[:, di, :, :], ps[:, di, :, :],
                                     AF.Sigmoid)
            for di in range(2):
                ve.tensor_tensor(out=t1[:, di, :, :],
                                 in0=sig[:, di, 2:4, :],
                                 in1=c_sb[:, di, :, :], op=ALU.mult)
                ve.scalar_tensor_tensor(
                    out=t2h[:, di, :, :], in0=sig[:, di, 6:8, :], scalar=0.5,
                    in1=sig[:, di, 0:2, :], op0=ALU.subtract, op1=ALU.mult)
                ve.scalar_tensor_tensor(
                    out=c_sb[:, di, :, :], in0=t2h[:, di, :, :], scalar=2.0,
                    in1=t1[:, di, :, :], op0=ALU.mult, op1=ALU.add)
                nc.scalar.activation(th[:, di, :, :], c_sb[:, di, :, :],
                                     AF.Tanh)
            for di in range(2):
                ve.tensor_tensor(out=h_sb[:, di, :, :],
                                 in0=sig[:, di, 4:6, :],
                                 in1=th[:, di, :, :], op=ALU.mult)
            for di, d in enumerate('fb'):
                t = ts_[d]
                # copies on DVE: a Pool reader of h_sb would put a WAR wait
                # on the next h write, displacing its tanh RAW wait onto a
                # SEQ-blocking EventSemaphore (1-wait-per-instruction HW rule)
                if conT_out is not None:
                    nc.vector.tensor_copy(
                        conT_out[d][:, :, :, t].rearrange("k a b -> k (a b)"),
                        h_sb[:, di, :, :].rearrange("k a b -> k (a b)"))
                if conB_out is not None:
                    nc.vector.tensor_copy(
                        conB_out[d][:, :, :, t].rearrange("k a b -> k (a b)"),
                        h_sb[:, di, :, :].rearrange("k a b -> k (a b)"))
                if hfin is not None and tau == T - 1:
                    nc.vector.tensor_copy(
                        hfin[d].rearrange("k a b -> k (a b)"),
                        h_sb[:, di, :, :].rearrange("k a b -> k (a b)"))
            if hook is not None:
                hook(tau)

    conT = {'f': perm.tile([128, 2, 8, 64], F32R, name="conT_f"),
            'b': perm.tile([128, 2, 8, 64], F32R, name="conT_b")}
    conB = {'f': perm.tile([128, 2, 8, 64], BF16, name="conB_f"),
            'b': perm.tile([128, 2, 8, 64], BF16, name="conB_b")}

    # fc weights (bf16 host-prepped): plain DMAs, no staging/convert
    fcp = ctx.enter_context(tc.tile_pool(name="fcp", bufs=1))
    fc1T = fcp.tile([128, 8, 512], BF16, name="fc1T")
    fc2T = fcp.tile([128, 4, 2], BF16, name="fc2T")
    fc1b = fcp.tile([BL, 512], BF16, name="fc1b")
    fc2b = fcp.tile([BL, 2], BF16, name="fc2b")
    for nm, tgt in (('fc1T', fc1T), ('fc2T', fc2T), ('fc1b', fc1b),
                    ('fc2b', fc2b)):
        nc.sync.dma_start(tgt[:], dr[nm][:])

    # matching prep interleaved into the ctx scan's engine idle time;
    # (dir, quarter) becomes ready as the scan's two fronts advance
    prep_ps_stack = ExitStack()
    bld_stack = ExitStack()
    prep = _make_prep(nc, tc, ctx, prep_ps_stack, bld_stack, conT, conB,
                      w2Tr, w2Tf, idb)
    pq = []
    for qi, (fq, bq) in enumerate(((0, 3), (1, 2), (2, 1))):
        rt = 16 * (qi + 1) - 1
        for cl in prep['units']('f', fq):
            pq.append((rt, cl))
        for cl in prep['units']('b', bq):
            pq.append((rt, cl))
        if qi == 1:
            for cl in prep['halves']('f', 0):
                pq.append((31, cl))
            for cl in prep['halves']('b', 1):
                pq.append((31, cl))
            for cl in prep['bld_units']('f', 0, 32):
                pq.append((31, cl))
    ppos = [0]

    def ctx_hook(tau):
        n = 0
        while ppos[0] < len(pq) and n < 2:
            rt, cl = pq[ppos[0]]
            if rt > tau:
                break
            cl()
            ppos[0] += 1
            n += 1

    with ExitStack() as ctx2:
        scan_layer(xgT, whhTb, conT, conB, None, "ctx", hook=ctx_hook)
    while ppos[0] < len(pq):
        pq[ppos[0]][1]()
        ppos[0] += 1
    for cl in prep['units']('f', 3):
        cl()
    for cl in prep['units']('b', 0):
        cl()
    for cl in prep['halves']('f', 1):
        cl()
    for cl in prep['halves']('b', 0):
        cl()
    prep['finals']()
    prep_ps_stack.close()
    for cl in prep['bld_units']('f', 32, 64, mix=True):
        cl()

    if PHASES == 'ctx':
        y_sb0 = perm.tile([BL, NL], F32, name="y_sb0")
        nc.vector.tensor_copy(y_sb0[:], conT['f'][0:BL, 0, 0, 0:NL])
        nc.sync.dma_start(y[:], y_sb0[:])
        return

    # ---------------- matching
    mvT = [perm.tile([128, 512], F32R, name="mvT0"),
           perm.tile([128, 512], F32R, name="mvT1")]
    # f32r memset unsupported; fill via ACT copy with scale=0 (+bias)
    fill_src = bass.AP(tensor=idf.tensor, offset=idf.offset,
                       ap=[idf.ap[0], [0, 512]])
    nc.scalar.activation(mvT[0][:], fill_src, AF.Copy, bias=0.0, scale=0.0)
    nc.scalar.activation(mvT[1][:], fill_src, AF.Copy, bias=0.0, scale=0.0)
    nc.scalar.activation(mvT[0][96:128, :],
                         bass.AP(tensor=idf.tensor, offset=idf.offset,
                                 ap=[[idf.ap[0][0], 32], [0, 512]]),
                         AF.Copy, bias=1.0, scale=0.0)
    mctx = _matching(nc, tc, ctx, conT, conB, w2Tr, w2Tf, w2Tb, mvT,
                     idf, idb, prep)
    bld_stack.close()

    pipe_stack = ExitStack()
    ctx.enter_context(pipe_stack)
    ax_unit, ax_tail = _make_ax_emit(nc, tc, pipe_stack, conB, mctx['n1s'],
                                     w2Tb, mvT, mctx)

    def ax_pair(pair, during=False):
        for ch in pair:
            for d in 'fb':
                for role in range(2):
                    for b in range(BL):
                        ax_unit(d, b, role, ch, pair, during)
        ax_tail(pair)

    if PHASES == 'match':
        for pair in ((0, 7), (1, 6), (2, 5), (3, 4)):
            ax_pair(pair)
        y_sb0 = perm.tile([BL, NL], F32, name="y_sb0")
        nc.vector.tensor_copy(y_sb0[:], mvT[0][0:BL, 0:NL])
        nc.sync.dma_start(y[:], y_sb0[:])
        return

    # ---------------- AX + agg projection pipelined under the agg scan.
    # The agg scan consumes xgaT cols from both ends inward (fw t=tau,
    # bw t=63-tau), in 8-token chunks: chunk pair (c, 7-c) is needed at
    # scan step 8c. Chunks 0/7 (plus their AX features) are computed
    # before the scan; the middle chunks' AX units + projections are
    # emitted from the scan's per-step hook so they execute in engine
    # idle time.
    xgaT = {'f': perm.tile([128, 8, 512], BF16, name="xgaT_f"),
            'b': perm.tile([128, 8, 512], BF16, name="xgaT_b")}
    ap_ps = pipe_stack.enter_context(tc.tile_pool(name="aggps", bufs=3,
                                                  space="PSUM"))

    def proj_chunk(c):
        c0 = 64 * c
        for di, d in enumerate('fb'):
            for gc in range(8):
                ps = ap_ps.tile([128, 64], F32, name=f"ap_{d}_{gc}_{c}",
                                tag="aps")
                for kc in range(2):
                    nc.tensor.matmul(ps[:], aggwT[d][:, kc, gc, :],
                                     mvT[kc][:, c0:c0 + 64],
                                     start=(kc == 0), stop=(kc == 1))
                nc.scalar.activation(xgaT[d][:, gc, c0:c0 + 64], ps[:],
                                     AF.Copy)

    ax_pair((0, 7))
    proj_chunk(0)
    proj_chunk(7)

    def tail_proj(pair):
        ax_tail(pair)
        proj_chunk(pair[0])
        proj_chunk(pair[1])

    units = []
    for cpair in ((1, 6), (2, 5), (3, 4)):
        for c in cpair:
            for d in 'fb':
                for role in range(2):
                    for b in range(BL):
                        units.append((ax_unit, d, b, role, c, cpair, True))
        units.append((tail_proj, cpair))
    qpos = [0]

    def agg_hook(tau):
        # EMISSION-ORDER CORRECTNESS: the tile tracker only sees deps from
        # writes emitted BEFORE a read. Chunk pair k (chunks k, 7-k) is read
        # by scan step 8k, so its units+projection must be fully emitted
        # strictly before that step's instructions. Pace linearly to each
        # deadline (~4.2 units/step through step 21).
        target = min(len(units), (tau + 3) * len(units) // 25 + 1)
        while qpos[0] < target:
            u = units[qpos[0]]
            qpos[0] += 1
            u[0](*u[1:])

    # ---------------- agg scans + fc
    hfin = {d: perm.tile([128, 2, 8], BF16, name=f"hfin_{d}") for d in 'fb'}
    with ExitStack() as ctx2:
        scan_layer(xgaT, awhhTb, None, None, hfin, "agg", hook=agg_hook)
    assert qpos[0] >= len(units)
    pipe_stack.close()
    fps = ctx.enter_context(tc.tile_pool(name="fcps", bufs=1, space="PSUM"))

    # x k-chunks: [hpf c0, hpf c1, hpb c0, hpb c1, hhf c0, hhf c1, hhb c0, hhb c1]
    ksl = []
    for role0 in (0, 4):
        for d in 'fb':
            for c in range(2):
                ksl.append(hfin[d][:, c, role0:role0 + BL])
    x1 = fps.tile([BL, 512], F32, name="x1")
    for kc in range(8):
        nc.tensor.matmul(x1[:], ksl[kc], fc1T[:, kc, :],
                         start=(kc == 0), stop=False)
    nc.tensor.matmul(x1[:], idb[0:BL, 0:BL], fc1b[:], start=False, stop=True)
    xt1 = fcp.tile([BL, 512], F32, name="xt1")
    nc.scalar.activation(xt1[:], x1[:], AF.Tanh)
    xt1ps = fps.tile([128, 4, BL], F32, name="xt1ps")
    for c in range(4):
        nc.tensor.transpose(xt1ps[:, c, :], xt1[:, 128 * c:128 * (c + 1)],
                            idf[0:BL, 0:BL])
    xt1T = fcp.tile([128, 4, BL], BF16, name="xt1T")
    nc.vector.tensor_copy(xt1T[:], xt1ps[:])
    yps = fps.tile([BL, NL], F32, name="yps")
    for c in range(4):
        nc.tensor.matmul(yps[:], xt1T[:, c, :], fc2T[:, c, :],
                         start=(c == 0), stop=False)
    nc.tensor.matmul(yps[:], idb[0:BL, 0:BL], fc2b[:], start=False,
                     stop=True)
    y_sb = fcp.tile([BL, NL], F32, name="y_sb")
    nc.vector.tensor_copy(y_sb[:], yps[:])
    nc.sync.dma_start(y[:], y_sb[:])

    if dbg:
      with tc.tile_pool(name="dbgp", bufs=1) as dbp:
        for d in 'fb':
            cf = dbp.tile([128, 2, 8, 64], F32, name=f"dbgc_{d}")
            nc.scalar.activation(cf[:], conT[d][:], AF.Copy)
            nc.sync.dma_start(dbg[f'conT_{d}'][:], cf[:])
        for i in range(2):
            mf = dbp.tile([128, 512], F32, name=f"dbgm_{i}")
            nc.scalar.activation(mf[:], mvT[i][:], AF.Copy)
            nc.sync.dma_start(dbg[f'mvT{i}'][:], mf[:])
        xtd = dbp.tile([128, 2, 40], F32, name="xtd")
        nc.vector.memset(xtd[:], 0.0)
        nc.vector.tensor_copy(xtd[:, :, 0:8], hfin['f'][:])
        nc.vector.tensor_copy(xtd[:, :, 32:40], hfin['b'][:])
        nc.sync.dma_start(dbg['xT'][:], xtd[:])


# ---------------------------------------------------------------- matching
# ---------------------------------------------------------------- matching

def _make_prep(nc, tc, ctx, psum_stack, bld_stack, conT, conB, w2r, w2f,
               idb):
    """Matching prep (norms / t-major transposes / per-token norms / MAX
    builds), emitted in 16-token quarters so most of it runs in engine
    idle time during the ctx scan. Sqrt/recip finals are batched post-scan
    (Sqrt shares no ACT table set with the scan's Sigmoid/Tanh; scattering
    them through the scan would pay 1.3us table reloads each). bld goes to
    the otherwise-idle Pool engine; the two dirs share one 40KB buffer
    (tag rotation serializes b's builds behind f's MAX reads).
    """
    prep = ctx.enter_context(tc.tile_pool(name="prep", bufs=1))
    bldp = bld_stack.enter_context(tc.tile_pool(name="bldp", bufs=1))
    n1sqp = psum_stack.enter_context(tc.tile_pool(name="n1sqp", bufs=1))
    prepps = psum_stack.enter_context(tc.tile_pool(name="prepps", bufs=2,
                                                   space="PSUM"))
    t = {}
    for d in 'fb':
        t[f'n1sq_{d}'] = n1sqp.tile([20, 4, 8, 64], F32, name=f"n1sq_{d}")
        t[f'n1_{d}'] = prep.tile([20, 4, 8, 64], F32, name=f"n1_{d}")
        t[f'rn1_{d}'] = prep.tile([20, 4, 8, 64], F32, name=f"rn1_{d}")
        t[f'ctm_{d}'] = prep.tile([64, 8, 256], BF16, name=f"ctm_{d}")
        t[f'rvn_{d}'] = prep.tile([64, 8], F32, name=f"rvn_{d}")
    bldt = {}

    def bld_tile(d):
        if d not in bldt:
            bldt[d] = bldp.tile([128, 2, 20, 8, 64], BF16, name=f"bld_{d}",
                                tag="bld", bufs=1)
        return bldt[d]

    def units(d, q):
        di = 0 if d == 'f' else 1
        cT, cB = conT[d], conB[d]
        q0 = 16 * q
        n1sq, ctm, rvn = t[f'n1sq_{d}'], t[f'ctm_{d}'], t[f'rvn_{d}']

        def u_norm():
            csq = prep.tile([128, 2, 8, 16], F32R, name=f"csq_{d}_{q}",
                            tag="csq", bufs=3)
            nc.scalar.activation(csq[:], cT[:, :, :, q0:q0 + 16], AF.Square)
            n1q = prepps.tile([20, 4, 8, 16], F32, name=f"n1q_{d}_{q}",
                              tag="n1q", bufs=2)
            for ty in range(4):
                for c in range(2):
                    nc.tensor.matmul(n1q[:, ty, :, :],
                                     w2r[:, c, di, 20 * ty:20 * ty + 20],
                                     csq[:, c, :, :],
                                     start=(c == 0), stop=(c == 1))
            nc.vector.tensor_copy(n1sq[:, :, :, q0:q0 + 16], n1q[:])

        return [u_norm]

    def halves(d, h):
        # engine partition accesses must be 32-aligned, so the t-major
        # transposes and per-token norms go by 32-token halves
        cB = conB[d]
        h0 = 32 * h
        ctm, rvn = t[f'ctm_{d}'], t[f'rvn_{d}']

        def u_ctm(s0):
            for s in range(s0, s0 + 2):
                tp = prepps.tile([32, 2, 128], BF16, name=f"ct_{d}_{h}_{s}",
                                 tag="ctp", bufs=2)
                for c in range(2):
                    nc.tensor.transpose(tp[:, c, :], cB[:, c, s, h0:h0 + 32],
                                        idb[:])
                if s % 2 == 0:
                    nc.scalar.activation(ctm[h0:h0 + 32, s, :],
                                         tp.rearrange("t c k -> t (c k)"),
                                         AF.Copy)
                else:
                    nc.vector.tensor_copy(ctm[h0:h0 + 32, s, :],
                                          tp.rearrange("t c k -> t (c k)"))

        def u_rvn(s0):
            # bf16 out scratch keeps the STT in 4x DVE perf mode; the f32
            # accum_out (exempt scalar operand) carries the precision
            for s in range(s0, s0 + 4):
                scr = prep.tile([32, 256], BF16, name=f"rs_{d}_{h}_{s}",
                                tag="rvs", bufs=2)
                nc.vector.scalar_tensor_tensor(
                    out=scr[:], in0=ctm[h0:h0 + 32, s, :], scalar=1.0,
                    in1=ctm[h0:h0 + 32, s, :], op0=ALU.mult, op1=ALU.mult,
                    accum_out=rvn[h0:h0 + 32, s:s + 1])

        return [lambda s0=s0: u_ctm(s0) for s0 in range(0, 8, 2)] + \
               [lambda: u_rvn(0), lambda: u_rvn(4)]

    def bld_units(d, tlo, thi, mix=False):
        # mix=True (post-scan): mostly DVE — the bf16 tensor_scalar hits
        # the 4x perf mode (~190ns vs ~800ns Pool); Pool-only when
        # interleaved under the ctx scan where DVE is contended
        di = 0 if d == 'f' else 1
        cB = conB[d]
        bld = bld_tile(d)
        out = []
        for c in range(2):
            for l0 in range(0, L, 2):
                def cl(c=c, l0=l0):
                    for l in range(l0, l0 + 2):
                        eng = (nc.vector if mix and l % 4 != 3
                               else nc.gpsimd)
                        eng.tensor_scalar_mul(
                            bld[:, c, l, :, tlo:thi], cB[:, c, :, tlo:thi],
                            w2f[:, c, di, 20 + l:21 + l])
                out.append(cl)
        return out

    def finals():
        # one sqrt-table residency for all four batched Sqrts
        for d in 'fb':
            nc.scalar.activation(
                t[f'n1_{d}'].rearrange("l y s t -> l (y s t)"),
                t[f'n1sq_{d}'].rearrange("l y s t -> l (y s t)"), AF.Sqrt)
            nc.scalar.activation(t[f'rvn_{d}'][:], t[f'rvn_{d}'][:], AF.Sqrt)
        for d in 'fb':
            n1, rn1 = t[f'n1_{d}'], t[f'rn1_{d}']
            nc.vector.tensor_scalar_max(
                rn1.rearrange("l y s t -> l (y s t)"),
                n1.rearrange("l y s t -> l (y s t)"), EPS)
            nc.vector.reciprocal(rn1.rearrange("l y s t -> l (y s t)"),
                                 rn1.rearrange("l y s t -> l (y s t)"))
            rvn = t[f'rvn_{d}']
            nc.vector.tensor_scalar_max(rvn[:], rvn[:], EPS)
            nc.vector.reciprocal(rvn[:], rvn[:])

    return {'t': t, 'units': units, 'halves': halves,
            'bld_units': bld_units, 'bld_tile': bld_tile, 'finals': finals}


def _matching(nc, tc, ctx, conT, conB, w2r, w2f, w2b, mvT, idf, idb,
              prep):
    stage, n1s = {}, {}
    # feature-type offsets into w2 cols (ty*20) and mv row slots (ty*32)
    # greedy DVE/Pool balancer: Pool runs TT ~3.9x slower than DVE-2x
    # pool pre-charged: Pool's 8us products block their dependent DVE
    # tree stages, so bias assignment away from Pool (swept optimum)
    rot = {'dve': 0.0, 'pool': 30.0}

    def veng(cost=1.0):
        if rot['dve'] + cost <= rot['pool'] + 3.3 * cost:
            rot['dve'] += cost
            return nc.vector
        rot['pool'] += 3.3 * cost
        return nc.gpsimd

    dramp = ctx.enter_context(tc.tile_pool(name="mdram", bufs=1,
                                           space="DRAM"))

    def mcol(mt, slot, ri, b):
        # (20, 64) view of mvT rows [slot:slot+20], cols 8t + ri*4 + b
        return mt[slot:slot + 20, :].rearrange("l (t s) -> l t s",
                                               s=8)[:, :, ri * BL + b]

    for di, d in enumerate('fb'):
        cT, cB = conT[d], conB[d]
        anchor_t = (T - 1) if d == 'f' else 0
        mt = mvT[di]
        n1 = prep['t'][f'n1_{d}']
        rn1 = prep['t'][f'rn1_{d}']
        ctm = prep['t'][f'ctm_{d}']
        rvn = prep['t'][f'rvn_{d}']

        with tc.tile_pool(name=f"mn_{d}", bufs=1) as mn:
          with tc.tile_pool(name=f"mnp_{d}", bufs=2, space="PSUM") as mnp:
            n1s[d] = n1
            # ---- FULL
            ancv = mn.tile([128, 2, 8], F32, name=f"ancv_{d}", tag="ancv")
            nc.vector.tensor_copy(ancv[:], cT[:, :, :, anchor_t])
            for b in range(BL):
                for ri, (s_me, s_an) in enumerate(((b, BL + b), (BL + b, b))):
                    anc = mn.tile([128, 2, 20], BF16, name=f"an_{d}_{b}_{ri}",
                                  tag="anc", bufs=2)
                    for c in range(2):
                        nc.vector.tensor_scalar_mul(
                            anc[:, c, :], w2b[:, c, di, 0:20],
                            ancv[:, c, s_an:s_an + 1])
                    nps = mnp.tile([20, 64], F32, name=f"nf_{d}_{b}_{ri}",
                                   tag="nf")
                    for c in range(2):
                        nc.tensor.matmul(nps[:], anc[:, c, :],
                                         cB[:, c, s_me, :],
                                         start=(c == 0), stop=(c == 1))
                    den = mn.tile([20, 64], F32, name=f"de_{d}_{b}_{ri}",
                                  tag="den", bufs=2)
                    nc.vector.tensor_scalar(
                        out=den[:], in0=n1[:, 0, s_me, :],
                        scalar1=n1[:, 0, s_an, anchor_t:anchor_t + 1],
                        scalar2=EPS, op0=ALU.mult, op1=ALU.max)
                    nc.vector.reciprocal(den[:], den[:])
                    nc.vector.tensor_tensor(out=mcol(mt, 0, ri, b),
                                            in0=nps[:], in1=den[:],
                                            op=ALU.mult)

          # ---- MAX (pair max over the other sequence)
          with tc.tile_pool(name=f"mx_{d}", bufs=1) as mxp, \
               tc.tile_pool(name=f"mxps_{d}", bufs=1, space="PSUM") as mxps:
              bld = prep['bld_tile'](d)
              # stage MAX-type recip norms to DRAM (bf16) for broadcasts
              rnb = mxp.tile([20, 8, 64], BF16, name=f"rnb_{d}", tag="rnb")
              nc.vector.tensor_copy(rnb.rearrange("l s t -> l (s t)"),
                                    rn1[:, 1, :, :].rearrange(
                                        "l s t -> l (s t)"))
              rnd = dramp.tile([20, 8, 64], BF16, name=f"rnd_{d}")
              nc.sync.dma_start(rnd[:], rnb[:])
              mxs_all = {}
              for bp in range(2):
                  for side in range(2):
                      rs_me = 2 * bp if side == 0 else 4 + 2 * bp
                      rs_ot = 4 + 2 * bp if side == 0 else 2 * bp
                      for hf in range(2):
                          pps = mxps.tile([128, 10, 128], F32,
                                          name=f"pp_{d}_{bp}_{side}_{hf}",
                                          tag="pps", bufs=2)
                          for u in range(10):
                              l = 10 * hf + u
                              for c in range(2):
                                  nc.tensor.matmul(
                                      pps[:, u, :],
                                      bld[:, c, l, rs_me:rs_me + 2,
                                          :].rearrange("k e t -> k (e t)"),
                                      cB[:, c, rs_ot:rs_ot + 2,
                                         :].rearrange("k e t -> k (e t)"),
                                      start=(c == 0), stop=(c == 1))
                          for b2 in range(2):
                              b = 2 * bp + b2
                              s_ot = rs_ot + b2
                              key = (side, b)
                              if key not in mxs_all:
                                  mxs_all[key] = mxp.tile(
                                      [64, 2, 10], F32,
                                      name=f"mxs_{d}_{side}_{b}",
                                      tag=f"mxs_{side}_{b2}")
                              nbcb = mxp.tile([64, 10, 64], BF16,
                                              name=f"nb_{d}_{bp}_{side}"
                                                   f"_{hf}_{b2}",
                                              tag="nbcb", bufs=3)
                              nc.sync.dma_start(
                                  nbcb[:],
                                  bass.AP(tensor=rnd.tensor,
                                          offset=rnd.offset
                                          + (10 * hf) * 512 + s_ot * 64,
                                          ap=[[0, 64], [512, 10], [1, 64]]))
                              # stage pps to SBUF bf16 on the idle ACT so
                              # the multiply runs 2x from SBUF instead of
                              # 1x from f32 psum (791ns -> ~390ns on DVE)
                              ppsc = mxp.tile([64, 10, 64], BF16,
                                              name=f"pc_{d}_{bp}_{side}"
                                                   f"_{hf}_{b2}",
                                              tag="ppsc", bufs=3)
                              nc.scalar.activation(
                                  ppsc[:],
                                  pps[64 * b2:64 * b2 + 64, :,
                                      64 * b2:64 * b2 + 64], AF.Copy)
                              pn = mxp.tile([64, 10, 64], BF16,
                                            name=f"pn_{d}_{bp}_{side}"
                                                 f"_{hf}_{b2}",
                                            tag="pn", bufs=3)
                              rot['dve'] += 0.4
                              nc.vector.tensor_tensor(
                                  out=pn[:], in0=ppsc[:],
                                  in1=nbcb[:], op=ALU.mult)
                              # bf16 TT tree-max (2x DVE) beats the 1x
                              # tensor_reduce on 640-elem tiles
                              cur = pn
                              for w in (32, 16, 8, 4, 2):
                                  nxt = mxp.tile(
                                      [64, 10, w], BF16,
                                      name=f"mt_{d}_{bp}_{side}"
                                           f"_{hf}_{b2}_{w}",
                                      tag=f"mt{w}", bufs=2)
                                  nc.vector.tensor_tensor(
                                      out=nxt[:], in0=cur[:, :, 0:w],
                                      in1=cur[:, :, w:2 * w], op=ALU.max)
                                  cur = nxt
                              nc.vector.tensor_tensor(
                                  out=mxs_all[key][:, hf, :],
                                  in0=cur[:, :, 0:1].rearrange(
                                      "t u o -> t (u o)"),
                                  in1=cur[:, :, 1:2].rearrange(
                                      "t u o -> t (u o)"),
                                  op=ALU.max)
              for side in range(2):
                  for b in range(BL):
                      yt = mxps.tile([20, 64], F32,
                                     name=f"yt_{d}_{b}_{side}", tag="yt",
                                     bufs=2)
                      nc.tensor.transpose(
                          yt[:],
                          mxs_all[(side, b)].rearrange(
                              "t hf u -> t (hf u)"),
                          idf[0:64, 0:64])
                      ri_me = 0 if side == 0 else 1
                      s_me = b if side == 0 else BL + b
                      nc.vector.tensor_tensor(
                          out=mcol(mt, 32, ri_me, b), in0=yt[:],
                          in1=rn1[:, 1, s_me, :], op=ALU.mult)
          if d == 'f':
              # dir b's MAX builds now: Pool is idle while DVE chews on
              # dir f's AM blocks; the shared bld buffer (tag bufs=1)
              # WARs behind f's pps reads automatically
              for cl in prep['bld_units']('b', 0, 64, mix=True):
                  cl()
          # ---- AM + AX per batch item
          with tc.tile_pool(name=f"am_{d}", bufs=3) as amp, \
               tc.tile_pool(name=f"amps_{d}", bufs=2, space="PSUM") as amps:
              for b in range(BL):
                  _am_ax_block(nc, tc, d, di, b, cT, cB, w2b, n1, rn1,
                               rvn, ctm, mt, idf, idb, amp, amps, dramp,
                               mcol, anchor_t, veng, stage)



    return {'stage': stage, 'n1s': n1s, 'rot': rot, 'veng': veng,
            'mcol': mcol, 'dramp': dramp}


def _am_ax_block(nc, tc, d, di, b, cT, cB, w2b, n1, rn1, rvn, ctm, mt, idf,
                 idb, amp, amps, dramp, mcol, anchor_t, veng, stage):
    AM_SLOT, AX_SLOT = 64, 96
    sp, sh = b, BL + b

    # raw attention + normalization (attn = rvn_p[i] * raw * rvn_h[j])
    att_ps = amps.tile([64, 64], F32, name=f"at_{d}_{b}", tag="t64", bufs=3)
    for c in range(2):
        nc.tensor.matmul(att_ps[:], cB[:, c, sp, :], cB[:, c, sh, :],
                         start=(c == 0), stop=(c == 1))
    a1 = amp.tile([64, 64], F32, name=f"a1_{d}_{b}", tag="a1")
    nc.scalar.activation(a1[:], att_ps[:], AF.Copy, scale=rvn[:, sp:sp + 1])
    a1t_ps = amps.tile([64, 64], F32, name=f"a1t_{d}_{b}", tag="t64", bufs=3)
    nc.tensor.transpose(a1t_ps[:], a1[:], idf[0:64, 0:64])
    attTn = amp.tile([64, 64], F32, name=f"aTn_{d}_{b}", tag="attTn")
    nc.scalar.activation(attTn[:], a1t_ps[:], AF.Copy,
                         scale=rvn[:, sh:sh + 1])
    attn_ps = amps.tile([64, 64], F32, name=f"an2_{d}_{b}", tag="t64", bufs=3)
    nc.tensor.transpose(attn_ps[:], attTn[:], idf[0:64, 0:64])
    attn = amp.tile([64, 64], F32, name=f"an_{d}_{b}", tag="attn")
    nc.scalar.activation(attn[:], attn_ps[:], AF.Copy)

    # row sums + clamped recips
    rs_h = amp.tile([64, 1], F32, name=f"rh_{d}_{b}", tag="rsh")
    nc.vector.tensor_reduce(out=rs_h[:], in_=attn[:], axis=AX_X, op=ALU.add)
    nc.vector.tensor_scalar_max(rs_h[:], rs_h[:], EPS)
    nc.vector.reciprocal(rs_h[:], rs_h[:])
    rs_p = amp.tile([64, 1], F32, name=f"rp_{d}_{b}", tag="rsp")
    nc.vector.tensor_reduce(out=rs_p[:], in_=attTn[:], axis=AX_X, op=ALU.add)
    nc.vector.tensor_scalar_max(rs_p[:], rs_p[:], EPS)
    nc.vector.reciprocal(rs_p[:], rs_p[:])

    # weighted mean rhs: ahT = T(attn * rs_h) bf16, bpT = T(attTn * rs_p)
    ah = amp.tile([64, 64], F32, name=f"ah_{d}_{b}", tag="ah")
    nc.scalar.activation(ah[:], attn[:], AF.Copy, scale=rs_h[:, 0:1])
    ahT_ps = amps.tile([64, 64], F32, name=f"ahT_{d}_{b}", tag="t64", bufs=3)
    nc.tensor.transpose(ahT_ps[:], ah[:], idf[0:64, 0:64])
    ahT = amp.tile([64, 64], BF16, name=f"ahTs_{d}_{b}", tag="ahTs")
    nc.scalar.activation(ahT[:], ahT_ps[:], AF.Copy)
    bp_ = amp.tile([64, 64], F32, name=f"bp_{d}_{b}", tag="bp")
    nc.scalar.activation(bp_[:], attTn[:], AF.Copy, scale=rs_p[:, 0:1])
    bpT_ps = amps.tile([64, 64], F32, name=f"bpT_{d}_{b}", tag="t64", bufs=3)
    nc.tensor.transpose(bpT_ps[:], bp_[:], idf[0:64, 0:64])
    bpT = amp.tile([64, 64], BF16, name=f"bpTs_{d}_{b}", tag="bpTs")
    nc.scalar.activation(bpT[:], bpT_ps[:], AF.Copy)

    # am vectors + cosine under w_am
    for role, (rhs, s_ctm, s_me) in enumerate(
            ((ahT, sh, sp), (bpT, sp, sh))):
        amv_ps = amps.tile([128, 2, 64], F32, name=f"av_{d}_{b}_{role}",
                           tag="amv", bufs=2)
        for c in range(2):
            nc.tensor.matmul(amv_ps[:, c, :],
                             ctm[:, s_ctm, 128 * c:128 * (c + 1)], rhs[:],
                             start=True, stop=True)
        amv = amp.tile([128, 2, 64], F32R, name=f"am_{d}_{b}_{role}",
                       tag="amv_sb")
        nc.scalar.activation(amv.rearrange("k c t -> k (c t)"),
                             amv_ps.rearrange("k c t -> k (c t)"), AF.Copy)
        prod = amp.tile([128, 2, 64], BF16, name=f"pr_{d}_{b}_{role}",
                        tag="prod")
        for c in range(2):
            nc.vector.tensor_tensor(out=prod[:, c, :], in0=cB[:, c, s_me, :],
                                    in1=amv[:, c, :], op=ALU.mult)
        nump = amps.tile([20, 64], F32, name=f"nu_{d}_{b}_{role}", tag="s20",
                         bufs=2)
        for c in range(2):
            nc.tensor.matmul(nump[:], w2b[:, c, di, 40:60],
                             prod[:, c, :], start=(c == 0), stop=(c == 1))
        amsq = amp.tile([128, 2, 64], BF16, name=f"as_{d}_{b}_{role}",
                        tag="amsq")
        nc.scalar.activation(amsq.rearrange("k c t -> k (c t)"),
                             amv.rearrange("k c t -> k (c t)"), AF.Square)
        n2p = amps.tile([20, 64], F32, name=f"n2_{d}_{b}_{role}", tag="s20",
                        bufs=2)
        for c in range(2):
            nc.tensor.matmul(n2p[:], w2b[:, c, di, 40:60],
                             amsq[:, c, :], start=(c == 0), stop=(c == 1))
        n2s = amp.tile([20, 64], F32, name=f"ns_{d}_{b}_{role}", tag="n2s")
        nc.scalar.activation(n2s[:], n2p[:], AF.Sqrt)
        den = amp.tile([20, 64], F32, name=f"dn_{d}_{b}_{role}", tag="amden")
        nc.vector.tensor_tensor(out=den[:], in0=n1[:, 2, s_me, :],
                                in1=n2s[:], op=ALU.mult)
        nc.vector.tensor_scalar_max(den[:], den[:], EPS)
        nc.vector.reciprocal(den[:], den[:])
        nc.vector.tensor_tensor(out=mcol(mt, AM_SLOT, role, b), in0=nump[:],
                                in1=den[:], op=ALU.mult)

    # ---- stage normalized attention (bf16) to DRAM for the AX phases
    atb = amp.tile([64, 64], BF16, name=f"ab_{d}_{b}", tag="atb")
    nc.vector.tensor_copy(atb[:], attn[:])
    atbT = amp.tile([64, 64], BF16, name=f"abT_{d}_{b}", tag="atbT")
    nc.vector.tensor_copy(atbT[:], attTn[:])
    dsc = dramp.tile([64, 64], BF16, name=f"dx_{d}_{b}")
    nc.sync.dma_start(dsc[:], atb[:])
    dscT = dramp.tile([64, 64], BF16, name=f"dxT_{d}_{b}")
    nc.sync.dma_start(dscT[:], atbT[:])
    stage[(d, b)] = (dsc, dscT)



def _make_ax_emit(nc, tc, ctx, conB, n1s, w2b, mvT, mctx):
    """AX feature (max-attentive cosine), chunked by groups of 8 output
    tokens so the middle chunks interleave with the agg scan's emission.

    ax_unit(d, b, role, ch, during): products + joint bf16 tree-max +
    numerator/norm matmuls accumulated into per-(d, chunk) psum tiles.
    ax_tail(ch): ONE batched Sqrt per dir (both dirs adjacent in ACT
    program order — Sqrt lives in a different ACT table set than the
    scan's Sigmoid/Tanh, so scattering per-unit Sqrts through the scan
    would thrash 1.3us table reloads), then den/recip/feature write for
    all 8 (role, b) units of the chunk at once.

    `during=True` alternates products Pool/DVE for scan-concurrent
    execution; `during=False` uses the greedy DVE/Pool balancer.
    """
    veng = mctx['veng']
    stage, rot = mctx['stage'], mctx['rot']
    axp = ctx.enter_context(tc.tile_pool(name="axp", bufs=2))
    axps = ctx.enter_context(tc.tile_pool(name="axps", bufs=2, space="PSUM"))
    pcnt = [0]
    acc = {}

    def ax_unit(d, b, role, ch, pair, during=False):
        di = 0 if d == 'f' else 1
        i0 = 8 * ch
        cB = conB[d]
        u = role * BL + b
        if pair not in acc:
            acc[pair] = axps.tile([20, 4, 2, 8, 8], F32,
                                  name=f"acc_{pair[0]}_{pair[1]}",
                                  tag="axacc", bufs=2)
        slot = 2 * di + (0 if ch == pair[0] else 1)
        nuxc = acc[pair][:, slot, 0, :, :]
        n2c = acc[pair][:, slot, 1, :, :]
        sp, sh = b, BL + b
        dsc, dscT = stage[(d, b)]
        src = dsc if role == 0 else dscT
        s_v = sh if role == 0 else sp
        s_me = sp if role == 0 else sh
        # broadcast the staged attn rows [i0:i0+8) to all 128 partitions
        bc = axp.tile([128, 8, 64], BF16,
                      name=f"bc_{d}_{b}_{role}_{ch}", tag="bc", bufs=5)
        nc.sync.dma_start(
            bc[:], bass.AP(tensor=src.tensor, offset=src.offset + i0 * 64,
                           ap=[[0, 128], [64, 8], [1, 64]]))
        prod = axp.tile([128, 2, 8, 64], BF16,
                        name=f"xp_{d}_{b}_{role}_{ch}", tag="xprod", bufs=3)
        pcnt[0] += 1
        eng = nc.gpsimd if pcnt[0] % 3 != 0 else nc.vector
        vb = cB[:, :, s_v, :]
        eng.tensor_tensor(
            out=prod[:],
            in0=bass.AP(tensor=vb.tensor, offset=vb.offset,
                        ap=[vb.ap[0], vb.ap[1], [0, 8], vb.ap[2]]),
            in1=bass.AP(tensor=bc.tensor, offset=bc.offset,
                        ap=[bc.ap[0], [0, 2], bc.ap[1], bc.ap[2]]),
            op=ALU.mult)
        rot['dve'] += 0.62  # tree max: DVE only
        cur = prod
        for w in (32, 16, 8, 4, 2):
            nxt = axp.tile([128, 2, 8, w], BF16,
                           name=f"tm_{d}_{b}_{role}_{ch}_{w}",
                           tag=f"tm{w}", bufs=2)
            nc.vector.tensor_tensor(out=nxt[:], in0=cur[:, :, :, 0:w],
                                    in1=cur[:, :, :, w:2 * w], op=ALU.max)
            cur = nxt
        axm = axp.tile([128, 2, 8], F32R,
                       name=f"axm_{d}_{b}_{role}_{ch}", tag="axm", bufs=3)
        nc.vector.tensor_tensor(
            out=axm[:],
            in0=cur[:, :, :, 0:1].rearrange("k c t o -> k c (t o)"),
            in1=cur[:, :, :, 1:2].rearrange("k c t o -> k c (t o)"),
            op=ALU.max)
        prodx = axp.tile([128, 2, 8], BF16,
                         name=f"px_{d}_{b}_{role}_{ch}", tag="prodx", bufs=3)
        nc.vector.tensor_tensor(out=prodx[:], in0=cB[:, :, s_me, i0:i0 + 8],
                                in1=axm[:], op=ALU.mult)
        for c in range(2):
            nc.tensor.matmul(nuxc[:, u, :], w2b[:, c, di, 60:80],
                             prodx[:, c, :], start=(c == 0), stop=(c == 1))
        axsq = axp.tile([128, 2, 8], BF16,
                        name=f"xs_{d}_{b}_{role}_{ch}", tag="axsq", bufs=3)
        nc.scalar.activation(axsq.rearrange("k c t -> k (c t)"),
                             axm.rearrange("k c t -> k (c t)"), AF.Square)
        for c in range(2):
            nc.tensor.matmul(n2c[:, u, :], w2b[:, c, di, 60:80],
                             axsq[:, c, :], start=(c == 0), stop=(c == 1))

    def ax_tail(pair):
        at = acc.pop(pair)
        sq = {}
        for di, d in enumerate('fb'):
            for cpos, ch in enumerate(pair):
                n2s = axp.tile([20, 8, 8], F32, name=f"n2s_{d}_{ch}",
                               tag="n2s", bufs=4)
                nc.scalar.activation(n2s[:], at[:, 2 * di + cpos, 1, :, :],
                                     AF.Sqrt)
                sq[(d, ch)] = n2s
        for di, d in enumerate('fb'):
            n1 = n1s[d]
            for cpos, ch in enumerate(pair):
                i0 = 8 * ch
                nuxc = at[:, 2 * di + cpos, 0, :, :]
                n2s = sq[(d, ch)]
                den = axp.tile([20, 8, 8], F32, name=f"dnc_{d}_{ch}",
                               tag="denc", bufs=2)
                nc.vector.tensor_tensor(
                    out=den[:], in0=n1[:, 3, :, i0:i0 + 8],
                    in1=n2s[:], op=ALU.mult)
                nc.vector.tensor_scalar_max(
                    den.rearrange("l s t -> l (s t)"),
                    den.rearrange("l s t -> l (s t)"), EPS)
                nc.vector.reciprocal(den.rearrange("l s t -> l (s t)"),
                                     den.rearrange("l s t -> l (s t)"))
                out = mvT[di][96:116, 8 * i0:8 * i0 + 64].rearrange(
                    "l (t s) -> l t s", s=8)
                nc.vector.tensor_tensor(out=out,
                                        in0=nuxc.rearrange("l s t -> l t s"),
                                        in1=den.rearrange("l s t -> l t s"),
                                        op=ALU.mult)

    return ax_unit, ax_tail


# ---------------------------------------------------------------- entry

def _get_nc(debug=False):
    key = ('dbg' if debug else 'rel')
    if key not in _CACHE:
        _CACHE[key] = build_nc(debug)
    return _CACHE[key]


def kernel(**inputs):
    nc = _get_nc(False)
    w = _prep_weights(inputs)
    in_maps = []
    for core in range(NCORES):
        m = dict(w)
        m['tokp'] = _prep_tokens(inputs['q1_inputs'], inputs['q2_inputs'],
                                 core)
        in_maps.append(m)
    res = run_bass_kernel_spmd(nc, in_maps, core_ids=list(range(NCORES)))
    out = np.concatenate([res.results[c]['y'] for c in range(NCORES)], axis=0)
    return out.astype(np.float32)


def run_debug(inputs):
    nc = _get_nc(True)
    w = _prep_weights(inputs)
    in_maps = []
    for core in range(NCORES):
        m = dict(w)
        m['tokp'] = _prep_tokens(inputs['q1_inputs'], inputs['q2_inputs'],
                                 core)
        in_maps.append(m)
    res = run_bass_kernel_spmd(nc, in_maps, core_ids=list(range(NCORES)))
    return res

